# revision 78
# baseline (speedup 1.0000x reference)
"""Fused multi-head attention layer (RoPE + ALiBi + softmax + out-proj) on 8 TRN2 cores.

Sharding (v4, heads-split): core c -> (batch b = c//2, head group g = c%2).
Each core computes q/k/v for its 4 heads {g, 2+g, 4+g, 6+g} over ALL 2048
positions, runs banded attention, and projects through its heads' slice of
proj_w, producing a partial [N, C] output; the host sums the two partials
per batch. Pairing adjacent-radius heads per graph position keeps the SPMD
union of ALiBi bands tight, and query blocks have exact (not unioned)
trim bounds.

Pipeline features:
- RoPE rotate-half via a sign-folded sin table + partition-permutation
  matmul (no duplicate rot projections).
- ALiBi bias on PE as c8-scaled identity matmuls against a single shared
  anti-diagonal master pattern table (column-shifted AP views), trimmed to
  the band columns.
- Score/exp/attn-V column ranges prefix-trimmed per (position, block, jt).
- Streams software-pipelined; V/next-bundle projections fill PE gaps.
"""

import functools
import math
import os
import sys

import numpy as np

sys.path.insert(0, "/opt/trn_rl_repo")

import ml_dtypes  # noqa: E402

import concourse.bass as bass  # noqa: E402
import concourse.tile as tile  # noqa: E402
from concourse import bacc, mybir, bass_utils  # noqa: E402

BF16 = mybir.dt.bfloat16
F32 = mybir.dt.float32
NPBF = ml_dtypes.bfloat16

B, N, C, H, D = 4, 2048, 512, 8, 64
NCORES = 8
JT = N // 128        # 16 j-tiles of 128 key positions
NP_ = 4              # head positions per core
T_CUT = 30.0         # ALiBi cutoff in logits: exp(-30) is negligible
SCALE = D ** -0.5

# c8_h = alibi_slope_h * MAX_BIAS = 2^-(h+1) * 8 = 2^(2-h)
C8 = [2.0 ** (2 - h) for h in range(H)]
RADIUS = [T_CUT / c for c in C8]   # band reach (key positions) per head
# graph position p holds heads {2p, 2p+1}; the union band is the odd head's
UR = [RADIUS[2 * p + 1] for p in range(NP_)]

LAST_RESULT = None  # test harness reads exec_time_ns from here


def _clamp(v, lo, hi):
    return max(lo, min(hi, v))


# Frame for (position p, slot-pair sp): cols 0:512 = query block 2sp
# (i0 = 1024sp), cols 512:1024 = block 2sp+1 (i0 = 1024sp+512). Exact bounds.
def _qm(p, sp, sl, jt):
    i0 = 512 * (2 * sp + sl)
    return _clamp(int(math.floor(128 * jt + 127 + UR[p])) + 1 - i0, 0, 512)


QM = [[[[_qm(p, sp, sl, jt) for jt in range(JT)] for sl in range(2)]
       for sp in range(2)] for p in range(NP_)]
QLIM = [[[(QM[p][sp][0][jt] if QM[p][sp][0][jt] < 512
           else 512 + QM[p][sp][1][jt]) for jt in range(JT)]
         for sp in range(2)] for p in range(NP_)]
JTMIN = [[min(jt for jt in range(JT) if QLIM[p][sp][jt] > 0)
          for sp in range(2)] for p in range(NP_)]
LAST1 = [[min((jt for jt in range(JT) if QM[p][sp][1][jt] > 0), default=None)
          for sp in range(2)] for p in range(NP_)]


def _bias_range(p, sp, jt):
    qm0, qm1 = QM[p][sp][0][jt], QM[p][sp][1][jt]
    bs0 = max(0, 128 * jt - 1024 * sp + 1)
    bs1 = max(0, 128 * jt - 1024 * sp - 511)
    r = []
    if bs0 < qm0:
        r.append((bs0, qm0))
    if bs1 < qm1:
        r.append((512 + bs1, 512 + qm1))
    if len(r) == 2:
        assert r[0][1] == 512 and r[1][0] == 512, (p, sp, jt, r)
        r = [(r[0][0], r[1][1])]
    return r[0] if r else None


BIASR = [[[_bias_range(p, sp, jt) for jt in range(JT)] for sp in range(2)]
         for p in range(NP_)]


def _heads(g):
    return [2 * p + g for p in range(NP_)]


def _rope_tables():
    inv = 1.0 / (10000.0 ** (np.arange(0, D, 2, dtype=np.float32) / D))
    f = np.arange(N, dtype=np.float32)[:, None] * inv[None, :]
    sin = np.concatenate([np.sin(f), np.sin(f)], axis=-1).astype(np.float32)
    cos = np.concatenate([np.cos(f), np.cos(f)], axis=-1).astype(np.float32)
    return sin, cos  # [N, D]


def _st_table(sin):
    # sign-folded, half-swapped sin table, indexed by SOURCE row r: after the
    # XOR-32 partition permutation, dest row d gets rot_half(q)[d]*sin[d].
    st = np.empty_like(sin)            # [N, D]
    st[:, 0:32] = sin[:, 32:64]
    st[:, 32:64] = -sin[:, 0:32]
    return st


def _shared_inputs(qkv_w, proj_w, proj_b):
    # [I | P32]: P32 is the XOR-32 partition permutation (within 64-blocks)
    shifteye = np.zeros((128, 256), np.float32)
    shifteye[:, 0:128] = np.eye(128)
    for r in range(128):
        shifteye[r, 128 + (r ^ 32)] = 1.0

    # master ALiBi pattern: master[k, x] = min(k - x, 0); the tile for
    # (jt, block i0) is the column window shifted by o = i0 - 128*jt
    jl = np.arange(128, dtype=np.float32)[:, None]
    xl = np.arange(2048, dtype=np.float32)[None, :]
    master = np.minimum(jl - xl, 0.0).astype(NPBF)

    sin, cos = _rope_tables()
    cos2 = np.tile(cos.T, (2, 1)).astype(NPBF)    # [128, N]
    ssin2 = np.tile(_st_table(sin).T, (2, 1)).astype(NPBF)
    return {
        "shifteye": shifteye.astype(NPBF),
        "master": master,
        "cos2": cos2, "ssin2": ssin2,
    }, qkv_w, (proj_w, proj_b)


def _core_inputs(c, x, shared, qkv_w, proj):
    proj_w, proj_b = proj
    b, g = c // 2, c % 2
    heads = _heads(g)
    cols = np.concatenate([np.arange(64 * h, 64 * h + 64) for h in heads])

    wqT = np.ascontiguousarray(qkv_w[0:C].T)[:, cols] * SCALE     # [C, 256]
    wkT = np.ascontiguousarray(qkv_w[C:2 * C].T)[:, cols]
    wvT = np.ascontiguousarray(qkv_w[2 * C:3 * C].T)[:, cols]
    wcat = np.concatenate([wqT, wkT, wvT], axis=1).astype(NPBF)   # [C, 768]

    c8eye = np.zeros((NP_, 128, 128), np.float32)
    for p in range(NP_):
        np.fill_diagonal(c8eye[p], C8[heads[p]])

    projwt = np.ascontiguousarray(proj_w.T)[cols, :]              # [256, C]
    biasb = np.tile(proj_b[None, :], (128, 1)) if g == 0 else \
        np.zeros((128, C), np.float32)

    return {
        "xt": np.ascontiguousarray(x[b].T).astype(NPBF),          # [C, N]
        "wcat": wcat,
        "c8eye": c8eye.astype(NPBF),
        "projwt": projwt.astype(NPBF),
        "biasb": biasb.astype(np.float32),
        **shared,
    }


def _build_graph():
    nc = bacc.Bacc("TRN2", target_bir_lowering=False, debug=False,
                   num_devices=NCORES)

    xt_d = nc.dram_tensor("xt", [C, N], BF16, kind="ExternalInput").ap()
    wcat_d = nc.dram_tensor("wcat", [C, 768], BF16, kind="ExternalInput").ap()
    cos2_d = nc.dram_tensor("cos2", [128, N], BF16, kind="ExternalInput").ap()
    ssin2_d = nc.dram_tensor("ssin2", [128, N], BF16, kind="ExternalInput").ap()
    shifteye_d = nc.dram_tensor("shifteye", [128, 256], BF16, kind="ExternalInput").ap()
    c8eye_d = nc.dram_tensor("c8eye", [NP_, 128, 128], BF16, kind="ExternalInput").ap()
    master_d = nc.dram_tensor("master", [128, 2048], BF16, kind="ExternalInput").ap()
    projwt_d = nc.dram_tensor("projwt", [256, C], BF16, kind="ExternalInput").ap()
    biasb_d = nc.dram_tensor("biasb", [128, C], F32, kind="ExternalInput").ap()
    out_d = nc.dram_tensor("out", [N, C], F32, kind="ExternalOutput").ap()

    with tile.TileContext(nc) as tc:
        _body(nc, tc, xt_d, wcat_d, cos2_d, ssin2_d, shifteye_d, c8eye_d,
              master_d, projwt_d, biasb_d, out_d)
    nc.compile()
    return nc


def _body(nc, tc, xt_d, wcat_d, cos2_d, ssin2_d, shifteye_d, c8eye_d,
          master_d, projwt_d, biasb_d, out_d):
    from contextlib import ExitStack
    ctx = ExitStack()
    persist = ctx.enter_context(tc.tile_pool(name="persist", bufs=1))
    rope_pool = ctx.enter_context(tc.tile_pool(name="rope", bufs=2))
    exp_pool = ctx.enter_context(tc.tile_pool(name="exp", bufs=3))
    norm_pool = ctx.enter_context(tc.tile_pool(name="norm", bufs=2))
    fin_pool = ctx.enter_context(tc.tile_pool(name="final", bufs=4))
    pspool = ctx.enter_context(tc.tile_pool(name="ps", bufs=2, space="PSUM"))

    def ptile(shape, dtype, tag):
        return persist.tile(shape, dtype, tag=tag, name=tag)

    Exp = mybir.ActivationFunctionType.Exp

    # ---- persistent SBUF tiles ----
    w_sb = [ptile([128, 768], BF16, f"w{i}") for i in range(4)]
    xt_sb = [ptile([128, N], BF16, f"xt{i}") for i in range(4)]
    cos2 = ptile([128, N], BF16, "cos2")
    ssin2 = ptile([128, N], BF16, "ssin2")
    shifteye = ptile([128, 256], BF16, "shifteye")
    c8eye_sb = [ptile([128, 128], BF16, f"c8e{p}") for p in range(NP_)]
    master = ptile([128, 2048], BF16, "master")
    projw_sb = [ptile([128, C], BF16, f"pw{bd}") for bd in range(2)]
    biasb = ptile([128, C], F32, "biasb")
    q2_sb = [ptile([128, N], BF16, f"q2_{bd}") for bd in range(2)]
    k2_sb = [ptile([128, N], BF16, f"k2_{bd}") for bd in range(2)]
    v_sb = [ptile([128, NP_ * 65], BF16, f"v_{nt}") for nt in range(JT)]
    out_pair = [ptile([128, N], BF16, f"op_{bd}") for bd in range(2)]

    # ---- input DMAs, ordered to feed the PE emission order below ----
    # 1) V weights interleaved with the high xt columns so the first V
    # matmul starts after two transfers (V tiles run jt=15..0)
    for i in range(4):
        nc.sync.dma_start(w_sb[i][:, 512:768],
                          wcat_d[i * 128:(i + 1) * 128, 512:768])
        nc.sync.dma_start(xt_sb[i][:, 1536:2048],
                          xt_d[i * 128:(i + 1) * 128, 1536:2048])
    # 2) q/k weights + rope tables
    for i in range(4):
        nc.sync.dma_start(w_sb[i][:, 0:512], wcat_d[i * 128:(i + 1) * 128, 0:512])
    nc.sync.dma_start(shifteye[:], shifteye_d[:])
    nc.sync.dma_start(cos2[:], cos2_d[:])
    nc.sync.dma_start(ssin2[:], ssin2_d[:])
    # 3) remaining xt (descending), bias tables, proj weights
    for blk in (2, 1, 0):
        for i in range(4):
            nc.sync.dma_start(xt_sb[i][:, blk * 512:(blk + 1) * 512],
                              xt_d[i * 128:(i + 1) * 128, blk * 512:(blk + 1) * 512])
    nc.sync.dma_start(master[:], master_d[:])
    for p in range(NP_):
        nc.sync.dma_start(c8eye_sb[p][:], c8eye_d[p])
    for bd in range(2):
        nc.sync.dma_start(projw_sb[bd][:], projwt_d[bd * 128:(bd + 1) * 128, :])
    nc.sync.dma_start(biasb[:], biasb_d[:])

    # ---- helpers ----
    def v_tile(jt):
        psv = pspool.tile([128, 256], F32, tag="aux", name="psv")
        for ci in range(4):
            nc.tensor.matmul(
                psv[:], xt_sb[ci][:, jt * 128:(jt + 1) * 128],
                w_sb[ci][:, 512:768],
                start=(ci == 0), stop=(ci == 3))
        vdst = v_sb[jt].rearrange("p (h e) -> p h e", e=65)
        nc.vector.tensor_copy(vdst[:, :, 0:64],
                              psv.rearrange("p (h e) -> p h e", e=64))
        nc.gpsimd.memset(vdst[:, :, 64:65], 1.0)

    def qk_chunk_a(bd, kind, ch):
        # projection matmuls + cos/sin products for one 512-token chunk of
        # bundle bd (positions 2bd, 2bd+1), kind 0=q (scaled) 1=k.
        w_off = kind * 256 + bd * 128
        c0 = ch * 512
        ps_q = pspool.tile([128, 512], F32, tag="aux", name="ps_q")
        for ci in range(4):
            nc.tensor.matmul(
                ps_q[:],
                w_sb[ci][:, w_off:w_off + 128],
                xt_sb[ci][:, c0:c0 + 512],
                start=(ci == 0), stop=(ci == 3))
        tc_c = rope_pool.tile([128, 512], BF16, tag="tc", name="tc_c")
        nc.vector.tensor_mul(tc_c[:], ps_q[:], cos2[:, c0:c0 + 512])
        tc_u = rope_pool.tile([128, 512], BF16, tag="tu", name="tc_u")
        nc.vector.tensor_mul(tc_u[:], ps_q[:], ssin2[:, c0:c0 + 512])
        return tc_c, tc_u

    def qk_chunk_b(bd, kind, ch, tc_c, tc_u):
        # combine: dst = tc_c + P32 @ tc_u (partition-XOR-32 via matmul)
        dst_sb = k2_sb[bd] if kind else q2_sb[bd]
        c0 = ch * 512
        ps2 = pspool.tile([128, 512], F32, tag="aux", name="ps2")
        nc.tensor.matmul(ps2[:], shifteye[:, 0:128], tc_c[:],
                         start=True, stop=False)
        nc.tensor.matmul(ps2[:], shifteye[:, 128:256], tc_u[:],
                         start=False, stop=True)
        nc.vector.tensor_copy(dst_sb[:, c0:c0 + 512], ps2[:])

    # software-pipelined chunk list -> closures (B of chunk i rides with
    # A of chunk i+1 so the PE never waits on the DVE products)
    def chunk_closures(chunks):
        state = {}

        def make(i, spec):
            def run():
                if i > 0:
                    pb, pkd, pch = chunks[i - 1]
                    qk_chunk_b(pb, pkd, pch, *state.pop(i - 1))
                if spec is not None:
                    bd, kd, ch = spec
                    state[i] = qk_chunk_a(bd, kd, ch)
            return run

        return [make(i, spec)
                for i, spec in enumerate(list(chunks) + [None])]

    def bundle_chunks(bd):
        return [(bd, 1, 3), (bd, 1, 2), (bd, 1, 1), (bd, 1, 0),
                (bd, 0, 0), (bd, 0, 1), (bd, 0, 2), (bd, 0, 3)]

    # ---- PE pre-phase: V tiles (desc) interleaved with bundle-0 qk ----
    qk0 = chunk_closures(bundle_chunks(0))
    vt = [lambda jt=jt: v_tile(jt) for jt in range(JT - 1, -1, -1)]
    pre = [vt[0], vt[1], qk0[0], vt[2], vt[3], qk0[1], vt[4], vt[5], qk0[2],
           vt[6], vt[7], qk0[3], vt[8], vt[9], qk0[4], vt[10], vt[11],
           qk0[5], vt[12], vt[13], qk0[6], vt[14], vt[15]]
    for f in pre:
        f()

    # fillers: project bundle 1 while streaming the first two bundle-0 heads;
    # chunks 0..7 of the output projection (whose inputs complete with the
    # sp=0 streams) run inside the sp=1 streams, spreading the output DMAs.
    qk1 = chunk_closures(bundle_chunks(1))
    fillers = {(0, 0): qk1[:5], (0, 1): qk1[5:],
               (0, 2): [qk0[7], qk0[8]],
               (1, 1): [lambda chk=chk: proj_chunk(chk) for chk in range(0, 4)],
               (1, 2): [lambda chk=chk: proj_chunk(chk) for chk in range(4, 8)]}

    def proj_chunk(chk):
        # full output projection of one 128-query chunk (own heads' partial)
        ps = pspool.tile([128, 512], F32, tag="aux", name="ps_proj")
        for bd in range(2):
            nc.tensor.matmul(ps[:], out_pair[bd][:, chk * 128:(chk + 1) * 128],
                             projw_sb[bd][:],
                             start=(bd == 0), stop=(bd == 1),
                             skip_group_check=True)
        fin = fin_pool.tile([128, 512], F32, tag="f", name="fin")
        nc.vector.tensor_add(fin[:], ps[:], biasb[:])
        nc.sync.dma_start(out_d[chk * 128:(chk + 1) * 128, :], fin[:])

    def norm_slot(sp, p, av, sl, direct=False):
        # one quick PSUM->SBUF copy releases the av tile; the actual
        # normalize (reciprocal/broadcast/mul) runs later off the staged
        # copy. The final stream normalizes straight out of PSUM (nothing
        # reuses its av tile) to shorten the chain gating the last chunks.
        bd, row = p // 2, (p % 2) * 64
        base = sl * 512
        if direct:
            stg = av[:, base:base + 512]
        else:
            stg = norm_pool.tile([65, 512], F32, tag="st", name="stg")
            nc.vector.tensor_copy(stg[:], av[0:65, base:base + 512])
        rec = norm_pool.tile([1, 512], F32, tag="rc", name="rec")
        nc.vector.reciprocal(rec[:], stg[64:65, :])
        bc = norm_pool.tile([64, 512], F32, tag="bc", name="bc")
        nc.gpsimd.partition_broadcast(bc[:], rec[:])
        nc.vector.tensor_mul(
            out_pair[bd][row:row + 64, 1024 * sp + base:1024 * sp + base + 512],
            stg[0:64, :], bc[:])

    # ---- attention streams: one per (slot-pair sp, position p) ----
    for sp in range(2):
        for p in range(NP_):
            bd, row = p // 2, (p % 2) * 64
            fl = fillers.get((sp, p), [])
            fi = 0
            jts = list(range(JT - 1, JTMIN[p][sp] - 1, -1))
            av = pspool.tile([65, 1024], F32, tag="av", bufs=1, name="av")
            pend = None  # (jt, av-mm closure), delayed one step
            for idx, jt in enumerate(jts):
                while fi < len(fl) and fi * len(jts) <= idx * len(fl):
                    fl[fi]()
                    fi += 1
                qm0, qm1, ql = (QM[p][sp][0][jt], QM[p][sp][1][jt],
                                QLIM[p][sp][jt])
                br = BIASR[p][sp][jt]
                ps = pspool.tile([128, 1024], F32, tag="sc", name="ps_sc")
                nc.tensor.matmul(
                    ps[:, 0:qm0],
                    k2_sb[bd][row:row + 64, jt * 128:(jt + 1) * 128],
                    q2_sb[bd][row:row + 64, 1024 * sp:1024 * sp + qm0],
                    start=True, stop=(br is None), tile_position=(row, 0),
                    skip_group_check=True)
                if qm1 > 0:
                    nc.tensor.matmul(
                        ps[:, 512:512 + qm1],
                        k2_sb[bd][row:row + 64, jt * 128:(jt + 1) * 128],
                        q2_sb[bd][row:row + 64,
                                  1024 * sp + 512:1024 * sp + 512 + qm1],
                        start=True, stop=(br is None), tile_position=(row, 0),
                        skip_group_check=True)
                if br is not None:
                    o = 1024 * sp - 128 * jt
                    # split at the 512-col PSUM bank boundary
                    for lo, hi in ((br[0], min(br[1], 512)),
                                   (max(br[0], 512), br[1])):
                        if lo < hi:
                            nc.tensor.matmul(
                                ps[:, lo:hi], c8eye_sb[p][:],
                                master[:, lo + o:hi + o],
                                start=False, stop=True, tile_position=(0, 0),
                                skip_group_check=True)
                et = exp_pool.tile([128, 1024], BF16, tag="e", name="et")
                nc.scalar.activation(et[:, 0:ql], ps[:, 0:ql], Exp)

                def av_mms(jt=jt, qm0=qm0, qm1=qm1, et=et):
                    nc.tensor.matmul(
                        av[0:65, 0:qm0], v_sb[jt][:, p * 65:(p + 1) * 65],
                        et[:, 0:qm0],
                        start=(jt == JT - 1), stop=(jt == jts[-1]),
                        skip_group_check=True)
                    if qm1 > 0:
                        nc.tensor.matmul(
                            av[0:65, 512:512 + qm1],
                            v_sb[jt][:, p * 65:(p + 1) * 65],
                            et[:, 512:512 + qm1],
                            start=(jt == JT - 1), stop=(jt == LAST1[p][sp]),
                            skip_group_check=True)

                if pend is not None:
                    pjt, pfn = pend
                    pfn()
                    if pjt == LAST1[p][sp]:
                        norm_slot(sp, p, av, 1)
                pend = (jt, av_mms)
            pjt, pfn = pend
            pfn()
            if pjt == LAST1[p][sp]:
                norm_slot(sp, p, av, 1)
            while fi < len(fl):
                fl[fi]()
                fi += 1
            norm_slot(sp, p, av, 0, direct=(sp == 1 and p == NP_ - 1))

    # ---- remaining output projection chunks (sp=1 queries) ----
    # software-pipelined: each chunk's bundle-0 partial runs during the final
    # norm drain; only the bundle-1 matmul waits on the last normalize.
    def proj_pre(chk):
        ps = pspool.tile([128, 512], F32, tag="aux", name="ps_proj")
        nc.tensor.matmul(ps[:], out_pair[0][:, chk * 128:(chk + 1) * 128],
                         projw_sb[0][:],
                         start=True, stop=False, skip_group_check=True)
        return ps

    def proj_fin(chk, ps):
        nc.tensor.matmul(ps[:], out_pair[1][:, chk * 128:(chk + 1) * 128],
                         projw_sb[1][:],
                         start=False, stop=True, skip_group_check=True)
        fin = fin_pool.tile([128, 512], F32, tag="f", name="fin")
        nc.vector.tensor_add(fin[:], ps[:], biasb[:])
        nc.sync.dma_start(out_d[chk * 128:(chk + 1) * 128, :], fin[:])

    # slot1 chunks (12..15) are unblocked at the last stream's midpoint;
    # slot0 chunks (8..11) wait for the final normalize, so run them last.
    order = [12, 13, 14, 15, 8, 9, 10, 11]
    pend_ps = {order[0]: proj_pre(order[0]), order[1]: proj_pre(order[1])}
    for i, chk in enumerate(order):
        proj_fin(chk, pend_ps.pop(chk))
        if i + 2 < len(order):
            pend_ps[order[i + 2]] = proj_pre(order[i + 2])

    ctx.close()


@functools.lru_cache(maxsize=1)
def _graph():
    return _build_graph()


def kernel(x, qkv_w, proj_w, proj_b):
    global LAST_RESULT
    x = np.asarray(x, np.float32)
    qkv_w = np.asarray(qkv_w, np.float32)
    proj_w = np.asarray(proj_w, np.float32)
    proj_b = np.asarray(proj_b, np.float32)

    nc = _graph()
    shared, qw, proj = _shared_inputs(qkv_w, proj_w, proj_b)
    in_maps = [_core_inputs(c, x, shared, qw, proj) for c in range(NCORES)]
    res = bass_utils.run_bass_kernel_spmd(nc, in_maps,
                                          core_ids=list(range(NCORES)),
                                          trace=False)
    LAST_RESULT = res
    out = np.zeros((B, N, C), np.float32)
    for b in range(B):
        out[b] = (np.asarray(res.results[2 * b]["out"], np.float32)
                  + np.asarray(res.results[2 * b + 1]["out"], np.float32))
    return out


# revision 81
# speedup vs baseline: 1.0274x; 1.0274x over previous
"""Fused multi-head attention layer (RoPE + ALiBi + softmax + out-proj) on 8 TRN2 cores.

Sharding (v4, heads-split): core c -> (batch b = c//2, head group g = c%2).
Each core computes q/k/v for its 4 heads {g, 2+g, 4+g, 6+g} over ALL 2048
positions, runs banded attention, and projects through its heads' slice of
proj_w, producing a partial [N, C] output; the host sums the two partials
per batch. Pairing adjacent-radius heads per graph position keeps the SPMD
union of ALiBi bands tight, and query blocks have exact (not unioned)
trim bounds.

Pipeline features:
- RoPE rotate-half via a sign-folded sin table + partition-permutation
  matmul (no duplicate rot projections).
- ALiBi bias on PE as c8-scaled identity matmuls against a single shared
  anti-diagonal master pattern table (column-shifted AP views), trimmed to
  the band columns.
- Score/exp/attn-V column ranges prefix-trimmed per (position, block, jt).
- Streams software-pipelined; V/next-bundle projections fill PE gaps.
"""

import functools
import math
import os
import sys

import numpy as np

sys.path.insert(0, "/opt/trn_rl_repo")

import ml_dtypes  # noqa: E402

import concourse.bass as bass  # noqa: E402
import concourse.tile as tile  # noqa: E402
from concourse import bacc, mybir, bass_utils  # noqa: E402

BF16 = mybir.dt.bfloat16
F32 = mybir.dt.float32
NPBF = ml_dtypes.bfloat16

B, N, C, H, D = 4, 2048, 512, 8, 64
NCORES = 8
JT = N // 128        # 16 j-tiles of 128 key positions
NP_ = 4              # head positions per core
T_CUT = 30.0         # ALiBi cutoff in logits: exp(-30) is negligible
SCALE = D ** -0.5

# c8_h = alibi_slope_h * MAX_BIAS = 2^-(h+1) * 8 = 2^(2-h)
C8 = [2.0 ** (2 - h) for h in range(H)]
RADIUS = [T_CUT / c for c in C8]   # band reach (key positions) per head
# graph position p holds heads {2p, 2p+1}; the union band is the odd head's
UR = [RADIUS[2 * p + 1] for p in range(NP_)]

LAST_RESULT = None  # test harness reads exec_time_ns from here


def _clamp(v, lo, hi):
    return max(lo, min(hi, v))


# Frame for (position p, slot-pair sp): cols 0:512 = query block 2sp
# (i0 = 1024sp), cols 512:1024 = block 2sp+1 (i0 = 1024sp+512). Exact bounds.
def _qm(p, sp, sl, jt):
    i0 = 512 * (2 * sp + sl)
    return _clamp(int(math.floor(128 * jt + 127 + UR[p])) + 1 - i0, 0, 512)


QM = [[[[_qm(p, sp, sl, jt) for jt in range(JT)] for sl in range(2)]
       for sp in range(2)] for p in range(NP_)]
QLIM = [[[(QM[p][sp][0][jt] if QM[p][sp][0][jt] < 512
           else 512 + QM[p][sp][1][jt]) for jt in range(JT)]
         for sp in range(2)] for p in range(NP_)]
JTMIN = [[min(jt for jt in range(JT) if QLIM[p][sp][jt] > 0)
          for sp in range(2)] for p in range(NP_)]
# per 128-query subtile qi of the 1024-col frame: the last (smallest) jt
# whose kept prefix still reaches it (attn-V accumulation stop point)
STOPJT = [[[min(jt for jt in range(JT) if QLIM[p][sp][jt] > 128 * qi)
            for qi in range(8)] for sp in range(2)] for p in range(NP_)]


def _bias_range(p, sp, jt):
    qm0, qm1 = QM[p][sp][0][jt], QM[p][sp][1][jt]
    bs0 = max(0, 128 * jt - 1024 * sp + 1)
    bs1 = max(0, 128 * jt - 1024 * sp - 511)
    r = []
    if bs0 < qm0:
        r.append((bs0, qm0))
    if bs1 < qm1:
        r.append((512 + bs1, 512 + qm1))
    if len(r) == 2:
        assert r[0][1] == 512 and r[1][0] == 512, (p, sp, jt, r)
        r = [(r[0][0], r[1][1])]
    return r[0] if r else None


BIASR = [[[_bias_range(p, sp, jt) for jt in range(JT)] for sp in range(2)]
         for p in range(NP_)]


def _heads(g):
    return [2 * p + g for p in range(NP_)]


def _rope_tables():
    inv = 1.0 / (10000.0 ** (np.arange(0, D, 2, dtype=np.float32) / D))
    f = np.arange(N, dtype=np.float32)[:, None] * inv[None, :]
    sin = np.concatenate([np.sin(f), np.sin(f)], axis=-1).astype(np.float32)
    cos = np.concatenate([np.cos(f), np.cos(f)], axis=-1).astype(np.float32)
    return sin, cos  # [N, D]


def _st_table(sin):
    # sign-folded, half-swapped sin table, indexed by SOURCE row r: after the
    # XOR-32 partition permutation, dest row d gets rot_half(q)[d]*sin[d].
    st = np.empty_like(sin)            # [N, D]
    st[:, 0:32] = sin[:, 32:64]
    st[:, 32:64] = -sin[:, 0:32]
    return st


def _shared_inputs(qkv_w, proj_w, proj_b):
    # [I | P32]: P32 is the XOR-32 partition permutation (within 64-blocks)
    shifteye = np.zeros((128, 256), np.float32)
    shifteye[:, 0:128] = np.eye(128)
    for r in range(128):
        shifteye[r, 128 + (r ^ 32)] = 1.0

    # master ALiBi pattern: master[k, x] = min(k - x, 0); the tile for
    # (jt, block i0) is the column window shifted by o = i0 - 128*jt
    jl = np.arange(128, dtype=np.float32)[:, None]
    xl = np.arange(2048, dtype=np.float32)[None, :]
    master = np.minimum(jl - xl, 0.0).astype(NPBF)

    sin, cos = _rope_tables()
    cos2 = np.tile(cos.T, (2, 1)).astype(NPBF)    # [128, N]
    ssin2 = np.tile(_st_table(sin).T, (2, 1)).astype(NPBF)
    return {
        "shifteye": shifteye.astype(NPBF),
        "master": master,
        "cos2": cos2, "ssin2": ssin2,
    }, qkv_w, (proj_w, proj_b)


def _core_inputs(c, x, shared, qkv_w, proj):
    proj_w, proj_b = proj
    b, g = c // 2, c % 2
    heads = _heads(g)
    cols = np.concatenate([np.arange(64 * h, 64 * h + 64) for h in heads])

    wqT = np.ascontiguousarray(qkv_w[0:C].T)[:, cols] * SCALE     # [C, 256]
    wkT = np.ascontiguousarray(qkv_w[C:2 * C].T)[:, cols]
    wvT = np.ascontiguousarray(qkv_w[2 * C:3 * C].T)[:, cols]
    wcat = np.concatenate([wqT, wkT, wvT], axis=1).astype(NPBF)   # [C, 768]

    c8eye = np.zeros((NP_, 128, 128), np.float32)
    for p in range(NP_):
        np.fill_diagonal(c8eye[p], C8[heads[p]])

    projwt = np.ascontiguousarray(proj_w.T)[cols, :]              # [256, C]
    biasb = np.tile(proj_b[None, :], (128, 1)) if g == 0 else \
        np.zeros((128, C), np.float32)

    return {
        "xt": np.ascontiguousarray(x[b].T).astype(NPBF),          # [C, N]
        "wcat": wcat,
        "c8eye": c8eye.astype(NPBF),
        "projwt": projwt.astype(NPBF),
        "biasb": biasb.astype(np.float32),
        **shared,
    }


def _build_graph():
    nc = bacc.Bacc("TRN2", target_bir_lowering=False, debug=False,
                   num_devices=NCORES)

    xt_d = nc.dram_tensor("xt", [C, N], BF16, kind="ExternalInput").ap()
    wcat_d = nc.dram_tensor("wcat", [C, 768], BF16, kind="ExternalInput").ap()
    cos2_d = nc.dram_tensor("cos2", [128, N], BF16, kind="ExternalInput").ap()
    ssin2_d = nc.dram_tensor("ssin2", [128, N], BF16, kind="ExternalInput").ap()
    shifteye_d = nc.dram_tensor("shifteye", [128, 256], BF16, kind="ExternalInput").ap()
    c8eye_d = nc.dram_tensor("c8eye", [NP_, 128, 128], BF16, kind="ExternalInput").ap()
    master_d = nc.dram_tensor("master", [128, 2048], BF16, kind="ExternalInput").ap()
    projwt_d = nc.dram_tensor("projwt", [256, C], BF16, kind="ExternalInput").ap()
    biasb_d = nc.dram_tensor("biasb", [128, C], F32, kind="ExternalInput").ap()
    out_d = nc.dram_tensor("out", [N, C], F32, kind="ExternalOutput").ap()

    with tile.TileContext(nc) as tc:
        _body(nc, tc, xt_d, wcat_d, cos2_d, ssin2_d, shifteye_d, c8eye_d,
              master_d, projwt_d, biasb_d, out_d)
    nc.compile()
    return nc


def _body(nc, tc, xt_d, wcat_d, cos2_d, ssin2_d, shifteye_d, c8eye_d,
          master_d, projwt_d, biasb_d, out_d):
    from contextlib import ExitStack
    ctx = ExitStack()
    persist = ctx.enter_context(tc.tile_pool(name="persist", bufs=1))
    rope_pool = ctx.enter_context(tc.tile_pool(name="rope", bufs=2))
    exp_pool = ctx.enter_context(tc.tile_pool(name="exp", bufs=3))
    norm_pool = ctx.enter_context(tc.tile_pool(name="norm", bufs=2))
    fin_pool = ctx.enter_context(tc.tile_pool(name="final", bufs=4))
    pspool = ctx.enter_context(tc.tile_pool(name="ps", bufs=2, space="PSUM"))

    def ptile(shape, dtype, tag):
        return persist.tile(shape, dtype, tag=tag, name=tag)

    Exp = mybir.ActivationFunctionType.Exp

    # ---- persistent SBUF tiles ----
    w_sb = [ptile([128, 768], BF16, f"w{i}") for i in range(4)]
    xt_sb = [ptile([128, N], BF16, f"xt{i}") for i in range(4)]
    cos2 = ptile([128, N], BF16, "cos2")
    ssin2 = ptile([128, N], BF16, "ssin2")
    shifteye = ptile([128, 256], BF16, "shifteye")
    c8eye_sb = [ptile([128, 128], BF16, f"c8e{p}") for p in range(NP_)]
    master = ptile([128, 2048], BF16, "master")
    projw_sb = [ptile([128, C], BF16, f"pw{bd}") for bd in range(2)]
    biasb = ptile([128, C], F32, "biasb")
    q2_sb = [ptile([128, N], BF16, f"q2_{bd}") for bd in range(2)]
    k2_sb = [ptile([128, N], BF16, f"k2_{bd}") for bd in range(2)]
    v_sb = [ptile([128, NP_ * 65], BF16, f"v_{nt}") for nt in range(JT)]
    out_pair = [ptile([128, N], BF16, f"op_{bd}") for bd in range(2)]

    # ---- input DMAs, ordered to feed the PE emission order below ----
    # 1) V weights interleaved with the high xt columns so the first V
    # matmul starts after two transfers (V tiles run jt=15..0)
    for i in range(4):
        nc.sync.dma_start(w_sb[i][:, 512:768],
                          wcat_d[i * 128:(i + 1) * 128, 512:768])
        nc.sync.dma_start(xt_sb[i][:, 1536:2048],
                          xt_d[i * 128:(i + 1) * 128, 1536:2048])
    # 2) q/k weights + rope tables
    for i in range(4):
        nc.sync.dma_start(w_sb[i][:, 0:512], wcat_d[i * 128:(i + 1) * 128, 0:512])
    nc.sync.dma_start(shifteye[:], shifteye_d[:])
    nc.sync.dma_start(cos2[:], cos2_d[:])
    nc.sync.dma_start(ssin2[:], ssin2_d[:])
    # 3) remaining xt (descending), bias tables, proj weights
    for blk in (2, 1, 0):
        for i in range(4):
            nc.sync.dma_start(xt_sb[i][:, blk * 512:(blk + 1) * 512],
                              xt_d[i * 128:(i + 1) * 128, blk * 512:(blk + 1) * 512])
    nc.sync.dma_start(master[:], master_d[:])
    for p in range(NP_):
        nc.sync.dma_start(c8eye_sb[p][:], c8eye_d[p])
    for bd in range(2):
        nc.sync.dma_start(projw_sb[bd][:], projwt_d[bd * 128:(bd + 1) * 128, :])
    nc.sync.dma_start(biasb[:], biasb_d[:])

    # ---- helpers ----
    def v_tile(jt):
        psv = pspool.tile([128, 256], F32, tag="aux", name="psv")
        for ci in range(4):
            nc.tensor.matmul(
                psv[:], xt_sb[ci][:, jt * 128:(jt + 1) * 128],
                w_sb[ci][:, 512:768],
                start=(ci == 0), stop=(ci == 3))
        vdst = v_sb[jt].rearrange("p (h e) -> p h e", e=65)
        nc.vector.tensor_copy(vdst[:, :, 0:64],
                              psv.rearrange("p (h e) -> p h e", e=64))
        nc.gpsimd.memset(vdst[:, :, 64:65], 1.0)

    def qk_chunk_a(bd, kind, ch):
        # projection matmuls + cos/sin products for one 512-token chunk of
        # bundle bd (positions 2bd, 2bd+1), kind 0=q (scaled) 1=k.
        w_off = kind * 256 + bd * 128
        c0 = ch * 512
        ps_q = pspool.tile([128, 512], F32, tag="aux", name="ps_q")
        for ci in range(4):
            nc.tensor.matmul(
                ps_q[:],
                w_sb[ci][:, w_off:w_off + 128],
                xt_sb[ci][:, c0:c0 + 512],
                start=(ci == 0), stop=(ci == 3))
        tc_c = rope_pool.tile([128, 512], BF16, tag="tc", name="tc_c")
        nc.vector.tensor_mul(tc_c[:], ps_q[:], cos2[:, c0:c0 + 512])
        tc_u = rope_pool.tile([128, 512], BF16, tag="tu", name="tc_u")
        nc.vector.tensor_mul(tc_u[:], ps_q[:], ssin2[:, c0:c0 + 512])
        return tc_c, tc_u

    def qk_chunk_b(bd, kind, ch, tc_c, tc_u):
        # combine: dst = tc_c + P32 @ tc_u (partition-XOR-32 via matmul)
        dst_sb = k2_sb[bd] if kind else q2_sb[bd]
        c0 = ch * 512
        ps2 = pspool.tile([128, 512], F32, tag="aux", name="ps2")
        nc.tensor.matmul(ps2[:], shifteye[:, 0:128], tc_c[:],
                         start=True, stop=False)
        nc.tensor.matmul(ps2[:], shifteye[:, 128:256], tc_u[:],
                         start=False, stop=True)
        nc.vector.tensor_copy(dst_sb[:, c0:c0 + 512], ps2[:])

    # software-pipelined chunk list -> closures (B of chunk i rides with
    # A of chunk i+1 so the PE never waits on the DVE products)
    def chunk_closures(chunks):
        state = {}

        def make(i, spec):
            def run():
                if i > 0:
                    pb, pkd, pch = chunks[i - 1]
                    qk_chunk_b(pb, pkd, pch, *state.pop(i - 1))
                if spec is not None:
                    bd, kd, ch = spec
                    state[i] = qk_chunk_a(bd, kd, ch)
            return run

        return [make(i, spec)
                for i, spec in enumerate(list(chunks) + [None])]

    def bundle_chunks(bd):
        return [(bd, 1, 3), (bd, 1, 2), (bd, 1, 1), (bd, 1, 0),
                (bd, 0, 0), (bd, 0, 1), (bd, 0, 2), (bd, 0, 3)]

    # ---- PE pre-phase: V tiles (desc) interleaved with bundle-0 qk ----
    qk0 = chunk_closures(bundle_chunks(0))
    vt = [lambda jt=jt: v_tile(jt) for jt in range(JT - 1, -1, -1)]
    pre = [vt[0], vt[1], qk0[0], vt[2], vt[3], qk0[1], vt[4], vt[5], qk0[2],
           vt[6], vt[7], qk0[3], vt[8], vt[9], qk0[4], vt[10], vt[11],
           qk0[5], vt[12], vt[13], qk0[6], vt[14], vt[15]]
    for f in pre:
        f()

    # fillers: project bundle 1 while streaming the first two bundle-0 heads;
    # chunks 0..7 of the output projection (whose inputs complete with the
    # sp=0 streams) run inside the sp=1 streams, spreading the output DMAs.
    qk1 = chunk_closures(bundle_chunks(1))
    fillers = {(0, 0): qk1[:5], (0, 1): qk1[5:],
               (0, 2): [qk0[7], qk0[8]],
               (1, 1): [lambda chk=chk: proj_chunk(chk) for chk in range(0, 4)],
               (1, 2): [lambda chk=chk: proj_chunk(chk) for chk in range(4, 8)]}

    def proj_chunk(chk):
        # full output projection of one 128-query chunk (own heads' partial)
        ps = pspool.tile([128, 512], F32, tag="aux", name="ps_proj")
        for bd in range(2):
            nc.tensor.matmul(ps[:], out_pair[bd][:, chk * 128:(chk + 1) * 128],
                             projw_sb[bd][:],
                             start=(bd == 0), stop=(bd == 1),
                             skip_group_check=True)
        fin = fin_pool.tile([128, 512], F32, tag="f", name="fin")
        nc.vector.tensor_add(fin[:], ps[:], biasb[:])
        nc.sync.dma_start(out_d[chk * 128:(chk + 1) * 128, :], fin[:])

    def norm_sub(sp, p, avt, qi):
        # attn-V is [query-part, head-dim]: the softmax denominator is a
        # per-partition scalar -> reciprocal + tensor_scalar multiply, then
        # PE-transpose back to [dim, query] for the projection.
        bd, row = p // 2, (p % 2) * 64
        col = 128 * (qi % 4)
        rec = norm_pool.tile([128, 1], F32, tag="rc", name="rec")
        nc.vector.reciprocal(rec[:], avt[:, col + 64:col + 65])
        stag = norm_pool.tile([128, 64], BF16, tag="st", name="stag")
        nc.vector.tensor_scalar_mul(stag[:], avt[:, col:col + 64], rec[:])
        pst = pspool.tile([64, 128], BF16, tag="aux", name="pst")
        nc.tensor.transpose(pst[:], stag[:], shifteye[:, 0:128])
        nc.vector.tensor_copy(
            out_pair[bd][row:row + 64,
                         1024 * sp + 128 * qi:1024 * sp + 128 * qi + 128],
            pst[:])

    # ---- attention streams: one per (slot-pair sp, position p) ----
    for sp in range(2):
        for p in range(NP_):
            bd, row = p // 2, (p % 2) * 64
            fl = fillers.get((sp, p), [])
            fi = 0
            jts = list(range(JT - 1, JTMIN[p][sp] - 1, -1))
            av01 = (pspool.tile([128, 512], F32, tag="av0", bufs=1, name="av0"),
                    pspool.tile([128, 512], F32, tag="av1", bufs=1, name="av1"))
            # PSUM accumulation state is per-bank on hardware: only read a
            # subtile once ALL subtiles sharing its bank have stopped.
            norm_at = {}
            norm_at.setdefault(STOPJT[p][sp][4], []).extend([4, 5, 6, 7])
            norm_at.setdefault(STOPJT[p][sp][0], []).extend([0, 1, 2, 3])
            pend = []  # [(jt, av-mm closure)], delayed two steps
            for idx, jt in enumerate(jts):
                while fi < len(fl) and fi * len(jts) <= idx * len(fl):
                    fl[fi]()
                    fi += 1
                qm0, qm1, ql = (QM[p][sp][0][jt], QM[p][sp][1][jt],
                                QLIM[p][sp][jt])
                br = BIASR[p][sp][jt]
                ps = pspool.tile([128, 1024], F32, tag="sc", name="ps_sc")
                nc.tensor.matmul(
                    ps[:, 0:qm0],
                    k2_sb[bd][row:row + 64, jt * 128:(jt + 1) * 128],
                    q2_sb[bd][row:row + 64, 1024 * sp:1024 * sp + qm0],
                    start=True, stop=(br is None), tile_position=(row, 0),
                    skip_group_check=True)
                if qm1 > 0:
                    nc.tensor.matmul(
                        ps[:, 512:512 + qm1],
                        k2_sb[bd][row:row + 64, jt * 128:(jt + 1) * 128],
                        q2_sb[bd][row:row + 64,
                                  1024 * sp + 512:1024 * sp + 512 + qm1],
                        start=True, stop=(br is None), tile_position=(row, 0),
                        skip_group_check=True)
                if br is not None:
                    o = 1024 * sp - 128 * jt
                    # split at the 512-col PSUM bank boundary
                    for lo, hi in ((br[0], min(br[1], 512)),
                                   (max(br[0], 512), br[1])):
                        if lo < hi:
                            nc.tensor.matmul(
                                ps[:, lo:hi], c8eye_sb[p][:],
                                master[:, lo + o:hi + o],
                                start=False, stop=True, tile_position=(0, 0),
                                skip_group_check=True)
                et = exp_pool.tile([128, 1024], BF16, tag="e", name="et")
                nc.scalar.activation(et[:, 0:ql], ps[:, 0:ql], Exp)

                def av_mms(jt=jt, ql=ql, et=et):
                    for qi in range((ql + 127) // 128):
                        m = min(128, ql - 128 * qi)
                        avt = av01[qi // 4]
                        col = 128 * (qi % 4)
                        nc.tensor.matmul(
                            avt[0:m, col:col + 65],
                            et[:, 128 * qi:128 * qi + m],
                            v_sb[jt][:, p * 65:(p + 1) * 65],
                            start=(jt == JT - 1 and qi % 4 == 0),
                            stop=(jt == STOPJT[p][sp][qi]),
                            skip_group_check=True)

                if len(pend) >= 2:
                    pjt, pfn = pend.pop(0)
                    pfn()
                    for qi in norm_at.get(pjt, []):
                        norm_sub(sp, p, av01[qi // 4], qi)
                pend.append((jt, av_mms))
            for pjt, pfn in pend:
                pfn()
                for qi in norm_at.get(pjt, []):
                    norm_sub(sp, p, av01[qi // 4], qi)
            while fi < len(fl):
                fl[fi]()
                fi += 1

    # ---- remaining output projection chunks (sp=1 queries) ----
    # software-pipelined: each chunk's bundle-0 partial runs during the final
    # norm drain; only the bundle-1 matmul waits on the last normalize.
    def proj_pre(chk):
        ps = pspool.tile([128, 512], F32, tag="aux", name="ps_proj")
        nc.tensor.matmul(ps[:], out_pair[0][:, chk * 128:(chk + 1) * 128],
                         projw_sb[0][:],
                         start=True, stop=False, skip_group_check=True)
        return ps

    def proj_fin(chk, ps):
        nc.tensor.matmul(ps[:], out_pair[1][:, chk * 128:(chk + 1) * 128],
                         projw_sb[1][:],
                         start=False, stop=True, skip_group_check=True)
        fin = fin_pool.tile([128, 512], F32, tag="f", name="fin")
        nc.vector.tensor_add(fin[:], ps[:], biasb[:])
        nc.sync.dma_start(out_d[chk * 128:(chk + 1) * 128, :], fin[:])

    # slot1 chunks (12..15) are unblocked at the last stream's midpoint;
    # slot0 chunks (8..11) wait for the final normalize, so run them last.
    order = [12, 13, 14, 15, 8, 9, 10, 11]
    pend_ps = {order[0]: proj_pre(order[0]), order[1]: proj_pre(order[1])}
    for i, chk in enumerate(order):
        proj_fin(chk, pend_ps.pop(chk))
        if i + 2 < len(order):
            pend_ps[order[i + 2]] = proj_pre(order[i + 2])

    ctx.close()


@functools.lru_cache(maxsize=1)
def _graph():
    return _build_graph()


def kernel(x, qkv_w, proj_w, proj_b):
    global LAST_RESULT
    x = np.asarray(x, np.float32)
    qkv_w = np.asarray(qkv_w, np.float32)
    proj_w = np.asarray(proj_w, np.float32)
    proj_b = np.asarray(proj_b, np.float32)

    nc = _graph()
    shared, qw, proj = _shared_inputs(qkv_w, proj_w, proj_b)
    in_maps = [_core_inputs(c, x, shared, qw, proj) for c in range(NCORES)]
    res = bass_utils.run_bass_kernel_spmd(nc, in_maps,
                                          core_ids=list(range(NCORES)),
                                          trace=False)
    LAST_RESULT = res
    out = np.zeros((B, N, C), np.float32)
    for b in range(B):
        out[b] = (np.asarray(res.results[2 * b]["out"], np.float32)
                  + np.asarray(res.results[2 * b + 1]["out"], np.float32))
    return out


# revision 86
# speedup vs baseline: 1.0633x; 1.0349x over previous
"""Fused multi-head attention layer (RoPE + ALiBi + softmax + out-proj) on 8 TRN2 cores.

Sharding (v4, heads-split): core c -> (batch b = c//2, head group g = c%2).
Each core computes q/k/v for its 4 heads {g, 2+g, 4+g, 6+g} over ALL 2048
positions, runs banded attention, and projects through its heads' slice of
proj_w, producing a partial [N, C] output; the host sums the two partials
per batch. Pairing adjacent-radius heads per graph position keeps the SPMD
union of ALiBi bands tight, and query blocks have exact (not unioned)
trim bounds.

Pipeline features:
- RoPE rotate-half via a sign-folded sin table + partition-permutation
  matmul (no duplicate rot projections).
- ALiBi bias on PE as c8-scaled identity matmuls against a single shared
  anti-diagonal master pattern table (column-shifted AP views), trimmed to
  the band columns.
- Score/exp/attn-V column ranges prefix-trimmed per (position, block, jt).
- Streams software-pipelined; V/next-bundle projections fill PE gaps.
"""

import functools
import math
import os
import sys

import numpy as np

sys.path.insert(0, "/opt/trn_rl_repo")

import ml_dtypes  # noqa: E402

import concourse.bass as bass  # noqa: E402
import concourse.tile as tile  # noqa: E402
from concourse import bacc, mybir, bass_utils  # noqa: E402

BF16 = mybir.dt.bfloat16
F32 = mybir.dt.float32
NPBF = ml_dtypes.bfloat16

B, N, C, H, D = 4, 2048, 512, 8, 64
NCORES = 8
JT = N // 128        # 16 j-tiles of 128 key positions
NP_ = 4              # head positions per core
T_CUT = 30.0         # ALiBi cutoff in logits: exp(-30) is negligible
SCALE = D ** -0.5

# c8_h = alibi_slope_h * MAX_BIAS = 2^-(h+1) * 8 = 2^(2-h)
C8 = [2.0 ** (2 - h) for h in range(H)]
RADIUS = [T_CUT / c for c in C8]   # band reach (key positions) per head
# graph position p holds heads {2p, 2p+1}; the union band is the odd head's
UR = [RADIUS[2 * p + 1] for p in range(NP_)]

LAST_RESULT = None  # test harness reads exec_time_ns from here


def _clamp(v, lo, hi):
    return max(lo, min(hi, v))


# Frame for (position p, slot-pair sp): cols 0:512 = query block 2sp
# (i0 = 1024sp), cols 512:1024 = block 2sp+1 (i0 = 1024sp+512). Exact bounds.
def _qm(p, sp, sl, jt):
    i0 = 512 * (2 * sp + sl)
    return _clamp(int(math.floor(128 * jt + 127 + UR[p])) + 1 - i0, 0, 512)


QM = [[[[_qm(p, sp, sl, jt) for jt in range(JT)] for sl in range(2)]
       for sp in range(2)] for p in range(NP_)]
QLIM = [[[(QM[p][sp][0][jt] if QM[p][sp][0][jt] < 512
           else 512 + QM[p][sp][1][jt]) for jt in range(JT)]
         for sp in range(2)] for p in range(NP_)]
JTMIN = [[min(jt for jt in range(JT) if QLIM[p][sp][jt] > 0)
          for sp in range(2)] for p in range(NP_)]
# per 128-query subtile qi of the 1024-col frame: the last (smallest) jt
# whose kept prefix still reaches it (attn-V accumulation stop point)
STOPJT = [[[min(jt for jt in range(JT) if QLIM[p][sp][jt] > 128 * qi)
            for qi in range(8)] for sp in range(2)] for p in range(NP_)]


def _bias_range(p, sp, jt):
    qm0, qm1 = QM[p][sp][0][jt], QM[p][sp][1][jt]
    bs0 = max(0, 128 * jt - 1024 * sp + 1)
    bs1 = max(0, 128 * jt - 1024 * sp - 511)
    r = []
    if bs0 < qm0:
        r.append((bs0, qm0))
    if bs1 < qm1:
        r.append((512 + bs1, 512 + qm1))
    if len(r) == 2:
        assert r[0][1] == 512 and r[1][0] == 512, (p, sp, jt, r)
        r = [(r[0][0], r[1][1])]
    return r[0] if r else None


BIASR = [[[_bias_range(p, sp, jt) for jt in range(JT)] for sp in range(2)]
         for p in range(NP_)]


def _heads(g):
    return [2 * p + g for p in range(NP_)]


def _rope_tables():
    inv = 1.0 / (10000.0 ** (np.arange(0, D, 2, dtype=np.float32) / D))
    f = np.arange(N, dtype=np.float32)[:, None] * inv[None, :]
    sin = np.concatenate([np.sin(f), np.sin(f)], axis=-1).astype(np.float32)
    cos = np.concatenate([np.cos(f), np.cos(f)], axis=-1).astype(np.float32)
    return sin, cos  # [N, D]


def _st_table(sin):
    # sign-folded, half-swapped sin table, indexed by SOURCE row r: after the
    # XOR-32 partition permutation, dest row d gets rot_half(q)[d]*sin[d].
    st = np.empty_like(sin)            # [N, D]
    st[:, 0:32] = sin[:, 32:64]
    st[:, 32:64] = -sin[:, 0:32]
    return st


def _shared_inputs(qkv_w, proj_w, proj_b):
    # [I | P32]: P32 is the XOR-32 partition permutation (within 64-blocks)
    shifteye = np.zeros((128, 256), np.float32)
    shifteye[:, 0:128] = np.eye(128)
    for r in range(128):
        shifteye[r, 128 + (r ^ 32)] = 1.0

    # master ALiBi pattern: master[k, x] = min(k - x, 0); the tile for
    # (jt, block i0) is the column window shifted by o = i0 - 128*jt
    jl = np.arange(128, dtype=np.float32)[:, None]
    xl = np.arange(2048, dtype=np.float32)[None, :]
    master = np.minimum(jl - xl, 0.0).astype(NPBF)

    sin, cos = _rope_tables()
    cos2 = np.tile(cos.T, (2, 1)).astype(NPBF)    # [128, N]
    ssin2 = np.tile(_st_table(sin).T, (2, 1)).astype(NPBF)
    return {
        "shifteye": shifteye.astype(NPBF),
        "master": master,
        "cos2": cos2, "ssin2": ssin2,
    }, qkv_w, (proj_w, proj_b)


def _core_inputs(c, x, shared, qkv_w, proj):
    proj_w, proj_b = proj
    b, g = c // 2, c % 2
    heads = _heads(g)
    cols = np.concatenate([np.arange(64 * h, 64 * h + 64) for h in heads])

    wqT = np.ascontiguousarray(qkv_w[0:C].T)[:, cols] * SCALE     # [C, 256]
    wkT = np.ascontiguousarray(qkv_w[C:2 * C].T)[:, cols]
    wvT = np.ascontiguousarray(qkv_w[2 * C:3 * C].T)[:, cols]
    wcat = np.concatenate([wqT, wkT, wvT], axis=1).astype(NPBF)   # [C, 768]

    c8eye = np.zeros((NP_, 128, 128), np.float32)
    for p in range(NP_):
        np.fill_diagonal(c8eye[p], C8[heads[p]])

    projwt = np.ascontiguousarray(proj_w.T)[cols, :]              # [256, C]
    biasb = np.tile(proj_b[None, :], (128, 1)) if g == 0 else \
        np.zeros((128, C), np.float32)

    return {
        "xt": np.ascontiguousarray(x[b].T).astype(NPBF),          # [C, N]
        "wcat": wcat,
        "c8eye": c8eye.astype(NPBF),
        "projwt": projwt.astype(NPBF),
        "biasb": biasb.astype(np.float32),
        **shared,
    }


def _build_graph():
    nc = bacc.Bacc("TRN2", target_bir_lowering=False, debug=False,
                   num_devices=NCORES)

    xt_d = nc.dram_tensor("xt", [C, N], BF16, kind="ExternalInput").ap()
    wcat_d = nc.dram_tensor("wcat", [C, 768], BF16, kind="ExternalInput").ap()
    cos2_d = nc.dram_tensor("cos2", [128, N], BF16, kind="ExternalInput").ap()
    ssin2_d = nc.dram_tensor("ssin2", [128, N], BF16, kind="ExternalInput").ap()
    shifteye_d = nc.dram_tensor("shifteye", [128, 256], BF16, kind="ExternalInput").ap()
    c8eye_d = nc.dram_tensor("c8eye", [NP_, 128, 128], BF16, kind="ExternalInput").ap()
    master_d = nc.dram_tensor("master", [128, 2048], BF16, kind="ExternalInput").ap()
    projwt_d = nc.dram_tensor("projwt", [256, C], BF16, kind="ExternalInput").ap()
    biasb_d = nc.dram_tensor("biasb", [128, C], F32, kind="ExternalInput").ap()
    out_d = nc.dram_tensor("out", [N, C], F32, kind="ExternalOutput").ap()

    with tile.TileContext(nc) as tc:
        _body(nc, tc, xt_d, wcat_d, cos2_d, ssin2_d, shifteye_d, c8eye_d,
              master_d, projwt_d, biasb_d, out_d)
    nc.compile()
    return nc


def _body(nc, tc, xt_d, wcat_d, cos2_d, ssin2_d, shifteye_d, c8eye_d,
          master_d, projwt_d, biasb_d, out_d):
    from contextlib import ExitStack
    ctx = ExitStack()
    persist = ctx.enter_context(tc.tile_pool(name="persist", bufs=1))
    rope_pool = ctx.enter_context(tc.tile_pool(name="rope", bufs=2))
    exp_pool = ctx.enter_context(tc.tile_pool(name="exp", bufs=5))
    norm_pool = ctx.enter_context(tc.tile_pool(name="norm", bufs=2))
    fin_pool = ctx.enter_context(tc.tile_pool(name="final", bufs=4))
    pspool = ctx.enter_context(tc.tile_pool(name="ps", bufs=2, space="PSUM"))

    def ptile(shape, dtype, tag):
        return persist.tile(shape, dtype, tag=tag, name=tag)

    Exp = mybir.ActivationFunctionType.Exp

    # ---- persistent SBUF tiles ----
    w_sb = [ptile([128, 768], BF16, f"w{i}") for i in range(4)]
    xt_sb = [ptile([128, N], BF16, f"xt{i}") for i in range(4)]
    cos2 = ptile([128, N], BF16, "cos2")
    ssin2 = ptile([128, N], BF16, "ssin2")
    shifteye = ptile([128, 256], BF16, "shifteye")
    c8eye_sb = [ptile([128, 128], BF16, f"c8e{p}") for p in range(NP_)]
    master = ptile([128, 2048], BF16, "master")
    projw_sb = [ptile([128, C], BF16, f"pw{bd}") for bd in range(2)]
    biasb = ptile([128, C], F32, "biasb")
    q2_sb = [ptile([128, N], BF16, f"q2_{bd}") for bd in range(2)]
    k2_sb = [ptile([128, N], BF16, f"k2_{bd}") for bd in range(2)]
    v_sb = [ptile([128, NP_ * 65], BF16, f"v_{nt}") for nt in range(JT)]
    out_pair = [ptile([128, N], BF16, f"op_{bd}") for bd in range(2)]

    # ---- input DMAs, ordered to feed the PE emission order below ----
    # 1) V weights interleaved with the high xt columns so the first V
    # matmul starts after two transfers (V tiles run jt=15..0)
    for i in range(4):
        nc.sync.dma_start(w_sb[i][:, 512:768],
                          wcat_d[i * 128:(i + 1) * 128, 512:768])
        nc.sync.dma_start(xt_sb[i][:, 1536:2048],
                          xt_d[i * 128:(i + 1) * 128, 1536:2048])
    # 2) q/k weights + rope tables
    for i in range(4):
        nc.sync.dma_start(w_sb[i][:, 0:512], wcat_d[i * 128:(i + 1) * 128, 0:512])
    nc.sync.dma_start(shifteye[:], shifteye_d[:])
    nc.sync.dma_start(cos2[:], cos2_d[:])
    nc.sync.dma_start(ssin2[:], ssin2_d[:])
    # 3) remaining xt (descending), bias tables, proj weights
    for blk in (2, 1, 0):
        for i in range(4):
            nc.sync.dma_start(xt_sb[i][:, blk * 512:(blk + 1) * 512],
                              xt_d[i * 128:(i + 1) * 128, blk * 512:(blk + 1) * 512])
    nc.sync.dma_start(master[:], master_d[:])
    for p in range(NP_):
        nc.sync.dma_start(c8eye_sb[p][:], c8eye_d[p])
    for bd in range(2):
        nc.sync.dma_start(projw_sb[bd][:], projwt_d[bd * 128:(bd + 1) * 128, :])
    nc.sync.dma_start(biasb[:], biasb_d[:])

    # ---- helpers ----
    def v_tile(jt):
        psv = pspool.tile([128, 256], F32, tag="aux", name="psv")
        for ci in range(4):
            nc.tensor.matmul(
                psv[:], xt_sb[ci][:, jt * 128:(jt + 1) * 128],
                w_sb[ci][:, 512:768],
                start=(ci == 0), stop=(ci == 3))
        vdst = v_sb[jt].rearrange("p (h e) -> p h e", e=65)
        nc.vector.tensor_copy(vdst[:, :, 0:64],
                              psv.rearrange("p (h e) -> p h e", e=64))
        nc.gpsimd.memset(vdst[:, :, 64:65], 1.0)

    def qk_chunk_a(bd, kind, ch):
        # projection matmuls + cos/sin products for one 512-token chunk of
        # bundle bd (positions 2bd, 2bd+1), kind 0=q (scaled) 1=k.
        w_off = kind * 256 + bd * 128
        c0 = ch * 512
        ps_q = pspool.tile([128, 512], F32, tag="aux", name="ps_q")
        for ci in range(4):
            nc.tensor.matmul(
                ps_q[:],
                w_sb[ci][:, w_off:w_off + 128],
                xt_sb[ci][:, c0:c0 + 512],
                start=(ci == 0), stop=(ci == 3))
        tc_c = rope_pool.tile([128, 512], BF16, tag="tc", name="tc_c")
        nc.vector.tensor_mul(tc_c[:], ps_q[:], cos2[:, c0:c0 + 512])
        tc_u = rope_pool.tile([128, 512], BF16, tag="tu", name="tc_u")
        nc.vector.tensor_mul(tc_u[:], ps_q[:], ssin2[:, c0:c0 + 512])
        return tc_c, tc_u

    def qk_chunk_b(bd, kind, ch, tc_c, tc_u):
        # combine: dst = tc_c + P32 @ tc_u (partition-XOR-32 via matmul)
        dst_sb = k2_sb[bd] if kind else q2_sb[bd]
        c0 = ch * 512
        ps2 = pspool.tile([128, 512], F32, tag="aux", name="ps2")
        nc.tensor.matmul(ps2[:], shifteye[:, 0:128], tc_c[:],
                         start=True, stop=False)
        nc.tensor.matmul(ps2[:], shifteye[:, 128:256], tc_u[:],
                         start=False, stop=True)
        nc.vector.tensor_copy(dst_sb[:, c0:c0 + 512], ps2[:])

    # software-pipelined chunk list -> closures (B of chunk i rides with
    # A of chunk i+1 so the PE never waits on the DVE products)
    def chunk_closures(chunks):
        state = {}

        def make(i, spec):
            def run():
                if i > 0:
                    pb, pkd, pch = chunks[i - 1]
                    qk_chunk_b(pb, pkd, pch, *state.pop(i - 1))
                if spec is not None:
                    bd, kd, ch = spec
                    state[i] = qk_chunk_a(bd, kd, ch)
            return run

        return [make(i, spec)
                for i, spec in enumerate(list(chunks) + [None])]

    def bundle_chunks(bd):
        return [(bd, 1, 3), (bd, 1, 2), (bd, 1, 1), (bd, 1, 0),
                (bd, 0, 0), (bd, 0, 1), (bd, 0, 2), (bd, 0, 3)]

    # ---- PE pre-phase: V tiles (desc) interleaved with bundle-0 qk ----
    qk0 = chunk_closures(bundle_chunks(0))
    vt = [lambda jt=jt: v_tile(jt) for jt in range(JT - 1, -1, -1)]
    pre = [vt[0], vt[1], qk0[0], vt[2], vt[3], qk0[1], vt[4], vt[5], qk0[2],
           vt[6], vt[7], qk0[3], vt[8], vt[9], qk0[4], vt[10], vt[11],
           qk0[5], vt[12], vt[13], qk0[6], vt[14], vt[15]]
    for f in pre:
        f()

    # fillers: project bundle 1 while streaming the first two bundle-0 heads;
    # chunks 0..7 of the output projection (whose inputs complete with the
    # sp=0 streams) run inside the sp=1 streams, spreading the output DMAs.
    qk1 = chunk_closures(bundle_chunks(1))
    fillers = {(0, 0): qk1[:5], (0, 1): qk1[5:],
               (0, 2): [qk0[7], qk0[8]],
               (1, 1): [lambda chk=chk: proj_chunk(chk) for chk in range(0, 4)],
               (1, 2): [lambda chk=chk: proj_chunk(chk) for chk in range(4, 8)]}

    def proj_chunk(chk):
        # full output projection of one 128-query chunk (own heads' partial)
        ps = pspool.tile([128, 512], F32, tag="aux", name="ps_proj")
        for bd in range(2):
            nc.tensor.matmul(ps[:], out_pair[bd][:, chk * 128:(chk + 1) * 128],
                             projw_sb[bd][:],
                             start=(bd == 0), stop=(bd == 1),
                             skip_group_check=True)
        fin = fin_pool.tile([128, 512], F32, tag="f", name="fin")
        nc.vector.tensor_add(fin[:], ps[:], biasb[:])
        nc.sync.dma_start(out_d[chk * 128:(chk + 1) * 128, :], fin[:])

    def norm_sub(sp, p, avt, qi):
        # attn-V is [query-part, head-dim]: the softmax denominator is a
        # per-partition scalar -> reciprocal + tensor_scalar multiply, then
        # PE-transpose back to [dim, query] for the projection.
        bd, row = p // 2, (p % 2) * 64
        col = 128 * (qi % 4)
        rec = norm_pool.tile([128, 1], F32, tag="rc", name="rec")
        nc.vector.reciprocal(rec[:], avt[:, col + 64:col + 65])
        stag = norm_pool.tile([128, 64], BF16, tag="st", name="stag")
        nc.vector.tensor_scalar_mul(stag[:], avt[:, col:col + 64], rec[:])
        pst = pspool.tile([64, 128], BF16, tag="aux", name="pst")
        nc.tensor.transpose(pst[:], stag[:], shifteye[:, 0:128])
        nc.vector.tensor_copy(
            out_pair[bd][row:row + 64,
                         1024 * sp + 128 * qi:1024 * sp + 128 * qi + 128],
            pst[:])

    # ---- attention streams: one per (slot-pair sp, position p) ----
    for sp in range(2):
        for p in range(NP_):
            bd, row = p // 2, (p % 2) * 64
            fl = fillers.get((sp, p), [])
            fi = 0
            jts = list(range(JT - 1, JTMIN[p][sp] - 1, -1))
            av01 = (pspool.tile([128, 512], F32, tag="av0", bufs=1, name="av0"),
                    pspool.tile([128, 512], F32, tag="av1", bufs=1, name="av1"))
            # PSUM accumulation state is per-bank on hardware: only read a
            # subtile once ALL subtiles sharing its bank have stopped.
            norm_at = {}
            norm_at.setdefault(STOPJT[p][sp][4], []).extend([4, 5, 6, 7])
            norm_at.setdefault(STOPJT[p][sp][0], []).extend([0, 1, 2, 3])
            pend = []  # [(jt, av-mm closure)], delayed two steps
            for idx, jt in enumerate(jts):
                while fi < len(fl) and fi * len(jts) <= idx * len(fl):
                    fl[fi]()
                    fi += 1
                qm0, qm1, ql = (QM[p][sp][0][jt], QM[p][sp][1][jt],
                                QLIM[p][sp][jt])
                br = BIASR[p][sp][jt]
                ps = pspool.tile([128, 1024], F32, tag="sc", name="ps_sc")
                nc.tensor.matmul(
                    ps[:, 0:qm0],
                    k2_sb[bd][row:row + 64, jt * 128:(jt + 1) * 128],
                    q2_sb[bd][row:row + 64, 1024 * sp:1024 * sp + qm0],
                    start=True, stop=(br is None), tile_position=(row, 0),
                    skip_group_check=True)
                if qm1 > 0:
                    nc.tensor.matmul(
                        ps[:, 512:512 + qm1],
                        k2_sb[bd][row:row + 64, jt * 128:(jt + 1) * 128],
                        q2_sb[bd][row:row + 64,
                                  1024 * sp + 512:1024 * sp + 512 + qm1],
                        start=True, stop=(br is None), tile_position=(row, 0),
                        skip_group_check=True)
                if br is not None:
                    o = 1024 * sp - 128 * jt
                    # split at the 512-col PSUM bank boundary
                    for lo, hi in ((br[0], min(br[1], 512)),
                                   (max(br[0], 512), br[1])):
                        if lo < hi:
                            nc.tensor.matmul(
                                ps[:, lo:hi], c8eye_sb[p][:],
                                master[:, lo + o:hi + o],
                                start=False, stop=True, tile_position=(0, 0),
                                skip_group_check=True)
                et = exp_pool.tile([128, 1024], BF16, tag="e", name="et")
                nc.scalar.activation(et[:, 0:ql], ps[:, 0:ql], Exp)

                def av_mms(jt=jt, ql=ql, et=et):
                    for qi in range((ql + 127) // 128):
                        m = min(128, ql - 128 * qi)
                        avt = av01[qi // 4]
                        col = 128 * (qi % 4)
                        nc.tensor.matmul(
                            avt[0:m, col:col + 65],
                            et[:, 128 * qi:128 * qi + m],
                            v_sb[jt][:, p * 65:(p + 1) * 65],
                            start=(jt == JT - 1 and qi % 4 == 0),
                            stop=(jt == STOPJT[p][sp][qi]),
                            skip_group_check=True)

                if len(pend) >= 4:
                    pjt, pfn = pend.pop(0)
                    pfn()
                    for qi in norm_at.get(pjt, []):
                        norm_sub(sp, p, av01[qi // 4], qi)
                pend.append((jt, av_mms))
            for pjt, pfn in pend:
                pfn()
                for qi in norm_at.get(pjt, []):
                    norm_sub(sp, p, av01[qi // 4], qi)
            while fi < len(fl):
                fl[fi]()
                fi += 1

    # ---- remaining output projection chunks (sp=1 queries) ----
    # software-pipelined: each chunk's bundle-0 partial runs during the final
    # norm drain; only the bundle-1 matmul waits on the last normalize.
    def proj_pre(chk):
        ps = pspool.tile([128, 512], F32, tag="aux", name="ps_proj")
        nc.tensor.matmul(ps[:], out_pair[0][:, chk * 128:(chk + 1) * 128],
                         projw_sb[0][:],
                         start=True, stop=False, skip_group_check=True)
        return ps

    def proj_fin(chk, ps):
        nc.tensor.matmul(ps[:], out_pair[1][:, chk * 128:(chk + 1) * 128],
                         projw_sb[1][:],
                         start=False, stop=True, skip_group_check=True)
        fin = fin_pool.tile([128, 512], F32, tag="f", name="fin")
        nc.vector.tensor_add(fin[:], ps[:], biasb[:])
        nc.sync.dma_start(out_d[chk * 128:(chk + 1) * 128, :], fin[:])

    # slot1 chunks (12..15) are unblocked at the last stream's midpoint;
    # slot0 chunks (8..11) wait for the final normalize, so run them last.
    order = [12, 13, 14, 15, 8, 9, 10, 11]
    pend_ps = {order[0]: proj_pre(order[0]), order[1]: proj_pre(order[1])}
    for i, chk in enumerate(order):
        proj_fin(chk, pend_ps.pop(chk))
        if i + 2 < len(order):
            pend_ps[order[i + 2]] = proj_pre(order[i + 2])

    ctx.close()


@functools.lru_cache(maxsize=1)
def _graph():
    return _build_graph()


def kernel(x, qkv_w, proj_w, proj_b):
    global LAST_RESULT
    x = np.asarray(x, np.float32)
    qkv_w = np.asarray(qkv_w, np.float32)
    proj_w = np.asarray(proj_w, np.float32)
    proj_b = np.asarray(proj_b, np.float32)

    nc = _graph()
    shared, qw, proj = _shared_inputs(qkv_w, proj_w, proj_b)
    in_maps = [_core_inputs(c, x, shared, qw, proj) for c in range(NCORES)]
    res = bass_utils.run_bass_kernel_spmd(nc, in_maps,
                                          core_ids=list(range(NCORES)),
                                          trace=False)
    LAST_RESULT = res
    out = np.zeros((B, N, C), np.float32)
    for b in range(B):
        out[b] = (np.asarray(res.results[2 * b]["out"], np.float32)
                  + np.asarray(res.results[2 * b + 1]["out"], np.float32))
    return out


# revision 89
# speedup vs baseline: 1.0689x; 1.0053x over previous
"""Fused multi-head attention layer (RoPE + ALiBi + softmax + out-proj) on 8 TRN2 cores.

Sharding (v4, heads-split): core c -> (batch b = c//2, head group g = c%2).
Each core computes q/k/v for its 4 heads {g, 2+g, 4+g, 6+g} over ALL 2048
positions, runs banded attention, and projects through its heads' slice of
proj_w, producing a partial [N, C] output; the host sums the two partials
per batch. Pairing adjacent-radius heads per graph position keeps the SPMD
union of ALiBi bands tight, and query blocks have exact (not unioned)
trim bounds.

Pipeline features:
- RoPE rotate-half via a sign-folded sin table + partition-permutation
  matmul (no duplicate rot projections).
- ALiBi bias on PE as c8-scaled identity matmuls against a single shared
  anti-diagonal master pattern table (column-shifted AP views), trimmed to
  the band columns.
- Score/exp/attn-V column ranges prefix-trimmed per (position, block, jt).
- Streams software-pipelined; V/next-bundle projections fill PE gaps.
"""

import functools
import math
import os
import sys

import numpy as np

sys.path.insert(0, "/opt/trn_rl_repo")

import ml_dtypes  # noqa: E402

import concourse.bass as bass  # noqa: E402
import concourse.tile as tile  # noqa: E402
from concourse import bacc, mybir, bass_utils  # noqa: E402

BF16 = mybir.dt.bfloat16
F32 = mybir.dt.float32
NPBF = ml_dtypes.bfloat16

B, N, C, H, D = 4, 2048, 512, 8, 64
NCORES = 8
JT = N // 128        # 16 j-tiles of 128 key positions
NP_ = 4              # head positions per core
T_CUT = 30.0         # ALiBi cutoff in logits: exp(-30) is negligible
SCALE = D ** -0.5

# c8_h = alibi_slope_h * MAX_BIAS = 2^-(h+1) * 8 = 2^(2-h)
C8 = [2.0 ** (2 - h) for h in range(H)]
RADIUS = [T_CUT / c for c in C8]   # band reach (key positions) per head
# graph position p holds heads {2p, 2p+1}; the union band is the odd head's
UR = [RADIUS[2 * p + 1] for p in range(NP_)]

LAST_RESULT = None  # test harness reads exec_time_ns from here


def _clamp(v, lo, hi):
    return max(lo, min(hi, v))


# Frame for (position p, slot-pair sp): cols 0:512 = query block 2sp
# (i0 = 1024sp), cols 512:1024 = block 2sp+1 (i0 = 1024sp+512). Exact bounds.
def _qm(p, sp, sl, jt):
    i0 = 512 * (2 * sp + sl)
    return _clamp(int(math.floor(128 * jt + 127 + UR[p])) + 1 - i0, 0, 512)


QM = [[[[_qm(p, sp, sl, jt) for jt in range(JT)] for sl in range(2)]
       for sp in range(2)] for p in range(NP_)]
QLIM = [[[(QM[p][sp][0][jt] if QM[p][sp][0][jt] < 512
           else 512 + QM[p][sp][1][jt]) for jt in range(JT)]
         for sp in range(2)] for p in range(NP_)]
JTMIN = [[min(jt for jt in range(JT) if QLIM[p][sp][jt] > 0)
          for sp in range(2)] for p in range(NP_)]
# per 128-query subtile qi of the 1024-col frame: the last (smallest) jt
# whose kept prefix still reaches it (attn-V accumulation stop point)
STOPJT = [[[min(jt for jt in range(JT) if QLIM[p][sp][jt] > 128 * qi)
            for qi in range(8)] for sp in range(2)] for p in range(NP_)]


def _bias_range(p, sp, jt):
    qm0, qm1 = QM[p][sp][0][jt], QM[p][sp][1][jt]
    bs0 = max(0, 128 * jt - 1024 * sp + 1)
    bs1 = max(0, 128 * jt - 1024 * sp - 511)
    r = []
    if bs0 < qm0:
        r.append((bs0, qm0))
    if bs1 < qm1:
        r.append((512 + bs1, 512 + qm1))
    if len(r) == 2:
        assert r[0][1] == 512 and r[1][0] == 512, (p, sp, jt, r)
        r = [(r[0][0], r[1][1])]
    return r[0] if r else None


BIASR = [[[_bias_range(p, sp, jt) for jt in range(JT)] for sp in range(2)]
         for p in range(NP_)]


def _heads(g):
    return [2 * p + g for p in range(NP_)]


def _rope_tables():
    inv = 1.0 / (10000.0 ** (np.arange(0, D, 2, dtype=np.float32) / D))
    f = np.arange(N, dtype=np.float32)[:, None] * inv[None, :]
    sin = np.concatenate([np.sin(f), np.sin(f)], axis=-1).astype(np.float32)
    cos = np.concatenate([np.cos(f), np.cos(f)], axis=-1).astype(np.float32)
    return sin, cos  # [N, D]


def _st_table(sin):
    # sign-folded, half-swapped sin table, indexed by SOURCE row r: after the
    # XOR-32 partition permutation, dest row d gets rot_half(q)[d]*sin[d].
    st = np.empty_like(sin)            # [N, D]
    st[:, 0:32] = sin[:, 32:64]
    st[:, 32:64] = -sin[:, 0:32]
    return st


def _shared_inputs(qkv_w, proj_w, proj_b):
    # [I | P32]: P32 is the XOR-32 partition permutation (within 64-blocks)
    shifteye = np.zeros((128, 256), np.float32)
    shifteye[:, 0:128] = np.eye(128)
    for r in range(128):
        shifteye[r, 128 + (r ^ 32)] = 1.0

    # master ALiBi pattern: master[k, x] = min(k - x, 0); the tile for
    # (jt, block i0) is the column window shifted by o = i0 - 128*jt
    jl = np.arange(128, dtype=np.float32)[:, None]
    xl = np.arange(2048, dtype=np.float32)[None, :]
    master = np.minimum(jl - xl, 0.0).astype(NPBF)

    sin, cos = _rope_tables()
    cos2 = np.tile(cos.T, (2, 1)).astype(NPBF)    # [128, N]
    ssin2 = np.tile(_st_table(sin).T, (2, 1)).astype(NPBF)
    return {
        "shifteye": shifteye.astype(NPBF),
        "master": master,
        "cos2": cos2, "ssin2": ssin2,
    }, qkv_w, (proj_w, proj_b)


def _core_inputs(c, x, shared, qkv_w, proj):
    proj_w, proj_b = proj
    b, g = c // 2, c % 2
    heads = _heads(g)
    cols = np.concatenate([np.arange(64 * h, 64 * h + 64) for h in heads])

    wqT = np.ascontiguousarray(qkv_w[0:C].T)[:, cols] * SCALE     # [C, 256]
    wkT = np.ascontiguousarray(qkv_w[C:2 * C].T)[:, cols]
    wvT = np.ascontiguousarray(qkv_w[2 * C:3 * C].T)[:, cols]
    wcat = np.concatenate([wqT, wkT, wvT], axis=1).astype(NPBF)   # [C, 768]

    c8eye = np.zeros((NP_, 128, 128), np.float32)
    for p in range(NP_):
        np.fill_diagonal(c8eye[p], C8[heads[p]])

    projwt = np.ascontiguousarray(proj_w.T)[cols, :]              # [256, C]
    biasb = np.tile(proj_b[None, :], (128, 1)) if g == 0 else \
        np.zeros((128, C), np.float32)

    return {
        "xt": np.ascontiguousarray(x[b].T).astype(NPBF),          # [C, N]
        "wcat": wcat,
        "c8eye": c8eye.astype(NPBF),
        "projwt": projwt.astype(NPBF),
        "biasb": biasb.astype(np.float32),
        **shared,
    }


def _build_graph():
    nc = bacc.Bacc("TRN2", target_bir_lowering=False, debug=False,
                   num_devices=NCORES)

    xt_d = nc.dram_tensor("xt", [C, N], BF16, kind="ExternalInput").ap()
    wcat_d = nc.dram_tensor("wcat", [C, 768], BF16, kind="ExternalInput").ap()
    cos2_d = nc.dram_tensor("cos2", [128, N], BF16, kind="ExternalInput").ap()
    ssin2_d = nc.dram_tensor("ssin2", [128, N], BF16, kind="ExternalInput").ap()
    shifteye_d = nc.dram_tensor("shifteye", [128, 256], BF16, kind="ExternalInput").ap()
    c8eye_d = nc.dram_tensor("c8eye", [NP_, 128, 128], BF16, kind="ExternalInput").ap()
    master_d = nc.dram_tensor("master", [128, 2048], BF16, kind="ExternalInput").ap()
    projwt_d = nc.dram_tensor("projwt", [256, C], BF16, kind="ExternalInput").ap()
    biasb_d = nc.dram_tensor("biasb", [128, C], F32, kind="ExternalInput").ap()
    out_d = nc.dram_tensor("out", [N, C], BF16, kind="ExternalOutput").ap()

    with tile.TileContext(nc) as tc:
        _body(nc, tc, xt_d, wcat_d, cos2_d, ssin2_d, shifteye_d, c8eye_d,
              master_d, projwt_d, biasb_d, out_d)
    nc.compile()
    return nc


def _body(nc, tc, xt_d, wcat_d, cos2_d, ssin2_d, shifteye_d, c8eye_d,
          master_d, projwt_d, biasb_d, out_d):
    from contextlib import ExitStack
    ctx = ExitStack()
    persist = ctx.enter_context(tc.tile_pool(name="persist", bufs=1))
    rope_pool = ctx.enter_context(tc.tile_pool(name="rope", bufs=2))
    exp_pool = ctx.enter_context(tc.tile_pool(name="exp", bufs=5))
    norm_pool = ctx.enter_context(tc.tile_pool(name="norm", bufs=2))
    fin_pool = ctx.enter_context(tc.tile_pool(name="final", bufs=4))
    pspool = ctx.enter_context(tc.tile_pool(name="ps", bufs=2, space="PSUM"))

    def ptile(shape, dtype, tag):
        return persist.tile(shape, dtype, tag=tag, name=tag)

    Exp = mybir.ActivationFunctionType.Exp

    # ---- persistent SBUF tiles ----
    w_sb = [ptile([128, 768], BF16, f"w{i}") for i in range(4)]
    xt_sb = [ptile([128, N], BF16, f"xt{i}") for i in range(4)]
    cos2 = ptile([128, N], BF16, "cos2")
    ssin2 = ptile([128, N], BF16, "ssin2")
    shifteye = ptile([128, 256], BF16, "shifteye")
    c8eye_sb = [ptile([128, 128], BF16, f"c8e{p}") for p in range(NP_)]
    master = ptile([128, 2048], BF16, "master")
    projw_sb = [ptile([128, C], BF16, f"pw{bd}") for bd in range(2)]
    biasb = ptile([128, C], F32, "biasb")
    q2_sb = [ptile([128, N], BF16, f"q2_{bd}") for bd in range(2)]
    k2_sb = [ptile([128, N], BF16, f"k2_{bd}") for bd in range(2)]
    v_sb = [ptile([128, NP_ * 65], BF16, f"v_{nt}") for nt in range(JT)]
    out_pair = [ptile([128, N], BF16, f"op_{bd}") for bd in range(2)]

    # ---- input DMAs, ordered to feed the PE emission order below ----
    # 1) V weights interleaved with the high xt columns so the first V
    # matmul starts after two transfers (V tiles run jt=15..0)
    for i in range(4):
        nc.sync.dma_start(w_sb[i][:, 512:768],
                          wcat_d[i * 128:(i + 1) * 128, 512:768])
        nc.sync.dma_start(xt_sb[i][:, 1536:2048],
                          xt_d[i * 128:(i + 1) * 128, 1536:2048])
    # 2) q/k weights + rope tables
    for i in range(4):
        nc.sync.dma_start(w_sb[i][:, 0:512], wcat_d[i * 128:(i + 1) * 128, 0:512])
    nc.sync.dma_start(shifteye[:], shifteye_d[:])
    nc.sync.dma_start(cos2[:], cos2_d[:])
    nc.sync.dma_start(ssin2[:], ssin2_d[:])
    # 3) remaining xt (descending), bias tables, proj weights
    for blk in (2, 1, 0):
        for i in range(4):
            nc.sync.dma_start(xt_sb[i][:, blk * 512:(blk + 1) * 512],
                              xt_d[i * 128:(i + 1) * 128, blk * 512:(blk + 1) * 512])
    nc.sync.dma_start(master[:], master_d[:])
    for p in range(NP_):
        nc.sync.dma_start(c8eye_sb[p][:], c8eye_d[p])
    for bd in range(2):
        nc.sync.dma_start(projw_sb[bd][:], projwt_d[bd * 128:(bd + 1) * 128, :])
    nc.sync.dma_start(biasb[:], biasb_d[:])

    # ---- helpers ----
    def v_tile(jt):
        psv = pspool.tile([128, 256], F32, tag="aux", name="psv")
        for ci in range(4):
            nc.tensor.matmul(
                psv[:], xt_sb[ci][:, jt * 128:(jt + 1) * 128],
                w_sb[ci][:, 512:768],
                start=(ci == 0), stop=(ci == 3))
        vdst = v_sb[jt].rearrange("p (h e) -> p h e", e=65)
        nc.vector.tensor_copy(vdst[:, :, 0:64],
                              psv.rearrange("p (h e) -> p h e", e=64))
        nc.gpsimd.memset(vdst[:, :, 64:65], 1.0)

    def qk_chunk_a(bd, kind, ch):
        # projection matmuls + cos/sin products for one 512-token chunk of
        # bundle bd (positions 2bd, 2bd+1), kind 0=q (scaled) 1=k.
        w_off = kind * 256 + bd * 128
        c0 = ch * 512
        ps_q = pspool.tile([128, 512], F32, tag="aux", name="ps_q")
        for ci in range(4):
            nc.tensor.matmul(
                ps_q[:],
                w_sb[ci][:, w_off:w_off + 128],
                xt_sb[ci][:, c0:c0 + 512],
                start=(ci == 0), stop=(ci == 3))
        tc_c = rope_pool.tile([128, 512], BF16, tag="tc", name="tc_c")
        nc.vector.tensor_mul(tc_c[:], ps_q[:], cos2[:, c0:c0 + 512])
        tc_u = rope_pool.tile([128, 512], BF16, tag="tu", name="tc_u")
        nc.vector.tensor_mul(tc_u[:], ps_q[:], ssin2[:, c0:c0 + 512])
        return tc_c, tc_u

    def qk_chunk_b(bd, kind, ch, tc_c, tc_u):
        # combine: dst = tc_c + P32 @ tc_u (partition-XOR-32 via matmul)
        dst_sb = k2_sb[bd] if kind else q2_sb[bd]
        c0 = ch * 512
        ps2 = pspool.tile([128, 512], F32, tag="aux", name="ps2")
        nc.tensor.matmul(ps2[:], shifteye[:, 0:128], tc_c[:],
                         start=True, stop=False)
        nc.tensor.matmul(ps2[:], shifteye[:, 128:256], tc_u[:],
                         start=False, stop=True)
        nc.vector.tensor_copy(dst_sb[:, c0:c0 + 512], ps2[:])

    # software-pipelined chunk list -> closures (B of chunk i rides with
    # A of chunk i+1 so the PE never waits on the DVE products)
    def chunk_closures(chunks):
        state = {}

        def make(i, spec):
            def run():
                if i > 0:
                    pb, pkd, pch = chunks[i - 1]
                    qk_chunk_b(pb, pkd, pch, *state.pop(i - 1))
                if spec is not None:
                    bd, kd, ch = spec
                    state[i] = qk_chunk_a(bd, kd, ch)
            return run

        return [make(i, spec)
                for i, spec in enumerate(list(chunks) + [None])]

    def bundle_chunks(bd):
        return [(bd, 1, 3), (bd, 1, 2), (bd, 1, 1), (bd, 1, 0),
                (bd, 0, 0), (bd, 0, 1), (bd, 0, 2), (bd, 0, 3)]

    # ---- PE pre-phase: V tiles (desc) interleaved with bundle-0 qk ----
    qk0 = chunk_closures(bundle_chunks(0))
    vt = [lambda jt=jt: v_tile(jt) for jt in range(JT - 1, -1, -1)]
    pre = [vt[0], vt[1], qk0[0], vt[2], vt[3], qk0[1], vt[4], vt[5], qk0[2],
           vt[6], vt[7], qk0[3], vt[8], vt[9], qk0[4], vt[10], vt[11],
           qk0[5], vt[12], vt[13], qk0[6], vt[14], vt[15]]
    for f in pre:
        f()

    # fillers: project bundle 1 while streaming the first two bundle-0 heads;
    # chunks 0..7 of the output projection (whose inputs complete with the
    # sp=0 streams) run inside the sp=1 streams, spreading the output DMAs.
    qk1 = chunk_closures(bundle_chunks(1))
    fillers = {(0, 0): qk1[:5], (0, 1): qk1[5:],
               (0, 2): [qk0[7], qk0[8]],
               (1, 1): [lambda chk=chk: proj_chunk(chk) for chk in range(0, 4)],
               (1, 2): [lambda chk=chk: proj_chunk(chk) for chk in range(4, 8)]}

    def proj_chunk(chk):
        # full output projection of one 128-query chunk (own heads' partial)
        ps = pspool.tile([128, 512], F32, tag="aux", name="ps_proj")
        for bd in range(2):
            nc.tensor.matmul(ps[:], out_pair[bd][:, chk * 128:(chk + 1) * 128],
                             projw_sb[bd][:],
                             start=(bd == 0), stop=(bd == 1),
                             skip_group_check=True)
        fin = fin_pool.tile([128, 512], BF16, tag="f", name="fin")
        nc.vector.tensor_add(fin[:], ps[:], biasb[:])
        nc.sync.dma_start(out_d[chk * 128:(chk + 1) * 128, :], fin[:])

    def norm_sub(sp, p, avt, qi):
        # attn-V is [query-part, head-dim]: the softmax denominator is a
        # per-partition scalar -> reciprocal + tensor_scalar multiply, then
        # PE-transpose back to [dim, query] for the projection.
        bd, row = p // 2, (p % 2) * 64
        col = 128 * (qi % 4)
        rec = norm_pool.tile([128, 1], F32, tag="rc", name="rec")
        nc.vector.reciprocal(rec[:], avt[:, col + 64:col + 65])
        stag = norm_pool.tile([128, 64], BF16, tag="st", name="stag")
        nc.vector.tensor_scalar_mul(stag[:], avt[:, col:col + 64], rec[:])
        pst = pspool.tile([64, 128], BF16, tag="aux", name="pst")
        nc.tensor.transpose(pst[:], stag[:], shifteye[:, 0:128])
        nc.vector.tensor_copy(
            out_pair[bd][row:row + 64,
                         1024 * sp + 128 * qi:1024 * sp + 128 * qi + 128],
            pst[:])

    # ---- attention streams: one per (slot-pair sp, position p) ----
    for sp in range(2):
        for p in range(NP_):
            bd, row = p // 2, (p % 2) * 64
            fl = fillers.get((sp, p), [])
            fi = 0
            jts = list(range(JT - 1, JTMIN[p][sp] - 1, -1))
            av01 = (pspool.tile([128, 512], F32, tag="av0", bufs=1, name="av0"),
                    pspool.tile([128, 512], F32, tag="av1", bufs=1, name="av1"))
            # PSUM accumulation state is per-bank on hardware: only read a
            # subtile once ALL subtiles sharing its bank have stopped.
            norm_at = {}
            norm_at.setdefault(STOPJT[p][sp][4], []).extend([4, 5, 6, 7])
            norm_at.setdefault(STOPJT[p][sp][0], []).extend([0, 1, 2, 3])
            pend = []  # [(jt, av-mm closure)], delayed two steps
            for idx, jt in enumerate(jts):
                while fi < len(fl) and fi * len(jts) <= idx * len(fl):
                    fl[fi]()
                    fi += 1
                qm0, qm1, ql = (QM[p][sp][0][jt], QM[p][sp][1][jt],
                                QLIM[p][sp][jt])
                br = BIASR[p][sp][jt]
                ps = pspool.tile([128, 1024], F32, tag="sc", name="ps_sc")
                nc.tensor.matmul(
                    ps[:, 0:qm0],
                    k2_sb[bd][row:row + 64, jt * 128:(jt + 1) * 128],
                    q2_sb[bd][row:row + 64, 1024 * sp:1024 * sp + qm0],
                    start=True, stop=(br is None), tile_position=(row, 0),
                    skip_group_check=True)
                if qm1 > 0:
                    nc.tensor.matmul(
                        ps[:, 512:512 + qm1],
                        k2_sb[bd][row:row + 64, jt * 128:(jt + 1) * 128],
                        q2_sb[bd][row:row + 64,
                                  1024 * sp + 512:1024 * sp + 512 + qm1],
                        start=True, stop=(br is None), tile_position=(row, 0),
                        skip_group_check=True)
                if br is not None:
                    o = 1024 * sp - 128 * jt
                    # split at the 512-col PSUM bank boundary
                    for lo, hi in ((br[0], min(br[1], 512)),
                                   (max(br[0], 512), br[1])):
                        if lo < hi:
                            nc.tensor.matmul(
                                ps[:, lo:hi], c8eye_sb[p][:],
                                master[:, lo + o:hi + o],
                                start=False, stop=True, tile_position=(0, 0),
                                skip_group_check=True)
                et = exp_pool.tile([128, 1024], BF16, tag="e", name="et")
                nc.scalar.activation(et[:, 0:ql], ps[:, 0:ql], Exp)

                def av_mms(jt=jt, ql=ql, et=et):
                    for qi in range((ql + 127) // 128):
                        m = min(128, ql - 128 * qi)
                        avt = av01[qi // 4]
                        col = 128 * (qi % 4)
                        nc.tensor.matmul(
                            avt[0:m, col:col + 65],
                            et[:, 128 * qi:128 * qi + m],
                            v_sb[jt][:, p * 65:(p + 1) * 65],
                            start=(jt == JT - 1 and qi % 4 == 0),
                            stop=(jt == STOPJT[p][sp][qi]),
                            skip_group_check=True)

                if len(pend) >= 4:
                    pjt, pfn = pend.pop(0)
                    pfn()
                    for qi in norm_at.get(pjt, []):
                        norm_sub(sp, p, av01[qi // 4], qi)
                pend.append((jt, av_mms))
            for pjt, pfn in pend:
                pfn()
                for qi in norm_at.get(pjt, []):
                    norm_sub(sp, p, av01[qi // 4], qi)
            while fi < len(fl):
                fl[fi]()
                fi += 1

    # ---- remaining output projection chunks (sp=1 queries) ----
    # software-pipelined: each chunk's bundle-0 partial runs during the final
    # norm drain; only the bundle-1 matmul waits on the last normalize.
    def proj_pre(chk):
        ps = pspool.tile([128, 512], F32, tag="aux", name="ps_proj")
        nc.tensor.matmul(ps[:], out_pair[0][:, chk * 128:(chk + 1) * 128],
                         projw_sb[0][:],
                         start=True, stop=False, skip_group_check=True)
        return ps

    def proj_fin(chk, ps):
        nc.tensor.matmul(ps[:], out_pair[1][:, chk * 128:(chk + 1) * 128],
                         projw_sb[1][:],
                         start=False, stop=True, skip_group_check=True)
        fin = fin_pool.tile([128, 512], BF16, tag="f", name="fin")
        nc.vector.tensor_add(fin[:], ps[:], biasb[:])
        nc.sync.dma_start(out_d[chk * 128:(chk + 1) * 128, :], fin[:])

    # slot1 chunks (12..15) are unblocked at the last stream's midpoint;
    # slot0 chunks (8..11) wait for the final normalize, so run them last.
    order = [12, 13, 14, 15, 8, 9, 10, 11]
    pend_ps = {order[0]: proj_pre(order[0]), order[1]: proj_pre(order[1])}
    for i, chk in enumerate(order):
        proj_fin(chk, pend_ps.pop(chk))
        if i + 2 < len(order):
            pend_ps[order[i + 2]] = proj_pre(order[i + 2])

    ctx.close()


@functools.lru_cache(maxsize=1)
def _graph():
    return _build_graph()


def kernel(x, qkv_w, proj_w, proj_b):
    global LAST_RESULT
    x = np.asarray(x, np.float32)
    qkv_w = np.asarray(qkv_w, np.float32)
    proj_w = np.asarray(proj_w, np.float32)
    proj_b = np.asarray(proj_b, np.float32)

    nc = _graph()
    shared, qw, proj = _shared_inputs(qkv_w, proj_w, proj_b)
    in_maps = [_core_inputs(c, x, shared, qw, proj) for c in range(NCORES)]
    res = bass_utils.run_bass_kernel_spmd(nc, in_maps,
                                          core_ids=list(range(NCORES)),
                                          trace=False)
    LAST_RESULT = res
    out = np.zeros((B, N, C), np.float32)
    for b in range(B):
        out[b] = (np.asarray(res.results[2 * b]["out"], np.float32)
                  + np.asarray(res.results[2 * b + 1]["out"], np.float32))
    return out


# revision 98
# speedup vs baseline: 1.0755x; 1.0062x over previous
"""Fused multi-head attention layer (RoPE + ALiBi + softmax + out-proj) on 8 TRN2 cores.

Sharding (v4, heads-split): core c -> (batch b = c//2, head group g = c%2).
Each core computes q/k/v for its 4 heads {g, 2+g, 4+g, 6+g} over ALL 2048
positions, runs banded attention, and projects through its heads' slice of
proj_w, producing a partial [N, C] output; the host sums the two partials
per batch. Pairing adjacent-radius heads per graph position keeps the SPMD
union of ALiBi bands tight, and query blocks have exact (not unioned)
trim bounds.

Pipeline features:
- RoPE rotate-half via a sign-folded sin table + partition-permutation
  matmul (no duplicate rot projections).
- ALiBi bias on PE as c8-scaled identity matmuls against a single shared
  anti-diagonal master pattern table (column-shifted AP views), trimmed to
  the band columns.
- Score/exp/attn-V column ranges prefix-trimmed per (position, block, jt).
- Streams software-pipelined; V/next-bundle projections fill PE gaps.
"""

import functools
import math
import os
import sys

import numpy as np

sys.path.insert(0, "/opt/trn_rl_repo")

import ml_dtypes  # noqa: E402

import concourse.bass as bass  # noqa: E402
import concourse.tile as tile  # noqa: E402
from concourse import bacc, mybir, bass_utils  # noqa: E402

BF16 = mybir.dt.bfloat16
F32 = mybir.dt.float32
NPBF = ml_dtypes.bfloat16

B, N, C, H, D = 4, 2048, 512, 8, 64
NCORES = 8
JT = N // 128        # 16 j-tiles of 128 key positions
NP_ = 4              # head positions per core
T_CUT = 30.0         # ALiBi cutoff in logits: exp(-30) is negligible
SCALE = D ** -0.5

# c8_h = alibi_slope_h * MAX_BIAS = 2^-(h+1) * 8 = 2^(2-h)
C8 = [2.0 ** (2 - h) for h in range(H)]
RADIUS = [T_CUT / c for c in C8]   # band reach (key positions) per head
# graph position p holds heads {2p, 2p+1}; the union band is the odd head's
UR = [RADIUS[2 * p + 1] for p in range(NP_)]

LAST_RESULT = None  # test harness reads exec_time_ns from here


def _clamp(v, lo, hi):
    return max(lo, min(hi, v))


# Frame for (position p, slot-pair sp): cols 0:512 = query block 2sp
# (i0 = 1024sp), cols 512:1024 = block 2sp+1 (i0 = 1024sp+512). Exact bounds.
def _qm(p, sp, sl, jt):
    i0 = 512 * (2 * sp + sl)
    return _clamp(int(math.floor(128 * jt + 127 + UR[p])) + 1 - i0, 0, 512)


QM = [[[[_qm(p, sp, sl, jt) for jt in range(JT)] for sl in range(2)]
       for sp in range(2)] for p in range(NP_)]
QLIM = [[[(QM[p][sp][0][jt] if QM[p][sp][0][jt] < 512
           else 512 + QM[p][sp][1][jt]) for jt in range(JT)]
         for sp in range(2)] for p in range(NP_)]
JTMIN = [[min(jt for jt in range(JT) if QLIM[p][sp][jt] > 0)
          for sp in range(2)] for p in range(NP_)]
# per 128-query subtile qi of the 1024-col frame: the last (smallest) jt
# whose kept prefix still reaches it (attn-V accumulation stop point)
STOPJT = [[[min(jt for jt in range(JT) if QLIM[p][sp][jt] > 128 * qi)
            for qi in range(8)] for sp in range(2)] for p in range(NP_)]


def _bias_range(p, sp, jt):
    qm0, qm1 = QM[p][sp][0][jt], QM[p][sp][1][jt]
    bs0 = max(0, 128 * jt - 1024 * sp + 1)
    bs1 = max(0, 128 * jt - 1024 * sp - 511)
    r = []
    if bs0 < qm0:
        r.append((bs0, qm0))
    if bs1 < qm1:
        r.append((512 + bs1, 512 + qm1))
    if len(r) == 2:
        assert r[0][1] == 512 and r[1][0] == 512, (p, sp, jt, r)
        r = [(r[0][0], r[1][1])]
    return r[0] if r else None


BIASR = [[[_bias_range(p, sp, jt) for jt in range(JT)] for sp in range(2)]
         for p in range(NP_)]


def _heads(g):
    return [2 * p + g for p in range(NP_)]


def _rope_tables():
    inv = 1.0 / (10000.0 ** (np.arange(0, D, 2, dtype=np.float32) / D))
    f = np.arange(N, dtype=np.float32)[:, None] * inv[None, :]
    sin = np.concatenate([np.sin(f), np.sin(f)], axis=-1).astype(np.float32)
    cos = np.concatenate([np.cos(f), np.cos(f)], axis=-1).astype(np.float32)
    return sin, cos  # [N, D]


def _st_table(sin):
    # sign-folded, half-swapped sin table, indexed by SOURCE row r: after the
    # XOR-32 partition permutation, dest row d gets rot_half(q)[d]*sin[d].
    st = np.empty_like(sin)            # [N, D]
    st[:, 0:32] = sin[:, 32:64]
    st[:, 32:64] = -sin[:, 0:32]
    return st


def _shared_inputs(qkv_w, proj_w, proj_b):
    # [I | P32]: P32 is the XOR-32 partition permutation (within 64-blocks)
    shifteye = np.zeros((128, 256), np.float32)
    shifteye[:, 0:128] = np.eye(128)
    for r in range(128):
        shifteye[r, 128 + (r ^ 32)] = 1.0

    # master ALiBi pattern: master[k, x] = min(k - x, 0); the tile for
    # (jt, block i0) is the column window shifted by o = i0 - 128*jt
    jl = np.arange(128, dtype=np.float32)[:, None]
    xl = np.arange(2048, dtype=np.float32)[None, :]
    master = np.minimum(jl - xl, 0.0).astype(NPBF)

    sin, cos = _rope_tables()
    cos2 = np.tile(cos.T, (2, 1)).astype(NPBF)    # [128, N]
    ssin2 = np.tile(_st_table(sin).T, (2, 1)).astype(NPBF)
    return {
        "shifteye": shifteye.astype(NPBF),
        "master": master,
        "cos2": cos2, "ssin2": ssin2,
    }, qkv_w, (proj_w, proj_b)


def _core_inputs(c, x, shared, qkv_w, proj):
    proj_w, proj_b = proj
    b, g = c // 2, c % 2
    heads = _heads(g)
    cols = np.concatenate([np.arange(64 * h, 64 * h + 64) for h in heads])

    wqT = np.ascontiguousarray(qkv_w[0:C].T)[:, cols] * SCALE     # [C, 256]
    wkT = np.ascontiguousarray(qkv_w[C:2 * C].T)[:, cols]
    wvT = np.ascontiguousarray(qkv_w[2 * C:3 * C].T)[:, cols]
    wcat = np.concatenate([wqT, wkT, wvT], axis=1).astype(NPBF)   # [C, 768]

    c8eye = np.zeros((NP_, 128, 128), np.float32)
    for p in range(NP_):
        np.fill_diagonal(c8eye[p], C8[heads[p]])

    projwt = np.ascontiguousarray(proj_w.T)[cols, :]              # [256, C]
    biasb = np.tile(proj_b[None, :], (128, 1)) if g == 0 else \
        np.zeros((128, C), np.float32)

    return {
        "xt": np.ascontiguousarray(x[b].T).astype(NPBF),          # [C, N]
        "wcat": wcat,
        "c8eye": c8eye.astype(NPBF),
        "projwt": projwt.astype(NPBF),
        "biasb": biasb.astype(np.float32),
        **shared,
    }


def _build_graph():
    nc = bacc.Bacc("TRN2", target_bir_lowering=False, debug=False,
                   num_devices=NCORES)

    xt_d = nc.dram_tensor("xt", [C, N], BF16, kind="ExternalInput").ap()
    wcat_d = nc.dram_tensor("wcat", [C, 768], BF16, kind="ExternalInput").ap()
    cos2_d = nc.dram_tensor("cos2", [128, N], BF16, kind="ExternalInput").ap()
    ssin2_d = nc.dram_tensor("ssin2", [128, N], BF16, kind="ExternalInput").ap()
    shifteye_d = nc.dram_tensor("shifteye", [128, 256], BF16, kind="ExternalInput").ap()
    c8eye_d = nc.dram_tensor("c8eye", [NP_, 128, 128], BF16, kind="ExternalInput").ap()
    master_d = nc.dram_tensor("master", [128, 2048], BF16, kind="ExternalInput").ap()
    projwt_d = nc.dram_tensor("projwt", [256, C], BF16, kind="ExternalInput").ap()
    biasb_d = nc.dram_tensor("biasb", [128, C], F32, kind="ExternalInput").ap()
    out_d = nc.dram_tensor("out", [N, C], BF16, kind="ExternalOutput").ap()

    with tile.TileContext(nc) as tc:
        _body(nc, tc, xt_d, wcat_d, cos2_d, ssin2_d, shifteye_d, c8eye_d,
              master_d, projwt_d, biasb_d, out_d)
    nc.compile()
    return nc


def _body(nc, tc, xt_d, wcat_d, cos2_d, ssin2_d, shifteye_d, c8eye_d,
          master_d, projwt_d, biasb_d, out_d):
    from contextlib import ExitStack
    ctx = ExitStack()
    persist = ctx.enter_context(tc.tile_pool(name="persist", bufs=1))
    rope_pool = ctx.enter_context(tc.tile_pool(name="rope", bufs=2))
    exp_pool = ctx.enter_context(tc.tile_pool(name="exp", bufs=5))
    norm_pool = ctx.enter_context(tc.tile_pool(name="norm", bufs=2))
    fin_pool = ctx.enter_context(tc.tile_pool(name="final", bufs=4))
    pspool = ctx.enter_context(tc.tile_pool(name="ps", bufs=2, space="PSUM"))

    def ptile(shape, dtype, tag):
        return persist.tile(shape, dtype, tag=tag, name=tag)

    Exp = mybir.ActivationFunctionType.Exp

    # ---- persistent SBUF tiles ----
    w_sb = [ptile([128, 768], BF16, f"w{i}") for i in range(4)]
    xt_sb = [ptile([128, N], BF16, f"xt{i}") for i in range(4)]
    cos2 = ptile([128, N], BF16, "cos2")
    ssin2 = ptile([128, N], BF16, "ssin2")
    shifteye = ptile([128, 256], BF16, "shifteye")
    c8eye_sb = [ptile([128, 128], BF16, f"c8e{p}") for p in range(NP_)]
    master = ptile([128, 2048], BF16, "master")
    projw_sb = [ptile([128, C], BF16, f"pw{bd}") for bd in range(2)]
    biasb = ptile([128, C], F32, "biasb")
    q2_sb = [ptile([128, N], BF16, f"q2_{bd}") for bd in range(2)]
    k2_sb = [ptile([128, N], BF16, f"k2_{bd}") for bd in range(2)]
    v_sb = [ptile([128, NP_ * 65], BF16, f"v_{nt}") for nt in range(JT)]
    out_pair = [ptile([128, N], BF16, f"op_{bd}") for bd in range(2)]

    # ---- input DMAs, ordered to feed the PE emission order below ----
    # 1) V weights interleaved with the high xt columns so the first V
    # matmul starts after two transfers (V tiles run jt=15..0)
    for i in range(4):
        nc.sync.dma_start(w_sb[i][:, 512:768],
                          wcat_d[i * 128:(i + 1) * 128, 512:768])
        nc.sync.dma_start(xt_sb[i][:, 1536:2048],
                          xt_d[i * 128:(i + 1) * 128, 1536:2048])
    # 2) q/k weights + rope tables
    for i in range(4):
        nc.sync.dma_start(w_sb[i][:, 0:512], wcat_d[i * 128:(i + 1) * 128, 0:512])
    nc.sync.dma_start(shifteye[:], shifteye_d[:])
    nc.sync.dma_start(cos2[:], cos2_d[:])
    nc.sync.dma_start(ssin2[:], ssin2_d[:])
    # 3) remaining xt (descending), bias tables, proj weights
    for blk in (2, 1, 0):
        for i in range(4):
            nc.sync.dma_start(xt_sb[i][:, blk * 512:(blk + 1) * 512],
                              xt_d[i * 128:(i + 1) * 128, blk * 512:(blk + 1) * 512])
    nc.sync.dma_start(master[:], master_d[:])
    for p in range(NP_):
        nc.sync.dma_start(c8eye_sb[p][:], c8eye_d[p])
    for bd in range(2):
        nc.sync.dma_start(projw_sb[bd][:], projwt_d[bd * 128:(bd + 1) * 128, :])
    nc.sync.dma_start(biasb[:], biasb_d[:])

    # ---- helpers ----
    def v_tile(jt):
        psv = pspool.tile([128, 256], F32, tag="aux", name="psv")
        for ci in range(4):
            nc.tensor.matmul(
                psv[:], xt_sb[ci][:, jt * 128:(jt + 1) * 128],
                w_sb[ci][:, 512:768],
                start=(ci == 0), stop=(ci == 3))
        vdst = v_sb[jt].rearrange("p (h e) -> p h e", e=65)
        nc.vector.tensor_copy(vdst[:, :, 0:64],
                              psv.rearrange("p (h e) -> p h e", e=64))
        nc.gpsimd.memset(vdst[:, :, 64:65], 1.0)

    def qk_chunk_a(bd, kind, ch):
        # projection matmuls + cos/sin products for one 512-token chunk of
        # bundle bd (positions 2bd, 2bd+1), kind 0=q (scaled) 1=k.
        w_off = kind * 256 + bd * 128
        c0 = ch * 512
        ps_q = pspool.tile([128, 512], F32, tag="aux", name="ps_q")
        for ci in range(4):
            nc.tensor.matmul(
                ps_q[:],
                w_sb[ci][:, w_off:w_off + 128],
                xt_sb[ci][:, c0:c0 + 512],
                start=(ci == 0), stop=(ci == 3))
        tc_c = rope_pool.tile([128, 512], BF16, tag="tc", name="tc_c")
        nc.vector.tensor_mul(tc_c[:], ps_q[:], cos2[:, c0:c0 + 512])
        tc_u = rope_pool.tile([128, 512], BF16, tag="tu", name="tc_u")
        nc.vector.tensor_mul(tc_u[:], ps_q[:], ssin2[:, c0:c0 + 512])
        return tc_c, tc_u

    def qk_chunk_b(bd, kind, ch, tc_c, tc_u):
        # combine: dst = tc_c + P32 @ tc_u (partition-XOR-32 via matmul)
        dst_sb = k2_sb[bd] if kind else q2_sb[bd]
        c0 = ch * 512
        ps2 = pspool.tile([128, 512], F32, tag="aux", name="ps2")
        nc.tensor.matmul(ps2[:], shifteye[:, 0:128], tc_c[:],
                         start=True, stop=False)
        nc.tensor.matmul(ps2[:], shifteye[:, 128:256], tc_u[:],
                         start=False, stop=True)
        nc.vector.tensor_copy(dst_sb[:, c0:c0 + 512], ps2[:])

    # software-pipelined chunk list -> closures (B of chunk i rides with
    # A of chunk i+1 so the PE never waits on the DVE products)
    def chunk_closures(chunks):
        state = {}

        def make(i, spec):
            def run():
                if i > 0:
                    pb, pkd, pch = chunks[i - 1]
                    qk_chunk_b(pb, pkd, pch, *state.pop(i - 1))
                if spec is not None:
                    bd, kd, ch = spec
                    state[i] = qk_chunk_a(bd, kd, ch)
            return run

        return [make(i, spec)
                for i, spec in enumerate(list(chunks) + [None])]

    def bundle_chunks(bd):
        return [(bd, 1, 3), (bd, 1, 2), (bd, 1, 1), (bd, 1, 0),
                (bd, 0, 0), (bd, 0, 1), (bd, 0, 2), (bd, 0, 3)]

    # ---- PE pre-phase: V tiles (desc) interleaved with bundle-0 qk ----
    qk0 = chunk_closures(bundle_chunks(0))
    vt = [lambda jt=jt: v_tile(jt) for jt in range(JT - 1, -1, -1)]
    pre = [vt[0], vt[1], qk0[0], vt[2], vt[3], qk0[1], vt[4], vt[5], qk0[2],
           vt[6], vt[7], qk0[3], vt[8], vt[9], qk0[4], vt[10], vt[11],
           qk0[5], vt[12], vt[13], qk0[6], vt[14], vt[15]]
    for f in pre:
        f()

    # fillers: project bundle 1 while streaming the first two bundle-0 heads;
    # chunks 0..7 of the output projection (whose inputs complete with the
    # sp=0 streams) run inside the sp=1 streams, spreading the output DMAs.
    qk1 = chunk_closures(bundle_chunks(1))
    fillers = {(0, 0): qk1[:5], (0, 1): qk1[5:],
               (0, 2): [qk0[7], qk0[8]],
               (1, 1): [lambda chk=chk: proj_chunk(chk) for chk in range(0, 4)],
               (1, 2): [lambda chk=chk: proj_chunk(chk) for chk in range(4, 8)]}

    def proj_chunk(chk):
        # full output projection of one 128-query chunk (own heads' partial)
        ps = pspool.tile([128, 512], F32, tag="aux", name="ps_proj")
        for bd in range(2):
            nc.tensor.matmul(ps[:], out_pair[bd][:, chk * 128:(chk + 1) * 128],
                             projw_sb[bd][:],
                             start=(bd == 0), stop=(bd == 1),
                             skip_group_check=True)
        fin = fin_pool.tile([128, 512], BF16, tag="f", name="fin")
        nc.vector.tensor_add(fin[:], ps[:], biasb[:])
        nc.sync.dma_start(out_d[chk * 128:(chk + 1) * 128, :], fin[:])

    def norm_sub(sp, p, avt, qi):
        # attn-V is [query-part, head-dim]: the softmax denominator is a
        # per-partition scalar -> reciprocal + tensor_scalar multiply, then
        # PE-transpose back to [dim, query] for the projection.
        bd, row = p // 2, (p % 2) * 64
        col = 128 * (qi % 4)
        rec = norm_pool.tile([128, 1], F32, tag="rc", name="rec")
        nc.vector.reciprocal(rec[:], avt[:, col + 64:col + 65])
        stag = norm_pool.tile([128, 64], BF16, tag="st", name="stag")
        nc.vector.tensor_scalar_mul(stag[:], avt[:, col:col + 64], rec[:])
        pst = pspool.tile([64, 128], BF16, tag="aux", name="pst")
        nc.tensor.transpose(pst[:], stag[:], shifteye[:, 0:128])
        nc.vector.tensor_copy(
            out_pair[bd][row:row + 64,
                         1024 * sp + 128 * qi:1024 * sp + 128 * qi + 128],
            pst[:])

    # ---- attention streams: one per (slot-pair sp, position p) ----
    for sp in range(2):
        for p in range(NP_):
            bd, row = p // 2, (p % 2) * 64
            fl = fillers.get((sp, p), [])
            fi = 0
            jts = list(range(JT - 1, JTMIN[p][sp] - 1, -1))
            av01 = (pspool.tile([128, 512], F32, tag="av0", bufs=1, name="av0"),
                    pspool.tile([128, 512], F32, tag="av1", bufs=1, name="av1"))
            # PSUM accumulation state is per-bank on hardware: only read a
            # subtile once ALL subtiles sharing its bank have stopped.
            norm_at = {}
            norm_at.setdefault(STOPJT[p][sp][4], []).extend([4, 5, 6, 7])
            norm_at.setdefault(STOPJT[p][sp][0], []).extend([0, 1, 2, 3])
            pend = []  # [(jt, av-mm closure)], delayed two steps
            for idx, jt in enumerate(jts):
                while fi < len(fl) and (fi + 1) * len(jts) <= idx * len(fl):
                    fl[fi]()
                    fi += 1
                qm0, qm1, ql = (QM[p][sp][0][jt], QM[p][sp][1][jt],
                                QLIM[p][sp][jt])
                br = BIASR[p][sp][jt]
                ps = pspool.tile([128, 1024], F32, tag="sc", name="ps_sc")
                nc.tensor.matmul(
                    ps[:, 0:qm0],
                    k2_sb[bd][row:row + 64, jt * 128:(jt + 1) * 128],
                    q2_sb[bd][row:row + 64, 1024 * sp:1024 * sp + qm0],
                    start=True, stop=(br is None), tile_position=(row, 0),
                    skip_group_check=True)
                if qm1 > 0:
                    nc.tensor.matmul(
                        ps[:, 512:512 + qm1],
                        k2_sb[bd][row:row + 64, jt * 128:(jt + 1) * 128],
                        q2_sb[bd][row:row + 64,
                                  1024 * sp + 512:1024 * sp + 512 + qm1],
                        start=True, stop=(br is None), tile_position=(row, 0),
                        skip_group_check=True)
                if br is not None:
                    o = 1024 * sp - 128 * jt
                    # split at the 512-col PSUM bank boundary
                    for lo, hi in ((br[0], min(br[1], 512)),
                                   (max(br[0], 512), br[1])):
                        if lo < hi:
                            nc.tensor.matmul(
                                ps[:, lo:hi], c8eye_sb[p][:],
                                master[:, lo + o:hi + o],
                                start=False, stop=True, tile_position=(0, 0),
                                skip_group_check=True)
                et = exp_pool.tile([128, 1024], BF16, tag="e", name="et")
                nc.scalar.activation(et[:, 0:ql], ps[:, 0:ql], Exp)

                def av_mms(jt=jt, ql=ql, et=et):
                    for qi in range((ql + 127) // 128):
                        m = min(128, ql - 128 * qi)
                        avt = av01[qi // 4]
                        col = 128 * (qi % 4)
                        nc.tensor.matmul(
                            avt[0:m, col:col + 65],
                            et[:, 128 * qi:128 * qi + m],
                            v_sb[jt][:, p * 65:(p + 1) * 65],
                            start=(jt == JT - 1 and qi % 4 == 0),
                            stop=(jt == STOPJT[p][sp][qi]),
                            skip_group_check=True)

                if len(pend) >= 4:
                    pjt, pfn = pend.pop(0)
                    pfn()
                    for qi in norm_at.get(pjt, []):
                        norm_sub(sp, p, av01[qi // 4], qi)
                pend.append((jt, av_mms))
            for pjt, pfn in pend:
                pfn()
                for qi in norm_at.get(pjt, []):
                    norm_sub(sp, p, av01[qi // 4], qi)
            while fi < len(fl):
                fl[fi]()
                fi += 1

    # ---- remaining output projection chunks (sp=1 queries) ----
    # software-pipelined: each chunk's bundle-0 partial runs during the final
    # norm drain; only the bundle-1 matmul waits on the last normalize.
    def proj_pre(chk):
        ps = pspool.tile([128, 512], F32, tag="aux", name="ps_proj")
        nc.tensor.matmul(ps[:], out_pair[0][:, chk * 128:(chk + 1) * 128],
                         projw_sb[0][:],
                         start=True, stop=False, skip_group_check=True)
        return ps

    def proj_fin(chk, ps):
        nc.tensor.matmul(ps[:], out_pair[1][:, chk * 128:(chk + 1) * 128],
                         projw_sb[1][:],
                         start=False, stop=True, skip_group_check=True)
        fin = fin_pool.tile([128, 512], BF16, tag="f", name="fin")
        nc.vector.tensor_add(fin[:], ps[:], biasb[:])
        nc.sync.dma_start(out_d[chk * 128:(chk + 1) * 128, :], fin[:])

    # slot1 chunks (12..15) are unblocked at the last stream's midpoint;
    # slot0 chunks (8..11) wait for the final normalize, so run them last.
    order = [12, 13, 14, 15, 8, 9, 10, 11]
    pend_ps = {order[0]: proj_pre(order[0]), order[1]: proj_pre(order[1])}
    for i, chk in enumerate(order):
        proj_fin(chk, pend_ps.pop(chk))
        if i + 2 < len(order):
            pend_ps[order[i + 2]] = proj_pre(order[i + 2])

    ctx.close()


@functools.lru_cache(maxsize=1)
def _graph():
    return _build_graph()


def kernel(x, qkv_w, proj_w, proj_b):
    global LAST_RESULT
    x = np.asarray(x, np.float32)
    qkv_w = np.asarray(qkv_w, np.float32)
    proj_w = np.asarray(proj_w, np.float32)
    proj_b = np.asarray(proj_b, np.float32)

    nc = _graph()
    shared, qw, proj = _shared_inputs(qkv_w, proj_w, proj_b)
    in_maps = [_core_inputs(c, x, shared, qw, proj) for c in range(NCORES)]
    res = bass_utils.run_bass_kernel_spmd(nc, in_maps,
                                          core_ids=list(range(NCORES)),
                                          trace=False)
    LAST_RESULT = res
    out = np.zeros((B, N, C), np.float32)
    for b in range(B):
        out[b] = (np.asarray(res.results[2 * b]["out"], np.float32)
                  + np.asarray(res.results[2 * b + 1]["out"], np.float32))
    return out


# revision 103
# speedup vs baseline: 1.0802x; 1.0043x over previous
"""Fused multi-head attention layer (RoPE + ALiBi + softmax + out-proj) on 8 TRN2 cores.

Sharding (v4, heads-split): core c -> (batch b = c//2, head group g = c%2).
Each core computes q/k/v for its 4 heads {g, 2+g, 4+g, 6+g} over ALL 2048
positions, runs banded attention, and projects through its heads' slice of
proj_w, producing a partial [N, C] output; the host sums the two partials
per batch. Pairing adjacent-radius heads per graph position keeps the SPMD
union of ALiBi bands tight, and query blocks have exact (not unioned)
trim bounds.

Pipeline features:
- RoPE rotate-half via a sign-folded sin table + partition-permutation
  matmul (no duplicate rot projections).
- ALiBi bias on PE as c8-scaled identity matmuls against a single shared
  anti-diagonal master pattern table (column-shifted AP views), trimmed to
  the band columns.
- Score/exp/attn-V column ranges prefix-trimmed per (position, block, jt).
- Streams software-pipelined; V/next-bundle projections fill PE gaps.
"""

import functools
import math
import os
import sys

import numpy as np

sys.path.insert(0, "/opt/trn_rl_repo")

import ml_dtypes  # noqa: E402

import concourse.bass as bass  # noqa: E402
import concourse.tile as tile  # noqa: E402
from concourse import bacc, mybir, bass_utils  # noqa: E402

BF16 = mybir.dt.bfloat16
F32 = mybir.dt.float32
NPBF = ml_dtypes.bfloat16

B, N, C, H, D = 4, 2048, 512, 8, 64
NCORES = 8
JT = N // 128        # 16 j-tiles of 128 key positions
NP_ = 4              # head positions per core
T_CUT = 30.0         # ALiBi cutoff in logits: exp(-30) is negligible
SCALE = D ** -0.5

# c8_h = alibi_slope_h * MAX_BIAS = 2^-(h+1) * 8 = 2^(2-h)
C8 = [2.0 ** (2 - h) for h in range(H)]
RADIUS = [T_CUT / c for c in C8]   # band reach (key positions) per head
# graph position p holds heads {2p, 2p+1}; the union band is the odd head's
UR = [RADIUS[2 * p + 1] for p in range(NP_)]

LAST_RESULT = None  # test harness reads exec_time_ns from here


def _clamp(v, lo, hi):
    return max(lo, min(hi, v))


# Frame for (position p, slot-pair sp): cols 0:512 = query block 2sp
# (i0 = 1024sp), cols 512:1024 = block 2sp+1 (i0 = 1024sp+512). Exact bounds.
def _qm(p, sp, sl, jt):
    i0 = 512 * (2 * sp + sl)
    return _clamp(int(math.floor(128 * jt + 127 + UR[p])) + 1 - i0, 0, 512)


QM = [[[[_qm(p, sp, sl, jt) for jt in range(JT)] for sl in range(2)]
       for sp in range(2)] for p in range(NP_)]
QLIM = [[[(QM[p][sp][0][jt] if QM[p][sp][0][jt] < 512
           else 512 + QM[p][sp][1][jt]) for jt in range(JT)]
         for sp in range(2)] for p in range(NP_)]
JTMIN = [[min(jt for jt in range(JT) if QLIM[p][sp][jt] > 0)
          for sp in range(2)] for p in range(NP_)]
# per 128-query subtile qi of the 1024-col frame: the last (smallest) jt
# whose kept prefix still reaches it (attn-V accumulation stop point)
STOPJT = [[[min(jt for jt in range(JT) if QLIM[p][sp][jt] > 128 * qi)
            for qi in range(8)] for sp in range(2)] for p in range(NP_)]


def _bias_range(p, sp, jt):
    qm0, qm1 = QM[p][sp][0][jt], QM[p][sp][1][jt]
    bs0 = max(0, 128 * jt - 1024 * sp + 1)
    bs1 = max(0, 128 * jt - 1024 * sp - 511)
    r = []
    if bs0 < qm0:
        r.append((bs0, qm0))
    if bs1 < qm1:
        r.append((512 + bs1, 512 + qm1))
    if len(r) == 2:
        assert r[0][1] == 512 and r[1][0] == 512, (p, sp, jt, r)
        r = [(r[0][0], r[1][1])]
    return r[0] if r else None


BIASR = [[[_bias_range(p, sp, jt) for jt in range(JT)] for sp in range(2)]
         for p in range(NP_)]


def _heads(g):
    return [2 * p + g for p in range(NP_)]


def _rope_tables():
    inv = 1.0 / (10000.0 ** (np.arange(0, D, 2, dtype=np.float32) / D))
    f = np.arange(N, dtype=np.float32)[:, None] * inv[None, :]
    sin = np.concatenate([np.sin(f), np.sin(f)], axis=-1).astype(np.float32)
    cos = np.concatenate([np.cos(f), np.cos(f)], axis=-1).astype(np.float32)
    return sin, cos  # [N, D]


def _st_table(sin):
    # sign-folded, half-swapped sin table, indexed by SOURCE row r: after the
    # XOR-32 partition permutation, dest row d gets rot_half(q)[d]*sin[d].
    st = np.empty_like(sin)            # [N, D]
    st[:, 0:32] = sin[:, 32:64]
    st[:, 32:64] = -sin[:, 0:32]
    return st


def _shared_inputs(qkv_w, proj_w, proj_b):
    # [I | P32]: P32 is the XOR-32 partition permutation (within 64-blocks)
    shifteye = np.zeros((128, 256), np.float32)
    shifteye[:, 0:128] = np.eye(128)
    for r in range(128):
        shifteye[r, 128 + (r ^ 32)] = 1.0

    # master ALiBi pattern: master[k, x] = min(k - x, 0); the tile for
    # (jt, block i0) is the column window shifted by o = i0 - 128*jt
    jl = np.arange(128, dtype=np.float32)[:, None]
    xl = np.arange(2048, dtype=np.float32)[None, :]
    master = np.minimum(jl - xl, 0.0).astype(NPBF)

    sin, cos = _rope_tables()
    cos2 = np.tile(cos.T, (2, 1)).astype(NPBF)    # [128, N]
    ssin2 = np.tile(_st_table(sin).T, (2, 1)).astype(NPBF)
    return {
        "shifteye": shifteye.astype(NPBF),
        "master": master,
        "cos2": cos2, "ssin2": ssin2,
    }, qkv_w, (proj_w, proj_b)


def _core_inputs(c, x, shared, qkv_w, proj):
    proj_w, proj_b = proj
    b, g = c // 2, c % 2
    heads = _heads(g)
    cols = np.concatenate([np.arange(64 * h, 64 * h + 64) for h in heads])

    wqT = np.ascontiguousarray(qkv_w[0:C].T)[:, cols] * SCALE     # [C, 256]
    wkT = np.ascontiguousarray(qkv_w[C:2 * C].T)[:, cols]
    wvT = np.ascontiguousarray(qkv_w[2 * C:3 * C].T)[:, cols]
    wcat = np.concatenate([wqT, wkT, wvT], axis=1).astype(NPBF)   # [C, 768]

    c8eye = np.zeros((NP_, 128, 128), np.float32)
    for p in range(NP_):
        np.fill_diagonal(c8eye[p], C8[heads[p]])

    projwt = np.ascontiguousarray(proj_w.T)[cols, :]              # [256, C]
    biasb = np.tile(proj_b[None, :], (128, 1)) if g == 0 else \
        np.zeros((128, C), np.float32)

    return {
        "xt": np.ascontiguousarray(x[b].T).astype(NPBF),          # [C, N]
        "wcat": wcat,
        "c8eye": c8eye.astype(NPBF),
        "projwt": projwt.astype(NPBF),
        "biasb": biasb.astype(np.float32),
        **shared,
    }


def _build_graph():
    nc = bacc.Bacc("TRN2", target_bir_lowering=False, debug=False,
                   num_devices=NCORES)

    xt_d = nc.dram_tensor("xt", [C, N], BF16, kind="ExternalInput").ap()
    wcat_d = nc.dram_tensor("wcat", [C, 768], BF16, kind="ExternalInput").ap()
    cos2_d = nc.dram_tensor("cos2", [128, N], BF16, kind="ExternalInput").ap()
    ssin2_d = nc.dram_tensor("ssin2", [128, N], BF16, kind="ExternalInput").ap()
    shifteye_d = nc.dram_tensor("shifteye", [128, 256], BF16, kind="ExternalInput").ap()
    c8eye_d = nc.dram_tensor("c8eye", [NP_, 128, 128], BF16, kind="ExternalInput").ap()
    master_d = nc.dram_tensor("master", [128, 2048], BF16, kind="ExternalInput").ap()
    projwt_d = nc.dram_tensor("projwt", [256, C], BF16, kind="ExternalInput").ap()
    biasb_d = nc.dram_tensor("biasb", [128, C], F32, kind="ExternalInput").ap()
    out_d = nc.dram_tensor("out", [N, C], BF16, kind="ExternalOutput").ap()

    with tile.TileContext(nc) as tc:
        _body(nc, tc, xt_d, wcat_d, cos2_d, ssin2_d, shifteye_d, c8eye_d,
              master_d, projwt_d, biasb_d, out_d)
    nc.compile()
    return nc


def _body(nc, tc, xt_d, wcat_d, cos2_d, ssin2_d, shifteye_d, c8eye_d,
          master_d, projwt_d, biasb_d, out_d):
    from contextlib import ExitStack
    ctx = ExitStack()
    persist = ctx.enter_context(tc.tile_pool(name="persist", bufs=1))
    rope_pool = ctx.enter_context(tc.tile_pool(name="rope", bufs=2))
    exp_pool = ctx.enter_context(tc.tile_pool(name="exp", bufs=5))
    norm_pool = ctx.enter_context(tc.tile_pool(name="norm", bufs=2))
    fin_pool = ctx.enter_context(tc.tile_pool(name="final", bufs=8))
    pspool = ctx.enter_context(tc.tile_pool(name="ps", bufs=2, space="PSUM"))

    def ptile(shape, dtype, tag):
        return persist.tile(shape, dtype, tag=tag, name=tag)

    Exp = mybir.ActivationFunctionType.Exp

    # ---- persistent SBUF tiles ----
    w_sb = [ptile([128, 768], BF16, f"w{i}") for i in range(4)]
    xt_sb = [ptile([128, N], BF16, f"xt{i}") for i in range(4)]
    cos2 = ptile([128, N], BF16, "cos2")
    ssin2 = ptile([128, N], BF16, "ssin2")
    shifteye = ptile([128, 256], BF16, "shifteye")
    c8eye_sb = [ptile([128, 128], BF16, f"c8e{p}") for p in range(NP_)]
    master = ptile([128, 2048], BF16, "master")
    projw_sb = [ptile([128, C], BF16, f"pw{bd}") for bd in range(2)]
    biasb = ptile([128, C], F32, "biasb")
    q2_sb = [ptile([128, N], BF16, f"q2_{bd}") for bd in range(2)]
    k2_sb = [ptile([128, N], BF16, f"k2_{bd}") for bd in range(2)]
    v_sb = [ptile([128, NP_ * 65], BF16, f"v_{nt}") for nt in range(JT)]
    out_pair = [ptile([128, N], BF16, f"op_{bd}") for bd in range(2)]

    # ---- input DMAs, ordered to feed the PE emission order below ----
    # 1) V weights interleaved with the high xt columns so the first V
    # matmul starts after two transfers (V tiles run jt=15..0)
    for i in range(4):
        nc.sync.dma_start(w_sb[i][:, 512:768],
                          wcat_d[i * 128:(i + 1) * 128, 512:768])
        nc.sync.dma_start(xt_sb[i][:, 1536:2048],
                          xt_d[i * 128:(i + 1) * 128, 1536:2048])
    # 2) q/k weights + rope tables
    for i in range(4):
        nc.sync.dma_start(w_sb[i][:, 0:512], wcat_d[i * 128:(i + 1) * 128, 0:512])
    nc.sync.dma_start(shifteye[:], shifteye_d[:])
    nc.sync.dma_start(cos2[:], cos2_d[:])
    nc.sync.dma_start(ssin2[:], ssin2_d[:])
    # 3) remaining xt (descending), bias tables, proj weights
    for blk in (2, 1, 0):
        for i in range(4):
            nc.sync.dma_start(xt_sb[i][:, blk * 512:(blk + 1) * 512],
                              xt_d[i * 128:(i + 1) * 128, blk * 512:(blk + 1) * 512])
    nc.sync.dma_start(master[:], master_d[:])
    for p in range(NP_):
        nc.sync.dma_start(c8eye_sb[p][:], c8eye_d[p])
    for bd in range(2):
        nc.sync.dma_start(projw_sb[bd][:], projwt_d[bd * 128:(bd + 1) * 128, :])
    nc.sync.dma_start(biasb[:], biasb_d[:])

    # ---- helpers ----
    def v_tile(jt):
        psv = pspool.tile([128, 256], F32, tag="aux", name="psv")
        for ci in range(4):
            nc.tensor.matmul(
                psv[:], xt_sb[ci][:, jt * 128:(jt + 1) * 128],
                w_sb[ci][:, 512:768],
                start=(ci == 0), stop=(ci == 3))
        vdst = v_sb[jt].rearrange("p (h e) -> p h e", e=65)
        nc.vector.tensor_copy(vdst[:, :, 0:64],
                              psv.rearrange("p (h e) -> p h e", e=64))
        nc.gpsimd.memset(vdst[:, :, 64:65], 1.0)

    def qk_chunk_a(bd, kind, ch):
        # projection matmuls + cos/sin products for one 512-token chunk of
        # bundle bd (positions 2bd, 2bd+1), kind 0=q (scaled) 1=k.
        w_off = kind * 256 + bd * 128
        c0 = ch * 512
        ps_q = pspool.tile([128, 512], F32, tag="aux", name="ps_q")
        for ci in range(4):
            nc.tensor.matmul(
                ps_q[:],
                w_sb[ci][:, w_off:w_off + 128],
                xt_sb[ci][:, c0:c0 + 512],
                start=(ci == 0), stop=(ci == 3))
        tc_c = rope_pool.tile([128, 512], BF16, tag="tc", name="tc_c")
        nc.vector.tensor_mul(tc_c[:], ps_q[:], cos2[:, c0:c0 + 512])
        tc_u = rope_pool.tile([128, 512], BF16, tag="tu", name="tc_u")
        nc.vector.tensor_mul(tc_u[:], ps_q[:], ssin2[:, c0:c0 + 512])
        return tc_c, tc_u

    def qk_chunk_b(bd, kind, ch, tc_c, tc_u):
        # combine: dst = tc_c + P32 @ tc_u (partition-XOR-32 via matmul)
        dst_sb = k2_sb[bd] if kind else q2_sb[bd]
        c0 = ch * 512
        ps2 = pspool.tile([128, 512], F32, tag="aux", name="ps2")
        nc.tensor.matmul(ps2[:], shifteye[:, 0:128], tc_c[:],
                         start=True, stop=False)
        nc.tensor.matmul(ps2[:], shifteye[:, 128:256], tc_u[:],
                         start=False, stop=True)
        nc.vector.tensor_copy(dst_sb[:, c0:c0 + 512], ps2[:])

    # software-pipelined chunk list -> closures (B of chunk i rides with
    # A of chunk i+1 so the PE never waits on the DVE products)
    def chunk_closures(chunks):
        state = {}

        def make(i, spec):
            def run():
                if i > 0:
                    pb, pkd, pch = chunks[i - 1]
                    qk_chunk_b(pb, pkd, pch, *state.pop(i - 1))
                if spec is not None:
                    bd, kd, ch = spec
                    state[i] = qk_chunk_a(bd, kd, ch)
            return run

        return [make(i, spec)
                for i, spec in enumerate(list(chunks) + [None])]

    def bundle_chunks(bd):
        return [(bd, 1, 3), (bd, 1, 2), (bd, 1, 1), (bd, 1, 0),
                (bd, 0, 0), (bd, 0, 1), (bd, 0, 2), (bd, 0, 3)]

    # ---- PE pre-phase: V tiles (desc) interleaved with bundle-0 qk ----
    qk0 = chunk_closures(bundle_chunks(0))
    vt = [lambda jt=jt: v_tile(jt) for jt in range(JT - 1, -1, -1)]
    pre = [vt[0], vt[1], qk0[0], vt[2], vt[3], qk0[1], vt[4], vt[5], qk0[2],
           vt[6], vt[7], qk0[3], vt[8], vt[9], qk0[4], vt[10], vt[11],
           qk0[5], vt[12], vt[13], qk0[6], vt[14], vt[15]]
    for f in pre:
        f()

    # fillers: project bundle 1 while streaming the first two bundle-0 heads;
    # chunks 0..7 of the output projection (whose inputs complete with the
    # sp=0 streams) run inside the sp=1 streams, spreading the output DMAs.
    qk1 = chunk_closures(bundle_chunks(1))
    fillers = {(0, 0): qk1[:5], (0, 1): qk1[5:],
               (0, 2): [qk0[7], qk0[8]],
               (1, 1): [lambda chk=chk: proj_chunk(chk) for chk in range(0, 4)],
               (1, 2): [lambda chk=chk: proj_chunk(chk) for chk in range(4, 8)]}

    def proj_chunk(chk):
        # full output projection of one 128-query chunk (own heads' partial)
        ps = pspool.tile([128, 512], F32, tag="aux", name="ps_proj")
        for bd in range(2):
            nc.tensor.matmul(ps[:], out_pair[bd][:, chk * 128:(chk + 1) * 128],
                             projw_sb[bd][:],
                             start=(bd == 0), stop=(bd == 1),
                             skip_group_check=True)
        fin = fin_pool.tile([128, 512], BF16, tag="f", name="fin")
        nc.vector.tensor_add(fin[:], ps[:], biasb[:])
        nc.sync.dma_start(out_d[chk * 128:(chk + 1) * 128, :], fin[:])

    def norm_sub(sp, p, avt, qi):
        # attn-V is [query-part, head-dim]: the softmax denominator is a
        # per-partition scalar -> reciprocal + tensor_scalar multiply, then
        # PE-transpose back to [dim, query] for the projection.
        bd, row = p // 2, (p % 2) * 64
        col = 128 * (qi % 4)
        rec = norm_pool.tile([128, 1], F32, tag="rc", name="rec")
        nc.vector.reciprocal(rec[:], avt[:, col + 64:col + 65])
        stag = norm_pool.tile([128, 64], BF16, tag="st", name="stag")
        nc.vector.tensor_scalar_mul(stag[:], avt[:, col:col + 64], rec[:])
        pst = pspool.tile([64, 128], BF16, tag="aux", name="pst")
        nc.tensor.transpose(pst[:], stag[:], shifteye[:, 0:128])
        nc.vector.tensor_copy(
            out_pair[bd][row:row + 64,
                         1024 * sp + 128 * qi:1024 * sp + 128 * qi + 128],
            pst[:])

    # ---- attention streams: one per (slot-pair sp, position p) ----
    for sp in range(2):
        for p in range(NP_):
            bd, row = p // 2, (p % 2) * 64
            fl = fillers.get((sp, p), [])
            fi = 0
            jts = list(range(JT - 1, JTMIN[p][sp] - 1, -1))
            av01 = (pspool.tile([128, 512], F32, tag="av0", bufs=1, name="av0"),
                    pspool.tile([128, 512], F32, tag="av1", bufs=1, name="av1"))
            # PSUM accumulation state is per-bank on hardware: only read a
            # subtile once ALL subtiles sharing its bank have stopped.
            norm_at = {}
            norm_at.setdefault(STOPJT[p][sp][4], []).extend([4, 5, 6, 7])
            norm_at.setdefault(STOPJT[p][sp][0], []).extend([0, 1, 2, 3])
            pend = []  # [(jt, av-mm closure)], delayed two steps
            for idx, jt in enumerate(jts):
                while fi < len(fl) and (fi + 1) * len(jts) <= idx * len(fl):
                    fl[fi]()
                    fi += 1
                qm0, qm1, ql = (QM[p][sp][0][jt], QM[p][sp][1][jt],
                                QLIM[p][sp][jt])
                br = BIASR[p][sp][jt]
                ps = pspool.tile([128, 1024], F32, tag="sc", name="ps_sc")
                nc.tensor.matmul(
                    ps[:, 0:qm0],
                    k2_sb[bd][row:row + 64, jt * 128:(jt + 1) * 128],
                    q2_sb[bd][row:row + 64, 1024 * sp:1024 * sp + qm0],
                    start=True, stop=(br is None), tile_position=(row, 0),
                    skip_group_check=True)
                if qm1 > 0:
                    nc.tensor.matmul(
                        ps[:, 512:512 + qm1],
                        k2_sb[bd][row:row + 64, jt * 128:(jt + 1) * 128],
                        q2_sb[bd][row:row + 64,
                                  1024 * sp + 512:1024 * sp + 512 + qm1],
                        start=True, stop=(br is None), tile_position=(row, 0),
                        skip_group_check=True)
                if br is not None:
                    o = 1024 * sp - 128 * jt
                    # split at the 512-col PSUM bank boundary
                    for lo, hi in ((br[0], min(br[1], 512)),
                                   (max(br[0], 512), br[1])):
                        if lo < hi:
                            nc.tensor.matmul(
                                ps[:, lo:hi], c8eye_sb[p][:],
                                master[:, lo + o:hi + o],
                                start=False, stop=True, tile_position=(0, 0),
                                skip_group_check=True)
                et = exp_pool.tile([128, 1024], BF16, tag="e", name="et")
                nc.scalar.activation(et[:, 0:ql], ps[:, 0:ql], Exp)

                def av_mms(jt=jt, ql=ql, et=et):
                    for qi in range((ql + 127) // 128):
                        m = min(128, ql - 128 * qi)
                        avt = av01[qi // 4]
                        col = 128 * (qi % 4)
                        nc.tensor.matmul(
                            avt[0:m, col:col + 65],
                            et[:, 128 * qi:128 * qi + m],
                            v_sb[jt][:, p * 65:(p + 1) * 65],
                            start=(jt == JT - 1 and qi % 4 == 0),
                            stop=(jt == STOPJT[p][sp][qi]),
                            skip_group_check=True)

                if len(pend) >= 4:
                    pjt, pfn = pend.pop(0)
                    pfn()
                    for qi in norm_at.get(pjt, []):
                        norm_sub(sp, p, av01[qi // 4], qi)
                pend.append((jt, av_mms))
            for pjt, pfn in pend:
                pfn()
                for qi in norm_at.get(pjt, []):
                    norm_sub(sp, p, av01[qi // 4], qi)
            while fi < len(fl):
                fl[fi]()
                fi += 1

    # ---- remaining output projection chunks (sp=1 queries) ----
    # software-pipelined: each chunk's bundle-0 partial runs during the final
    # norm drain; only the bundle-1 matmul waits on the last normalize.
    def proj_pre(chk):
        ps = pspool.tile([128, 512], F32, tag="aux", name="ps_proj")
        nc.tensor.matmul(ps[:], out_pair[0][:, chk * 128:(chk + 1) * 128],
                         projw_sb[0][:],
                         start=True, stop=False, skip_group_check=True)
        return ps

    def proj_fin(chk, ps):
        nc.tensor.matmul(ps[:], out_pair[1][:, chk * 128:(chk + 1) * 128],
                         projw_sb[1][:],
                         start=False, stop=True, skip_group_check=True)
        fin = fin_pool.tile([128, 512], BF16, tag="f", name="fin")
        nc.vector.tensor_add(fin[:], ps[:], biasb[:])
        nc.sync.dma_start(out_d[chk * 128:(chk + 1) * 128, :], fin[:])

    # slot1 chunks (12..15) are unblocked at the last stream's midpoint;
    # slot0 chunks (8..11) wait for the final normalize, so run them last.
    order = [12, 13, 14, 15, 8, 9, 10, 11]
    pend_ps = {order[0]: proj_pre(order[0]), order[1]: proj_pre(order[1])}
    for i, chk in enumerate(order):
        proj_fin(chk, pend_ps.pop(chk))
        if i + 2 < len(order):
            pend_ps[order[i + 2]] = proj_pre(order[i + 2])

    ctx.close()


@functools.lru_cache(maxsize=1)
def _graph():
    return _build_graph()


def kernel(x, qkv_w, proj_w, proj_b):
    global LAST_RESULT
    x = np.asarray(x, np.float32)
    qkv_w = np.asarray(qkv_w, np.float32)
    proj_w = np.asarray(proj_w, np.float32)
    proj_b = np.asarray(proj_b, np.float32)

    nc = _graph()
    shared, qw, proj = _shared_inputs(qkv_w, proj_w, proj_b)
    in_maps = [_core_inputs(c, x, shared, qw, proj) for c in range(NCORES)]
    res = bass_utils.run_bass_kernel_spmd(nc, in_maps,
                                          core_ids=list(range(NCORES)),
                                          trace=False)
    LAST_RESULT = res
    out = np.zeros((B, N, C), np.float32)
    for b in range(B):
        out[b] = (np.asarray(res.results[2 * b]["out"], np.float32)
                  + np.asarray(res.results[2 * b + 1]["out"], np.float32))
    return out


# revision 107
# speedup vs baseline: 1.0848x; 1.0043x over previous
"""Fused multi-head attention layer (RoPE + ALiBi + softmax + out-proj) on 8 TRN2 cores.

Sharding (v4, heads-split): core c -> (batch b = c//2, head group g = c%2).
Each core computes q/k/v for its 4 heads {g, 2+g, 4+g, 6+g} over ALL 2048
positions, runs banded attention, and projects through its heads' slice of
proj_w, producing a partial [N, C] output; the host sums the two partials
per batch. Pairing adjacent-radius heads per graph position keeps the SPMD
union of ALiBi bands tight, and query blocks have exact (not unioned)
trim bounds.

Pipeline features:
- RoPE rotate-half via a sign-folded sin table + partition-permutation
  matmul (no duplicate rot projections).
- ALiBi bias on PE as c8-scaled identity matmuls against a single shared
  anti-diagonal master pattern table (column-shifted AP views), trimmed to
  the band columns.
- Score/exp/attn-V column ranges prefix-trimmed per (position, block, jt).
- Streams software-pipelined; V/next-bundle projections fill PE gaps.
"""

import functools
import math
import os
import sys

import numpy as np

sys.path.insert(0, "/opt/trn_rl_repo")

import ml_dtypes  # noqa: E402

import concourse.bass as bass  # noqa: E402
import concourse.tile as tile  # noqa: E402
from concourse import bacc, mybir, bass_utils  # noqa: E402

BF16 = mybir.dt.bfloat16
F32 = mybir.dt.float32
NPBF = ml_dtypes.bfloat16

B, N, C, H, D = 4, 2048, 512, 8, 64
NCORES = 8
JT = N // 128        # 16 j-tiles of 128 key positions
NP_ = 4              # head positions per core
T_CUT = 30.0         # ALiBi cutoff in logits: exp(-30) is negligible
SCALE = D ** -0.5

# c8_h = alibi_slope_h * MAX_BIAS = 2^-(h+1) * 8 = 2^(2-h)
C8 = [2.0 ** (2 - h) for h in range(H)]
RADIUS = [T_CUT / c for c in C8]   # band reach (key positions) per head
# graph position p holds heads {2p, 2p+1}; the union band is the odd head's
UR = [RADIUS[2 * p + 1] for p in range(NP_)]

LAST_RESULT = None  # test harness reads exec_time_ns from here


def _clamp(v, lo, hi):
    return max(lo, min(hi, v))


# Frame for (position p, slot-pair sp): cols 0:512 = query block 2sp
# (i0 = 1024sp), cols 512:1024 = block 2sp+1 (i0 = 1024sp+512). Exact bounds.
def _qm(p, sp, sl, jt):
    i0 = 512 * (2 * sp + sl)
    return _clamp(int(math.floor(128 * jt + 127 + UR[p])) + 1 - i0, 0, 512)


QM = [[[[_qm(p, sp, sl, jt) for jt in range(JT)] for sl in range(2)]
       for sp in range(2)] for p in range(NP_)]
QLIM = [[[(QM[p][sp][0][jt] if QM[p][sp][0][jt] < 512
           else 512 + QM[p][sp][1][jt]) for jt in range(JT)]
         for sp in range(2)] for p in range(NP_)]
JTMIN = [[min(jt for jt in range(JT) if QLIM[p][sp][jt] > 0)
          for sp in range(2)] for p in range(NP_)]
# per 128-query subtile qi of the 1024-col frame: the last (smallest) jt
# whose kept prefix still reaches it (attn-V accumulation stop point)
STOPJT = [[[min(jt for jt in range(JT) if QLIM[p][sp][jt] > 128 * qi)
            for qi in range(8)] for sp in range(2)] for p in range(NP_)]


def _bias_range(p, sp, jt):
    qm0, qm1 = QM[p][sp][0][jt], QM[p][sp][1][jt]
    bs0 = max(0, 128 * jt - 1024 * sp + 1)
    bs1 = max(0, 128 * jt - 1024 * sp - 511)
    r = []
    if bs0 < qm0:
        r.append((bs0, qm0))
    if bs1 < qm1:
        r.append((512 + bs1, 512 + qm1))
    if len(r) == 2:
        assert r[0][1] == 512 and r[1][0] == 512, (p, sp, jt, r)
        r = [(r[0][0], r[1][1])]
    return r[0] if r else None


BIASR = [[[_bias_range(p, sp, jt) for jt in range(JT)] for sp in range(2)]
         for p in range(NP_)]


def _heads(g):
    return [2 * p + g for p in range(NP_)]


def _rope_tables():
    inv = 1.0 / (10000.0 ** (np.arange(0, D, 2, dtype=np.float32) / D))
    f = np.arange(N, dtype=np.float32)[:, None] * inv[None, :]
    sin = np.concatenate([np.sin(f), np.sin(f)], axis=-1).astype(np.float32)
    cos = np.concatenate([np.cos(f), np.cos(f)], axis=-1).astype(np.float32)
    return sin, cos  # [N, D]


def _st_table(sin):
    # sign-folded, half-swapped sin table, indexed by SOURCE row r: after the
    # XOR-32 partition permutation, dest row d gets rot_half(q)[d]*sin[d].
    st = np.empty_like(sin)            # [N, D]
    st[:, 0:32] = sin[:, 32:64]
    st[:, 32:64] = -sin[:, 0:32]
    return st


def _shared_inputs(qkv_w, proj_w, proj_b):
    # [I | P32]: P32 is the XOR-32 partition permutation (within 64-blocks)
    shifteye = np.zeros((128, 256), np.float32)
    shifteye[:, 0:128] = np.eye(128)
    for r in range(128):
        shifteye[r, 128 + (r ^ 32)] = 1.0

    # master ALiBi pattern: master[k, x] = min(k - x, 0); the tile for
    # (jt, block i0) is the column window shifted by o = i0 - 128*jt
    jl = np.arange(128, dtype=np.float32)[:, None]
    xl = np.arange(2048, dtype=np.float32)[None, :]
    master = np.minimum(jl - xl, 0.0).astype(NPBF)

    sin, cos = _rope_tables()
    cos2 = np.tile(cos.T, (2, 1)).astype(NPBF)    # [128, N]
    ssin2 = np.tile(_st_table(sin).T, (2, 1)).astype(NPBF)
    return {
        "shifteye": shifteye.astype(NPBF),
        "master": master,
        "cos2": cos2, "ssin2": ssin2,
    }, qkv_w, (proj_w, proj_b)


def _core_inputs(c, x, shared, qkv_w, proj):
    proj_w, proj_b = proj
    b, g = c // 2, c % 2
    heads = _heads(g)
    cols = np.concatenate([np.arange(64 * h, 64 * h + 64) for h in heads])

    wqT = np.ascontiguousarray(qkv_w[0:C].T)[:, cols] * SCALE     # [C, 256]
    wkT = np.ascontiguousarray(qkv_w[C:2 * C].T)[:, cols]
    wvT = np.ascontiguousarray(qkv_w[2 * C:3 * C].T)[:, cols]
    wcat = np.concatenate([wqT, wkT, wvT], axis=1).astype(NPBF)   # [C, 768]

    c8eye = np.zeros((NP_, 128, 128), np.float32)
    for p in range(NP_):
        np.fill_diagonal(c8eye[p], C8[heads[p]])

    projwt = np.ascontiguousarray(proj_w.T)[cols, :]              # [256, C]
    biasb = np.tile(proj_b[None, :], (128, 1)) if g == 0 else \
        np.zeros((128, C), np.float32)

    return {
        "xt": np.ascontiguousarray(x[b].T).astype(NPBF),          # [C, N]
        "wcat": wcat,
        "c8eye": c8eye.astype(NPBF),
        "projwt": projwt.astype(NPBF),
        "biasb": biasb.astype(np.float32),
        **shared,
    }


def _build_graph():
    nc = bacc.Bacc("TRN2", target_bir_lowering=False, debug=False,
                   num_devices=NCORES)

    xt_d = nc.dram_tensor("xt", [C, N], BF16, kind="ExternalInput").ap()
    wcat_d = nc.dram_tensor("wcat", [C, 768], BF16, kind="ExternalInput").ap()
    cos2_d = nc.dram_tensor("cos2", [128, N], BF16, kind="ExternalInput").ap()
    ssin2_d = nc.dram_tensor("ssin2", [128, N], BF16, kind="ExternalInput").ap()
    shifteye_d = nc.dram_tensor("shifteye", [128, 256], BF16, kind="ExternalInput").ap()
    c8eye_d = nc.dram_tensor("c8eye", [NP_, 128, 128], BF16, kind="ExternalInput").ap()
    master_d = nc.dram_tensor("master", [128, 2048], BF16, kind="ExternalInput").ap()
    projwt_d = nc.dram_tensor("projwt", [256, C], BF16, kind="ExternalInput").ap()
    biasb_d = nc.dram_tensor("biasb", [128, C], F32, kind="ExternalInput").ap()
    out_d = nc.dram_tensor("out", [N, C], BF16, kind="ExternalOutput").ap()

    with tile.TileContext(nc) as tc:
        _body(nc, tc, xt_d, wcat_d, cos2_d, ssin2_d, shifteye_d, c8eye_d,
              master_d, projwt_d, biasb_d, out_d)
    nc.compile()
    return nc


def _body(nc, tc, xt_d, wcat_d, cos2_d, ssin2_d, shifteye_d, c8eye_d,
          master_d, projwt_d, biasb_d, out_d):
    from contextlib import ExitStack
    ctx = ExitStack()
    persist = ctx.enter_context(tc.tile_pool(name="persist", bufs=1))
    rope_pool = ctx.enter_context(tc.tile_pool(name="rope", bufs=2))
    exp_pool = ctx.enter_context(tc.tile_pool(name="exp", bufs=6))
    norm_pool = ctx.enter_context(tc.tile_pool(name="norm", bufs=4))
    fin_pool = ctx.enter_context(tc.tile_pool(name="final", bufs=8))
    pspool = ctx.enter_context(tc.tile_pool(name="ps", bufs=2, space="PSUM"))

    def ptile(shape, dtype, tag):
        return persist.tile(shape, dtype, tag=tag, name=tag)

    Exp = mybir.ActivationFunctionType.Exp

    # ---- persistent SBUF tiles ----
    w_sb = [ptile([128, 768], BF16, f"w{i}") for i in range(4)]
    xt_sb = [ptile([128, N], BF16, f"xt{i}") for i in range(4)]
    cos2 = ptile([128, N], BF16, "cos2")
    ssin2 = ptile([128, N], BF16, "ssin2")
    shifteye = ptile([128, 256], BF16, "shifteye")
    c8eye_sb = [ptile([128, 128], BF16, f"c8e{p}") for p in range(NP_)]
    master = ptile([128, 2048], BF16, "master")
    projw_sb = [ptile([128, C], BF16, f"pw{bd}") for bd in range(2)]
    biasb = ptile([128, C], F32, "biasb")
    q2_sb = [ptile([128, N], BF16, f"q2_{bd}") for bd in range(2)]
    k2_sb = [ptile([128, N], BF16, f"k2_{bd}") for bd in range(2)]
    v_sb = [ptile([128, NP_ * 65], BF16, f"v_{nt}") for nt in range(JT)]
    out_pair = [ptile([128, N], BF16, f"op_{bd}") for bd in range(2)]

    # ---- input DMAs, ordered to feed the PE emission order below ----
    # 1) V weights interleaved with the high xt columns so the first V
    # matmul starts after two transfers (V tiles run jt=15..0)
    for i in range(4):
        nc.sync.dma_start(w_sb[i][:, 512:768],
                          wcat_d[i * 128:(i + 1) * 128, 512:768])
        nc.sync.dma_start(xt_sb[i][:, 1536:2048],
                          xt_d[i * 128:(i + 1) * 128, 1536:2048])
    # 2) q/k weights + rope tables
    for i in range(4):
        nc.sync.dma_start(w_sb[i][:, 0:512], wcat_d[i * 128:(i + 1) * 128, 0:512])
    nc.sync.dma_start(shifteye[:], shifteye_d[:])
    nc.sync.dma_start(cos2[:], cos2_d[:])
    nc.sync.dma_start(ssin2[:], ssin2_d[:])
    # 3) remaining xt (descending), bias tables, proj weights
    for blk in (2, 1, 0):
        for i in range(4):
            nc.sync.dma_start(xt_sb[i][:, blk * 512:(blk + 1) * 512],
                              xt_d[i * 128:(i + 1) * 128, blk * 512:(blk + 1) * 512])
    nc.sync.dma_start(master[:], master_d[:])
    for p in range(NP_):
        nc.sync.dma_start(c8eye_sb[p][:], c8eye_d[p])
    for bd in range(2):
        nc.sync.dma_start(projw_sb[bd][:], projwt_d[bd * 128:(bd + 1) * 128, :])
    nc.sync.dma_start(biasb[:], biasb_d[:])

    # ---- helpers ----
    def v_tile(jt):
        psv = pspool.tile([128, 256], F32, tag="aux", name="psv")
        for ci in range(4):
            nc.tensor.matmul(
                psv[:], xt_sb[ci][:, jt * 128:(jt + 1) * 128],
                w_sb[ci][:, 512:768],
                start=(ci == 0), stop=(ci == 3))
        vdst = v_sb[jt].rearrange("p (h e) -> p h e", e=65)
        nc.vector.tensor_copy(vdst[:, :, 0:64],
                              psv.rearrange("p (h e) -> p h e", e=64))
        nc.gpsimd.memset(vdst[:, :, 64:65], 1.0)

    def qk_chunk_a(bd, kind, ch):
        # projection matmuls + cos/sin products for one 512-token chunk of
        # bundle bd (positions 2bd, 2bd+1), kind 0=q (scaled) 1=k.
        w_off = kind * 256 + bd * 128
        c0 = ch * 512
        ps_q = pspool.tile([128, 512], F32, tag="aux", name="ps_q")
        for ci in range(4):
            nc.tensor.matmul(
                ps_q[:],
                w_sb[ci][:, w_off:w_off + 128],
                xt_sb[ci][:, c0:c0 + 512],
                start=(ci == 0), stop=(ci == 3))
        tc_c = rope_pool.tile([128, 512], BF16, tag="tc", name="tc_c")
        nc.vector.tensor_mul(tc_c[:], ps_q[:], cos2[:, c0:c0 + 512])
        tc_u = rope_pool.tile([128, 512], BF16, tag="tu", name="tc_u")
        nc.vector.tensor_mul(tc_u[:], ps_q[:], ssin2[:, c0:c0 + 512])
        return tc_c, tc_u

    def qk_chunk_b(bd, kind, ch, tc_c, tc_u):
        # combine: dst = tc_c + P32 @ tc_u (partition-XOR-32 via matmul)
        dst_sb = k2_sb[bd] if kind else q2_sb[bd]
        c0 = ch * 512
        ps2 = pspool.tile([128, 512], F32, tag="aux", name="ps2")
        nc.tensor.matmul(ps2[:], shifteye[:, 0:128], tc_c[:],
                         start=True, stop=False)
        nc.tensor.matmul(ps2[:], shifteye[:, 128:256], tc_u[:],
                         start=False, stop=True)
        nc.vector.tensor_copy(dst_sb[:, c0:c0 + 512], ps2[:])

    # software-pipelined chunk list -> closures (B of chunk i rides with
    # A of chunk i+1 so the PE never waits on the DVE products)
    def chunk_closures(chunks):
        state = {}

        def make(i, spec):
            def run():
                if i > 0:
                    pb, pkd, pch = chunks[i - 1]
                    qk_chunk_b(pb, pkd, pch, *state.pop(i - 1))
                if spec is not None:
                    bd, kd, ch = spec
                    state[i] = qk_chunk_a(bd, kd, ch)
            return run

        return [make(i, spec)
                for i, spec in enumerate(list(chunks) + [None])]

    def bundle_chunks(bd):
        return [(bd, 1, 3), (bd, 1, 2), (bd, 1, 1), (bd, 1, 0),
                (bd, 0, 0), (bd, 0, 1), (bd, 0, 2), (bd, 0, 3)]

    # ---- PE pre-phase: V tiles (desc) interleaved with bundle-0 qk ----
    qk0 = chunk_closures(bundle_chunks(0))
    vt = [lambda jt=jt: v_tile(jt) for jt in range(JT - 1, -1, -1)]
    pre = [vt[0], vt[1], qk0[0], vt[2], vt[3], qk0[1], vt[4], vt[5], qk0[2],
           vt[6], vt[7], qk0[3], vt[8], vt[9], qk0[4], vt[10], vt[11],
           qk0[5], vt[12], vt[13], qk0[6], vt[14], vt[15]]
    for f in pre:
        f()

    # fillers: project bundle 1 while streaming the first two bundle-0 heads;
    # chunks 0..7 of the output projection (whose inputs complete with the
    # sp=0 streams) run inside the sp=1 streams, spreading the output DMAs.
    qk1 = chunk_closures(bundle_chunks(1))
    fillers = {(0, 0): qk1[:5], (0, 1): qk1[5:],
               (0, 2): [qk0[7], qk0[8]],
               (1, 1): [lambda chk=chk: proj_chunk(chk) for chk in range(0, 4)],
               (1, 2): [lambda chk=chk: proj_chunk(chk) for chk in range(4, 8)]}

    def proj_chunk(chk):
        # full output projection of one 128-query chunk (own heads' partial)
        ps = pspool.tile([128, 512], F32, tag="aux", name="ps_proj")
        for bd in range(2):
            nc.tensor.matmul(ps[:], out_pair[bd][:, chk * 128:(chk + 1) * 128],
                             projw_sb[bd][:],
                             start=(bd == 0), stop=(bd == 1),
                             skip_group_check=True)
        fin = fin_pool.tile([128, 512], BF16, tag="f", name="fin")
        nc.vector.tensor_add(fin[:], ps[:], biasb[:])
        nc.sync.dma_start(out_d[chk * 128:(chk + 1) * 128, :], fin[:])

    def norm_sub(sp, p, avt, qi):
        # attn-V is [query-part, head-dim]: the softmax denominator is a
        # per-partition scalar -> reciprocal + tensor_scalar multiply, then
        # PE-transpose back to [dim, query] for the projection.
        bd, row = p // 2, (p % 2) * 64
        col = 128 * (qi % 4)
        rec = norm_pool.tile([128, 1], F32, tag="rc", name="rec")
        nc.vector.reciprocal(rec[:], avt[:, col + 64:col + 65])
        stag = norm_pool.tile([128, 64], BF16, tag="st", name="stag")
        nc.vector.tensor_scalar_mul(stag[:], avt[:, col:col + 64], rec[:])
        pst = pspool.tile([64, 128], BF16, tag="aux", name="pst")
        nc.tensor.transpose(pst[:], stag[:], shifteye[:, 0:128])
        nc.vector.tensor_copy(
            out_pair[bd][row:row + 64,
                         1024 * sp + 128 * qi:1024 * sp + 128 * qi + 128],
            pst[:])

    # ---- attention streams: one per (slot-pair sp, position p) ----
    for sp in range(2):
        for p in range(NP_):
            bd, row = p // 2, (p % 2) * 64
            fl = fillers.get((sp, p), [])
            fi = 0
            jts = list(range(JT - 1, JTMIN[p][sp] - 1, -1))
            av01 = (pspool.tile([128, 512], F32, tag="av0", bufs=1, name="av0"),
                    pspool.tile([128, 512], F32, tag="av1", bufs=1, name="av1"))
            # PSUM accumulation state is per-bank on hardware: only read a
            # subtile once ALL subtiles sharing its bank have stopped.
            norm_at = {}
            norm_at.setdefault(STOPJT[p][sp][4], []).extend([4, 5, 6, 7])
            norm_at.setdefault(STOPJT[p][sp][0], []).extend([0, 1, 2, 3])
            pend = []  # [(jt, av-mm closure)], delayed two steps
            for idx, jt in enumerate(jts):
                while fi < len(fl) and (fi + 1) * len(jts) <= idx * len(fl):
                    fl[fi]()
                    fi += 1
                qm0, qm1, ql = (QM[p][sp][0][jt], QM[p][sp][1][jt],
                                QLIM[p][sp][jt])
                br = BIASR[p][sp][jt]
                ps = pspool.tile([128, 1024], F32, tag="sc", name="ps_sc")
                nc.tensor.matmul(
                    ps[:, 0:qm0],
                    k2_sb[bd][row:row + 64, jt * 128:(jt + 1) * 128],
                    q2_sb[bd][row:row + 64, 1024 * sp:1024 * sp + qm0],
                    start=True, stop=(br is None), tile_position=(row, 0),
                    skip_group_check=True)
                if qm1 > 0:
                    nc.tensor.matmul(
                        ps[:, 512:512 + qm1],
                        k2_sb[bd][row:row + 64, jt * 128:(jt + 1) * 128],
                        q2_sb[bd][row:row + 64,
                                  1024 * sp + 512:1024 * sp + 512 + qm1],
                        start=True, stop=(br is None), tile_position=(row, 0),
                        skip_group_check=True)
                if br is not None:
                    o = 1024 * sp - 128 * jt
                    # split at the 512-col PSUM bank boundary
                    for lo, hi in ((br[0], min(br[1], 512)),
                                   (max(br[0], 512), br[1])):
                        if lo < hi:
                            nc.tensor.matmul(
                                ps[:, lo:hi], c8eye_sb[p][:],
                                master[:, lo + o:hi + o],
                                start=False, stop=True, tile_position=(0, 0),
                                skip_group_check=True)
                et = exp_pool.tile([128, 1024], BF16, tag="e", name="et")
                nc.scalar.activation(et[:, 0:ql], ps[:, 0:ql], Exp)

                def av_mms(jt=jt, ql=ql, et=et):
                    for qi in range((ql + 127) // 128):
                        m = min(128, ql - 128 * qi)
                        avt = av01[qi // 4]
                        col = 128 * (qi % 4)
                        nc.tensor.matmul(
                            avt[0:m, col:col + 65],
                            et[:, 128 * qi:128 * qi + m],
                            v_sb[jt][:, p * 65:(p + 1) * 65],
                            start=(jt == JT - 1 and qi % 4 == 0),
                            stop=(jt == STOPJT[p][sp][qi]),
                            skip_group_check=True)

                if len(pend) >= 4:
                    pjt, pfn = pend.pop(0)
                    pfn()
                    for qi in norm_at.get(pjt, []):
                        norm_sub(sp, p, av01[qi // 4], qi)
                pend.append((jt, av_mms))
            for pjt, pfn in pend:
                pfn()
                for qi in norm_at.get(pjt, []):
                    norm_sub(sp, p, av01[qi // 4], qi)
            while fi < len(fl):
                fl[fi]()
                fi += 1

    # ---- remaining output projection chunks (sp=1 queries) ----
    # software-pipelined: each chunk's bundle-0 partial runs during the final
    # norm drain; only the bundle-1 matmul waits on the last normalize.
    def proj_pre(chk):
        ps = pspool.tile([128, 512], F32, tag="aux", name="ps_proj")
        nc.tensor.matmul(ps[:], out_pair[0][:, chk * 128:(chk + 1) * 128],
                         projw_sb[0][:],
                         start=True, stop=False, skip_group_check=True)
        return ps

    def proj_fin(chk, ps):
        nc.tensor.matmul(ps[:], out_pair[1][:, chk * 128:(chk + 1) * 128],
                         projw_sb[1][:],
                         start=False, stop=True, skip_group_check=True)
        fin = fin_pool.tile([128, 512], BF16, tag="f", name="fin")
        nc.vector.tensor_add(fin[:], ps[:], biasb[:])
        nc.sync.dma_start(out_d[chk * 128:(chk + 1) * 128, :], fin[:])

    # slot1 chunks (12..15) are unblocked at the last stream's midpoint;
    # slot0 chunks (8..11) wait for the final normalize, so run them last.
    order = [12, 13, 14, 15, 8, 9, 10, 11]
    pend_ps = {order[0]: proj_pre(order[0]), order[1]: proj_pre(order[1])}
    for i, chk in enumerate(order):
        proj_fin(chk, pend_ps.pop(chk))
        if i + 2 < len(order):
            pend_ps[order[i + 2]] = proj_pre(order[i + 2])

    ctx.close()


@functools.lru_cache(maxsize=1)
def _graph():
    return _build_graph()


def kernel(x, qkv_w, proj_w, proj_b):
    global LAST_RESULT
    x = np.asarray(x, np.float32)
    qkv_w = np.asarray(qkv_w, np.float32)
    proj_w = np.asarray(proj_w, np.float32)
    proj_b = np.asarray(proj_b, np.float32)

    nc = _graph()
    shared, qw, proj = _shared_inputs(qkv_w, proj_w, proj_b)
    in_maps = [_core_inputs(c, x, shared, qw, proj) for c in range(NCORES)]
    res = bass_utils.run_bass_kernel_spmd(nc, in_maps,
                                          core_ids=list(range(NCORES)),
                                          trace=False)
    LAST_RESULT = res
    out = np.zeros((B, N, C), np.float32)
    for b in range(B):
        out[b] = (np.asarray(res.results[2 * b]["out"], np.float32)
                  + np.asarray(res.results[2 * b + 1]["out"], np.float32))
    return out


# revision 110
# speedup vs baseline: 1.0894x; 1.0043x over previous
"""Fused multi-head attention layer (RoPE + ALiBi + softmax + out-proj) on 8 TRN2 cores.

Sharding (v4, heads-split): core c -> (batch b = c//2, head group g = c%2).
Each core computes q/k/v for its 4 heads {g, 2+g, 4+g, 6+g} over ALL 2048
positions, runs banded attention, and projects through its heads' slice of
proj_w, producing a partial [N, C] output; the host sums the two partials
per batch. Pairing adjacent-radius heads per graph position keeps the SPMD
union of ALiBi bands tight, and query blocks have exact (not unioned)
trim bounds.

Pipeline features:
- RoPE rotate-half via a sign-folded sin table + partition-permutation
  matmul (no duplicate rot projections).
- ALiBi bias on PE as c8-scaled identity matmuls against a single shared
  anti-diagonal master pattern table (column-shifted AP views), trimmed to
  the band columns.
- Score/exp/attn-V column ranges prefix-trimmed per (position, block, jt).
- Streams software-pipelined; V/next-bundle projections fill PE gaps.
"""

import functools
import math
import os
import sys

import numpy as np

sys.path.insert(0, "/opt/trn_rl_repo")

import ml_dtypes  # noqa: E402

import concourse.bass as bass  # noqa: E402
import concourse.tile as tile  # noqa: E402
from concourse import bacc, mybir, bass_utils  # noqa: E402

BF16 = mybir.dt.bfloat16
F32 = mybir.dt.float32
NPBF = ml_dtypes.bfloat16

B, N, C, H, D = 4, 2048, 512, 8, 64
NCORES = 8
JT = N // 128        # 16 j-tiles of 128 key positions
NP_ = 4              # head positions per core
T_CUT = 30.0         # ALiBi cutoff in logits: exp(-30) is negligible
SCALE = D ** -0.5

# c8_h = alibi_slope_h * MAX_BIAS = 2^-(h+1) * 8 = 2^(2-h)
C8 = [2.0 ** (2 - h) for h in range(H)]
RADIUS = [T_CUT / c for c in C8]   # band reach (key positions) per head
# graph position p holds heads {2p, 2p+1}; the union band is the odd head's
UR = [RADIUS[2 * p + 1] for p in range(NP_)]

LAST_RESULT = None  # test harness reads exec_time_ns from here


def _clamp(v, lo, hi):
    return max(lo, min(hi, v))


# Frame for (position p, slot-pair sp): cols 0:512 = query block 2sp
# (i0 = 1024sp), cols 512:1024 = block 2sp+1 (i0 = 1024sp+512). Exact bounds.
def _qm(p, sp, sl, jt):
    i0 = 512 * (2 * sp + sl)
    return _clamp(int(math.floor(128 * jt + 127 + UR[p])) + 1 - i0, 0, 512)


QM = [[[[_qm(p, sp, sl, jt) for jt in range(JT)] for sl in range(2)]
       for sp in range(2)] for p in range(NP_)]
QLIM = [[[(QM[p][sp][0][jt] if QM[p][sp][0][jt] < 512
           else 512 + QM[p][sp][1][jt]) for jt in range(JT)]
         for sp in range(2)] for p in range(NP_)]
JTMIN = [[min(jt for jt in range(JT) if QLIM[p][sp][jt] > 0)
          for sp in range(2)] for p in range(NP_)]
# per 128-query subtile qi of the 1024-col frame: the last (smallest) jt
# whose kept prefix still reaches it (attn-V accumulation stop point)
STOPJT = [[[min(jt for jt in range(JT) if QLIM[p][sp][jt] > 128 * qi)
            for qi in range(8)] for sp in range(2)] for p in range(NP_)]


def _bias_range(p, sp, jt):
    qm0, qm1 = QM[p][sp][0][jt], QM[p][sp][1][jt]
    bs0 = max(0, 128 * jt - 1024 * sp + 1)
    bs1 = max(0, 128 * jt - 1024 * sp - 511)
    r = []
    if bs0 < qm0:
        r.append((bs0, qm0))
    if bs1 < qm1:
        r.append((512 + bs1, 512 + qm1))
    if len(r) == 2:
        assert r[0][1] == 512 and r[1][0] == 512, (p, sp, jt, r)
        r = [(r[0][0], r[1][1])]
    return r[0] if r else None


BIASR = [[[_bias_range(p, sp, jt) for jt in range(JT)] for sp in range(2)]
         for p in range(NP_)]


def _heads(g):
    return [2 * p + g for p in range(NP_)]


def _rope_tables():
    inv = 1.0 / (10000.0 ** (np.arange(0, D, 2, dtype=np.float32) / D))
    f = np.arange(N, dtype=np.float32)[:, None] * inv[None, :]
    sin = np.concatenate([np.sin(f), np.sin(f)], axis=-1).astype(np.float32)
    cos = np.concatenate([np.cos(f), np.cos(f)], axis=-1).astype(np.float32)
    return sin, cos  # [N, D]


def _st_table(sin):
    # sign-folded, half-swapped sin table, indexed by SOURCE row r: after the
    # XOR-32 partition permutation, dest row d gets rot_half(q)[d]*sin[d].
    st = np.empty_like(sin)            # [N, D]
    st[:, 0:32] = sin[:, 32:64]
    st[:, 32:64] = -sin[:, 0:32]
    return st


def _shared_inputs(qkv_w, proj_w, proj_b):
    # [I | P32]: P32 is the XOR-32 partition permutation (within 64-blocks)
    shifteye = np.zeros((128, 256), np.float32)
    shifteye[:, 0:128] = np.eye(128)
    for r in range(128):
        shifteye[r, 128 + (r ^ 32)] = 1.0

    # master ALiBi pattern: master[k, x] = min(k - x, 0); the tile for
    # (jt, block i0) is the column window shifted by o = i0 - 128*jt
    jl = np.arange(128, dtype=np.float32)[:, None]
    xl = np.arange(2048, dtype=np.float32)[None, :]
    master = np.minimum(jl - xl, 0.0).astype(NPBF)

    sin, cos = _rope_tables()
    cos2 = np.tile(cos.T, (2, 1)).astype(NPBF)    # [128, N]
    ssin2 = np.tile(_st_table(sin).T, (2, 1)).astype(NPBF)
    return {
        "shifteye": shifteye.astype(NPBF),
        "master": master,
        "cos2": cos2, "ssin2": ssin2,
    }, qkv_w, (proj_w, proj_b)


def _core_inputs(c, x, shared, qkv_w, proj):
    proj_w, proj_b = proj
    b, g = c // 2, c % 2
    heads = _heads(g)
    cols = np.concatenate([np.arange(64 * h, 64 * h + 64) for h in heads])

    wqT = np.ascontiguousarray(qkv_w[0:C].T)[:, cols] * SCALE     # [C, 256]
    wkT = np.ascontiguousarray(qkv_w[C:2 * C].T)[:, cols]
    wvT = np.ascontiguousarray(qkv_w[2 * C:3 * C].T)[:, cols]
    wcat = np.concatenate([wqT, wkT, wvT], axis=1).astype(NPBF)   # [C, 768]

    c8eye = np.zeros((NP_, 128, 128), np.float32)
    for p in range(NP_):
        np.fill_diagonal(c8eye[p], C8[heads[p]])

    projwt = np.ascontiguousarray(proj_w.T)[cols, :]              # [256, C]
    biasb = np.tile(proj_b[None, :], (128, 1)) if g == 0 else \
        np.zeros((128, C), np.float32)

    return {
        "xt": np.ascontiguousarray(x[b].T).astype(NPBF),          # [C, N]
        "wcat": wcat,
        "c8eye": c8eye.astype(NPBF),
        "projwt": projwt.astype(NPBF),
        "biasb": biasb.astype(np.float32),
        **shared,
    }


def _build_graph():
    nc = bacc.Bacc("TRN2", target_bir_lowering=False, debug=False,
                   num_devices=NCORES)

    xt_d = nc.dram_tensor("xt", [C, N], BF16, kind="ExternalInput").ap()
    wcat_d = nc.dram_tensor("wcat", [C, 768], BF16, kind="ExternalInput").ap()
    cos2_d = nc.dram_tensor("cos2", [128, N], BF16, kind="ExternalInput").ap()
    ssin2_d = nc.dram_tensor("ssin2", [128, N], BF16, kind="ExternalInput").ap()
    shifteye_d = nc.dram_tensor("shifteye", [128, 256], BF16, kind="ExternalInput").ap()
    c8eye_d = nc.dram_tensor("c8eye", [NP_, 128, 128], BF16, kind="ExternalInput").ap()
    master_d = nc.dram_tensor("master", [128, 2048], BF16, kind="ExternalInput").ap()
    projwt_d = nc.dram_tensor("projwt", [256, C], BF16, kind="ExternalInput").ap()
    biasb_d = nc.dram_tensor("biasb", [128, C], F32, kind="ExternalInput").ap()
    out_d = nc.dram_tensor("out", [N, C], BF16, kind="ExternalOutput").ap()

    with tile.TileContext(nc) as tc:
        _body(nc, tc, xt_d, wcat_d, cos2_d, ssin2_d, shifteye_d, c8eye_d,
              master_d, projwt_d, biasb_d, out_d)
    nc.compile()
    return nc


def _body(nc, tc, xt_d, wcat_d, cos2_d, ssin2_d, shifteye_d, c8eye_d,
          master_d, projwt_d, biasb_d, out_d):
    from contextlib import ExitStack
    ctx = ExitStack()
    persist = ctx.enter_context(tc.tile_pool(name="persist", bufs=1))
    rope_pool = ctx.enter_context(tc.tile_pool(name="rope", bufs=2))
    exp_pool = ctx.enter_context(tc.tile_pool(name="exp", bufs=6))
    norm_pool = ctx.enter_context(tc.tile_pool(name="norm", bufs=4))
    fin_pool = ctx.enter_context(tc.tile_pool(name="final", bufs=8))
    pspool = ctx.enter_context(tc.tile_pool(name="ps", bufs=2, space="PSUM"))

    def ptile(shape, dtype, tag):
        return persist.tile(shape, dtype, tag=tag, name=tag)

    Exp = mybir.ActivationFunctionType.Exp

    # ---- persistent SBUF tiles ----
    w_sb = [ptile([128, 768], BF16, f"w{i}") for i in range(4)]
    xt_sb = [ptile([128, N], BF16, f"xt{i}") for i in range(4)]
    cos2 = ptile([128, N], BF16, "cos2")
    ssin2 = ptile([128, N], BF16, "ssin2")
    shifteye = ptile([128, 256], BF16, "shifteye")
    c8eye_sb = [ptile([128, 128], BF16, f"c8e{p}") for p in range(NP_)]
    master = ptile([128, 2048], BF16, "master")
    projw_sb = [ptile([128, C], BF16, f"pw{bd}") for bd in range(2)]
    biasb = ptile([128, C], F32, "biasb")
    q2_sb = [ptile([128, N], BF16, f"q2_{bd}") for bd in range(2)]
    k2_sb = [ptile([128, N], BF16, f"k2_{bd}") for bd in range(2)]
    v_sb = [ptile([128, NP_ * 65], BF16, f"v_{nt}") for nt in range(JT)]
    out_pair = [ptile([128, N], BF16, f"op_{bd}") for bd in range(2)]

    # ---- input DMAs, ordered to feed the PE emission order below ----
    # 1) V weights interleaved with the high xt columns so the first V
    # matmul starts after two transfers (V tiles run jt=15..0)
    for i in range(4):
        nc.sync.dma_start(w_sb[i][:, 512:768],
                          wcat_d[i * 128:(i + 1) * 128, 512:768])
        nc.sync.dma_start(xt_sb[i][:, 1536:2048],
                          xt_d[i * 128:(i + 1) * 128, 1536:2048])
    # 2) q/k weights + rope tables
    for i in range(4):
        nc.sync.dma_start(w_sb[i][:, 0:512], wcat_d[i * 128:(i + 1) * 128, 0:512])
    nc.sync.dma_start(shifteye[:], shifteye_d[:])
    nc.sync.dma_start(cos2[:], cos2_d[:])
    nc.sync.dma_start(ssin2[:], ssin2_d[:])
    # 3) remaining xt (descending), bias tables, proj weights
    for blk in (2, 1, 0):
        for i in range(4):
            nc.sync.dma_start(xt_sb[i][:, blk * 512:(blk + 1) * 512],
                              xt_d[i * 128:(i + 1) * 128, blk * 512:(blk + 1) * 512])
    nc.sync.dma_start(master[:], master_d[:])
    for p in range(NP_):
        nc.sync.dma_start(c8eye_sb[p][:], c8eye_d[p])
    for bd in range(2):
        nc.sync.dma_start(projw_sb[bd][:], projwt_d[bd * 128:(bd + 1) * 128, :])
    nc.sync.dma_start(biasb[:], biasb_d[:])

    # ---- helpers ----
    def v_tile(jt):
        psv = pspool.tile([128, 256], F32, tag="aux", name="psv")
        for ci in range(4):
            nc.tensor.matmul(
                psv[:], xt_sb[ci][:, jt * 128:(jt + 1) * 128],
                w_sb[ci][:, 512:768],
                start=(ci == 0), stop=(ci == 3))
        vdst = v_sb[jt].rearrange("p (h e) -> p h e", e=65)
        nc.vector.tensor_copy(vdst[:, :, 0:64],
                              psv.rearrange("p (h e) -> p h e", e=64))
        nc.gpsimd.memset(vdst[:, :, 64:65], 1.0)

    def qk_chunk_a(bd, kind, ch):
        # projection matmuls + cos/sin products for one 512-token chunk of
        # bundle bd (positions 2bd, 2bd+1), kind 0=q (scaled) 1=k.
        w_off = kind * 256 + bd * 128
        c0 = ch * 512
        ps_q = pspool.tile([128, 512], F32, tag="aux", name="ps_q")
        for ci in range(4):
            nc.tensor.matmul(
                ps_q[:],
                w_sb[ci][:, w_off:w_off + 128],
                xt_sb[ci][:, c0:c0 + 512],
                start=(ci == 0), stop=(ci == 3))
        tc_c = rope_pool.tile([128, 512], BF16, tag="tc", name="tc_c")
        nc.vector.tensor_mul(tc_c[:], ps_q[:], cos2[:, c0:c0 + 512])
        tc_u = rope_pool.tile([128, 512], BF16, tag="tu", name="tc_u")
        nc.vector.tensor_mul(tc_u[:], ps_q[:], ssin2[:, c0:c0 + 512])
        return tc_c, tc_u

    def qk_chunk_b(bd, kind, ch, tc_c, tc_u):
        # combine: dst = tc_c + P32 @ tc_u (partition-XOR-32 via matmul)
        dst_sb = k2_sb[bd] if kind else q2_sb[bd]
        c0 = ch * 512
        ps2 = pspool.tile([128, 512], F32, tag="aux", name="ps2")
        nc.tensor.matmul(ps2[:], shifteye[:, 0:128], tc_c[:],
                         start=True, stop=False)
        nc.tensor.matmul(ps2[:], shifteye[:, 128:256], tc_u[:],
                         start=False, stop=True)
        nc.vector.tensor_copy(dst_sb[:, c0:c0 + 512], ps2[:])

    # software-pipelined chunk list -> closures (B of chunk i rides with
    # A of chunk i+1 so the PE never waits on the DVE products)
    def chunk_closures(chunks):
        state = {}

        def make(i, spec):
            def run():
                if i > 0:
                    pb, pkd, pch = chunks[i - 1]
                    qk_chunk_b(pb, pkd, pch, *state.pop(i - 1))
                if spec is not None:
                    bd, kd, ch = spec
                    state[i] = qk_chunk_a(bd, kd, ch)
            return run

        return [make(i, spec)
                for i, spec in enumerate(list(chunks) + [None])]

    def bundle_chunks(bd):
        return [(bd, 1, 3), (bd, 1, 2), (bd, 1, 1), (bd, 1, 0),
                (bd, 0, 0), (bd, 0, 1), (bd, 0, 2), (bd, 0, 3)]

    # ---- PE pre-phase: V tiles (desc) interleaved with bundle-0 qk ----
    qk0 = chunk_closures(bundle_chunks(0))
    vt = [lambda jt=jt: v_tile(jt) for jt in range(JT - 1, -1, -1)]
    pre = [vt[0], vt[1], qk0[0], vt[2], vt[3], qk0[1], vt[4], vt[5], qk0[2],
           vt[6], vt[7], qk0[3], vt[8], vt[9], qk0[4], vt[10], vt[11],
           qk0[5], vt[12], vt[13], qk0[6], vt[14], vt[15]]
    for f in pre:
        f()

    # fillers: project bundle 1 while streaming the first two bundle-0 heads;
    # chunks 0..7 of the output projection (whose inputs complete with the
    # sp=0 streams) run inside the sp=1 streams, spreading the output DMAs.
    qk1 = chunk_closures(bundle_chunks(1))
    fillers = {(0, 0): qk1[:5], (0, 1): qk1[5:],
               (0, 2): [qk0[7], qk0[8]],
               (1, 1): [lambda chk=chk: proj_chunk(chk) for chk in range(0, 4)],
               (1, 2): [lambda chk=chk: proj_chunk(chk) for chk in range(4, 8)]}

    def proj_chunk(chk):
        # full output projection of one 128-query chunk (own heads' partial)
        ps = pspool.tile([128, 512], F32, tag="aux", name="ps_proj")
        for bd in range(2):
            nc.tensor.matmul(ps[:], out_pair[bd][:, chk * 128:(chk + 1) * 128],
                             projw_sb[bd][:],
                             start=(bd == 0), stop=(bd == 1),
                             skip_group_check=True)
        fin = fin_pool.tile([128, 512], BF16, tag="f", name="fin")
        nc.vector.tensor_add(fin[:], ps[:], biasb[:])
        nc.sync.dma_start(out_d[chk * 128:(chk + 1) * 128, :], fin[:])

    def norm_sub(sp, p, avt, qi):
        # attn-V is [query-part, head-dim]: the softmax denominator is a
        # per-partition scalar -> reciprocal + tensor_scalar multiply, then
        # PE-transpose back to [dim, query] for the projection.
        bd, row = p // 2, (p % 2) * 64
        col = 128 * (qi % 4)
        rec = norm_pool.tile([128, 1], F32, tag="rc", name="rec")
        nc.vector.reciprocal(rec[:], avt[:, col + 64:col + 65])
        stag = norm_pool.tile([128, 64], BF16, tag="st", name="stag")
        nc.vector.tensor_scalar_mul(stag[:], avt[:, col:col + 64], rec[:])
        pst = pspool.tile([64, 128], BF16, tag="aux", name="pst")
        nc.tensor.transpose(pst[:], stag[:], shifteye[:, 0:128])
        nc.vector.tensor_copy(
            out_pair[bd][row:row + 64,
                         1024 * sp + 128 * qi:1024 * sp + 128 * qi + 128],
            pst[:])

    # ---- attention streams: one per (slot-pair sp, position p) ----
    for sp in range(2):
        for p in range(NP_):
            bd, row = p // 2, (p % 2) * 64
            fl = fillers.get((sp, p), [])
            fi = 0
            jts = list(range(JT - 1, JTMIN[p][sp] - 1, -1))
            av01 = (pspool.tile([128, 512], F32, tag="av0", bufs=1, name="av0"),
                    pspool.tile([128, 512], F32, tag="av1", bufs=1, name="av1"))
            # PSUM accumulation state is per-bank on hardware: only read a
            # subtile once ALL subtiles sharing its bank have stopped.
            norm_at = {}
            norm_at.setdefault(STOPJT[p][sp][4], []).extend([4, 5, 6, 7])
            norm_at.setdefault(STOPJT[p][sp][0], []).extend([0, 1, 2, 3])
            pend = []  # [(jt, av-mm closure)], delayed two steps
            for idx, jt in enumerate(jts):
                while fi < len(fl) and (fi + 1) * len(jts) <= idx * len(fl):
                    fl[fi]()
                    fi += 1
                qm0, qm1, ql = (QM[p][sp][0][jt], QM[p][sp][1][jt],
                                QLIM[p][sp][jt])
                br = BIASR[p][sp][jt]
                ps = pspool.tile([128, 1024], F32, tag="sc", name="ps_sc")
                nc.tensor.matmul(
                    ps[:, 0:qm0],
                    k2_sb[bd][row:row + 64, jt * 128:(jt + 1) * 128],
                    q2_sb[bd][row:row + 64, 1024 * sp:1024 * sp + qm0],
                    start=True, stop=(br is None), tile_position=(row, 0),
                    skip_group_check=True)
                if qm1 > 0:
                    nc.tensor.matmul(
                        ps[:, 512:512 + qm1],
                        k2_sb[bd][row:row + 64, jt * 128:(jt + 1) * 128],
                        q2_sb[bd][row:row + 64,
                                  1024 * sp + 512:1024 * sp + 512 + qm1],
                        start=True, stop=(br is None), tile_position=(row, 0),
                        skip_group_check=True)
                if br is not None:
                    o = 1024 * sp - 128 * jt
                    # split at the 512-col PSUM bank boundary
                    for lo, hi in ((br[0], min(br[1], 512)),
                                   (max(br[0], 512), br[1])):
                        if lo < hi:
                            nc.tensor.matmul(
                                ps[:, lo:hi], c8eye_sb[p][:],
                                master[:, lo + o:hi + o],
                                start=False, stop=True, tile_position=(0, 0),
                                skip_group_check=True)
                et = exp_pool.tile([128, 1024], BF16, tag="e", name="et")
                nc.scalar.activation(et[:, 0:ql], ps[:, 0:ql], Exp)

                def av_mms(jt=jt, ql=ql, et=et):
                    for qi in range((ql + 127) // 128):
                        m = min(128, ql - 128 * qi)
                        avt = av01[qi // 4]
                        col = 128 * (qi % 4)
                        nc.tensor.matmul(
                            avt[0:m, col:col + 65],
                            et[:, 128 * qi:128 * qi + m],
                            v_sb[jt][:, p * 65:(p + 1) * 65],
                            start=(jt == JT - 1 and qi % 4 == 0),
                            stop=(jt == STOPJT[p][sp][qi]),
                            skip_group_check=True)

                if len(pend) >= 5:
                    pjt, pfn = pend.pop(0)
                    pfn()
                    for qi in norm_at.get(pjt, []):
                        norm_sub(sp, p, av01[qi // 4], qi)
                pend.append((jt, av_mms))
            for pjt, pfn in pend:
                pfn()
                for qi in norm_at.get(pjt, []):
                    norm_sub(sp, p, av01[qi // 4], qi)
            while fi < len(fl):
                fl[fi]()
                fi += 1

    # ---- remaining output projection chunks (sp=1 queries) ----
    # software-pipelined: each chunk's bundle-0 partial runs during the final
    # norm drain; only the bundle-1 matmul waits on the last normalize.
    def proj_pre(chk):
        ps = pspool.tile([128, 512], F32, tag="aux", name="ps_proj")
        nc.tensor.matmul(ps[:], out_pair[0][:, chk * 128:(chk + 1) * 128],
                         projw_sb[0][:],
                         start=True, stop=False, skip_group_check=True)
        return ps

    def proj_fin(chk, ps):
        nc.tensor.matmul(ps[:], out_pair[1][:, chk * 128:(chk + 1) * 128],
                         projw_sb[1][:],
                         start=False, stop=True, skip_group_check=True)
        fin = fin_pool.tile([128, 512], BF16, tag="f", name="fin")
        nc.vector.tensor_add(fin[:], ps[:], biasb[:])
        nc.sync.dma_start(out_d[chk * 128:(chk + 1) * 128, :], fin[:])

    # slot1 chunks (12..15) are unblocked at the last stream's midpoint;
    # slot0 chunks (8..11) wait for the final normalize, so run them last.
    order = [12, 13, 14, 15, 8, 9, 10, 11]
    pend_ps = {order[0]: proj_pre(order[0]), order[1]: proj_pre(order[1])}
    for i, chk in enumerate(order):
        proj_fin(chk, pend_ps.pop(chk))
        if i + 2 < len(order):
            pend_ps[order[i + 2]] = proj_pre(order[i + 2])

    ctx.close()


@functools.lru_cache(maxsize=1)
def _graph():
    return _build_graph()


def kernel(x, qkv_w, proj_w, proj_b):
    global LAST_RESULT
    x = np.asarray(x, np.float32)
    qkv_w = np.asarray(qkv_w, np.float32)
    proj_w = np.asarray(proj_w, np.float32)
    proj_b = np.asarray(proj_b, np.float32)

    nc = _graph()
    shared, qw, proj = _shared_inputs(qkv_w, proj_w, proj_b)
    in_maps = [_core_inputs(c, x, shared, qw, proj) for c in range(NCORES)]
    res = bass_utils.run_bass_kernel_spmd(nc, in_maps,
                                          core_ids=list(range(NCORES)),
                                          trace=False)
    LAST_RESULT = res
    out = np.zeros((B, N, C), np.float32)
    for b in range(B):
        out[b] = (np.asarray(res.results[2 * b]["out"], np.float32)
                  + np.asarray(res.results[2 * b + 1]["out"], np.float32))
    return out


# revision 114
# speedup vs baseline: 1.0990x; 1.0088x over previous
"""Fused multi-head attention layer (RoPE + ALiBi + softmax + out-proj) on 8 TRN2 cores.

Sharding (v4, heads-split): core c -> (batch b = c//2, head group g = c%2).
Each core computes q/k/v for its 4 heads {g, 2+g, 4+g, 6+g} over ALL 2048
positions, runs banded attention, and projects through its heads' slice of
proj_w, producing a partial [N, C] output; the host sums the two partials
per batch. Pairing adjacent-radius heads per graph position keeps the SPMD
union of ALiBi bands tight, and query blocks have exact (not unioned)
trim bounds.

Pipeline features:
- RoPE rotate-half via a sign-folded sin table + partition-permutation
  matmul (no duplicate rot projections).
- ALiBi bias on PE as c8-scaled identity matmuls against a single shared
  anti-diagonal master pattern table (column-shifted AP views), trimmed to
  the band columns.
- Score/exp/attn-V column ranges prefix-trimmed per (position, block, jt).
- Streams software-pipelined; V/next-bundle projections fill PE gaps.
"""

import functools
import math
import os
import sys

import numpy as np

sys.path.insert(0, "/opt/trn_rl_repo")

import ml_dtypes  # noqa: E402

import concourse.bass as bass  # noqa: E402
import concourse.tile as tile  # noqa: E402
from concourse import bacc, mybir, bass_utils  # noqa: E402

BF16 = mybir.dt.bfloat16
F32 = mybir.dt.float32
NPBF = ml_dtypes.bfloat16

B, N, C, H, D = 4, 2048, 512, 8, 64
NCORES = 8
JT = N // 128        # 16 j-tiles of 128 key positions
NP_ = 4              # head positions per core
T_CUT = 26.0         # ALiBi cutoff in logits: exp(-26) is negligible
SCALE = D ** -0.5

# c8_h = alibi_slope_h * MAX_BIAS = 2^-(h+1) * 8 = 2^(2-h)
C8 = [2.0 ** (2 - h) for h in range(H)]
RADIUS = [T_CUT / c for c in C8]   # band reach (key positions) per head
# graph position p holds heads {2p, 2p+1}; the union band is the odd head's
UR = [RADIUS[2 * p + 1] for p in range(NP_)]

LAST_RESULT = None  # test harness reads exec_time_ns from here


def _clamp(v, lo, hi):
    return max(lo, min(hi, v))


# Frame for (position p, slot-pair sp): cols 0:512 = query block 2sp
# (i0 = 1024sp), cols 512:1024 = block 2sp+1 (i0 = 1024sp+512). Exact bounds.
def _qm(p, sp, sl, jt):
    i0 = 512 * (2 * sp + sl)
    return _clamp(int(math.floor(128 * jt + 127 + UR[p])) + 1 - i0, 0, 512)


QM = [[[[_qm(p, sp, sl, jt) for jt in range(JT)] for sl in range(2)]
       for sp in range(2)] for p in range(NP_)]
QLIM = [[[(QM[p][sp][0][jt] if QM[p][sp][0][jt] < 512
           else 512 + QM[p][sp][1][jt]) for jt in range(JT)]
         for sp in range(2)] for p in range(NP_)]
JTMIN = [[min(jt for jt in range(JT) if QLIM[p][sp][jt] > 0)
          for sp in range(2)] for p in range(NP_)]
# per 128-query subtile qi of the 1024-col frame: the last (smallest) jt
# whose kept prefix still reaches it (attn-V accumulation stop point)
STOPJT = [[[min(jt for jt in range(JT) if QLIM[p][sp][jt] > 128 * qi)
            for qi in range(8)] for sp in range(2)] for p in range(NP_)]


def _bias_range(p, sp, jt):
    qm0, qm1 = QM[p][sp][0][jt], QM[p][sp][1][jt]
    bs0 = max(0, 128 * jt - 1024 * sp + 1)
    bs1 = max(0, 128 * jt - 1024 * sp - 511)
    r = []
    if bs0 < qm0:
        r.append((bs0, qm0))
    if bs1 < qm1:
        r.append((512 + bs1, 512 + qm1))
    if len(r) == 2:
        assert r[0][1] == 512 and r[1][0] == 512, (p, sp, jt, r)
        r = [(r[0][0], r[1][1])]
    return r[0] if r else None


BIASR = [[[_bias_range(p, sp, jt) for jt in range(JT)] for sp in range(2)]
         for p in range(NP_)]


def _heads(g):
    return [2 * p + g for p in range(NP_)]


def _rope_tables():
    inv = 1.0 / (10000.0 ** (np.arange(0, D, 2, dtype=np.float32) / D))
    f = np.arange(N, dtype=np.float32)[:, None] * inv[None, :]
    sin = np.concatenate([np.sin(f), np.sin(f)], axis=-1).astype(np.float32)
    cos = np.concatenate([np.cos(f), np.cos(f)], axis=-1).astype(np.float32)
    return sin, cos  # [N, D]


def _st_table(sin):
    # sign-folded, half-swapped sin table, indexed by SOURCE row r: after the
    # XOR-32 partition permutation, dest row d gets rot_half(q)[d]*sin[d].
    st = np.empty_like(sin)            # [N, D]
    st[:, 0:32] = sin[:, 32:64]
    st[:, 32:64] = -sin[:, 0:32]
    return st


def _shared_inputs(qkv_w, proj_w, proj_b):
    # [I | P32]: P32 is the XOR-32 partition permutation (within 64-blocks)
    shifteye = np.zeros((128, 256), np.float32)
    shifteye[:, 0:128] = np.eye(128)
    for r in range(128):
        shifteye[r, 128 + (r ^ 32)] = 1.0

    # master ALiBi pattern: master[k, x] = min(k - x, 0); the tile for
    # (jt, block i0) is the column window shifted by o = i0 - 128*jt
    jl = np.arange(128, dtype=np.float32)[:, None]
    xl = np.arange(2048, dtype=np.float32)[None, :]
    master = np.minimum(jl - xl, 0.0).astype(NPBF)

    sin, cos = _rope_tables()
    cos2 = np.tile(cos.T, (2, 1)).astype(NPBF)    # [128, N]
    ssin2 = np.tile(_st_table(sin).T, (2, 1)).astype(NPBF)
    return {
        "shifteye": shifteye.astype(NPBF),
        "master": master,
        "cos2": cos2, "ssin2": ssin2,
    }, qkv_w, (proj_w, proj_b)


def _core_inputs(c, x, shared, qkv_w, proj):
    proj_w, proj_b = proj
    b, g = c // 2, c % 2
    heads = _heads(g)
    cols = np.concatenate([np.arange(64 * h, 64 * h + 64) for h in heads])

    wqT = np.ascontiguousarray(qkv_w[0:C].T)[:, cols] * SCALE     # [C, 256]
    wkT = np.ascontiguousarray(qkv_w[C:2 * C].T)[:, cols]
    wvT = np.ascontiguousarray(qkv_w[2 * C:3 * C].T)[:, cols]
    wcat = np.concatenate([wqT, wkT, wvT], axis=1).astype(NPBF)   # [C, 768]

    c8eye = np.zeros((NP_, 128, 128), np.float32)
    for p in range(NP_):
        np.fill_diagonal(c8eye[p], C8[heads[p]])

    projwt = np.ascontiguousarray(proj_w.T)[cols, :]              # [256, C]
    biasb = np.tile(proj_b[None, :], (128, 1)) if g == 0 else \
        np.zeros((128, C), np.float32)

    return {
        "xt": np.ascontiguousarray(x[b].T).astype(NPBF),          # [C, N]
        "wcat": wcat,
        "c8eye": c8eye.astype(NPBF),
        "projwt": projwt.astype(NPBF),
        "biasb": biasb.astype(np.float32),
        **shared,
    }


def _build_graph():
    nc = bacc.Bacc("TRN2", target_bir_lowering=False, debug=False,
                   num_devices=NCORES)

    xt_d = nc.dram_tensor("xt", [C, N], BF16, kind="ExternalInput").ap()
    wcat_d = nc.dram_tensor("wcat", [C, 768], BF16, kind="ExternalInput").ap()
    cos2_d = nc.dram_tensor("cos2", [128, N], BF16, kind="ExternalInput").ap()
    ssin2_d = nc.dram_tensor("ssin2", [128, N], BF16, kind="ExternalInput").ap()
    shifteye_d = nc.dram_tensor("shifteye", [128, 256], BF16, kind="ExternalInput").ap()
    c8eye_d = nc.dram_tensor("c8eye", [NP_, 128, 128], BF16, kind="ExternalInput").ap()
    master_d = nc.dram_tensor("master", [128, 2048], BF16, kind="ExternalInput").ap()
    projwt_d = nc.dram_tensor("projwt", [256, C], BF16, kind="ExternalInput").ap()
    biasb_d = nc.dram_tensor("biasb", [128, C], F32, kind="ExternalInput").ap()
    out_d = nc.dram_tensor("out", [N, C], BF16, kind="ExternalOutput").ap()

    with tile.TileContext(nc) as tc:
        _body(nc, tc, xt_d, wcat_d, cos2_d, ssin2_d, shifteye_d, c8eye_d,
              master_d, projwt_d, biasb_d, out_d)
    nc.compile()
    return nc


def _body(nc, tc, xt_d, wcat_d, cos2_d, ssin2_d, shifteye_d, c8eye_d,
          master_d, projwt_d, biasb_d, out_d):
    from contextlib import ExitStack
    ctx = ExitStack()
    persist = ctx.enter_context(tc.tile_pool(name="persist", bufs=1))
    rope_pool = ctx.enter_context(tc.tile_pool(name="rope", bufs=2))
    exp_pool = ctx.enter_context(tc.tile_pool(name="exp", bufs=6))
    norm_pool = ctx.enter_context(tc.tile_pool(name="norm", bufs=4))
    fin_pool = ctx.enter_context(tc.tile_pool(name="final", bufs=8))
    pspool = ctx.enter_context(tc.tile_pool(name="ps", bufs=2, space="PSUM"))

    def ptile(shape, dtype, tag):
        return persist.tile(shape, dtype, tag=tag, name=tag)

    Exp = mybir.ActivationFunctionType.Exp

    # ---- persistent SBUF tiles ----
    w_sb = [ptile([128, 768], BF16, f"w{i}") for i in range(4)]
    xt_sb = [ptile([128, N], BF16, f"xt{i}") for i in range(4)]
    cos2 = ptile([128, N], BF16, "cos2")
    ssin2 = ptile([128, N], BF16, "ssin2")
    shifteye = ptile([128, 256], BF16, "shifteye")
    c8eye_sb = [ptile([128, 128], BF16, f"c8e{p}") for p in range(NP_)]
    master = ptile([128, 2048], BF16, "master")
    projw_sb = [ptile([128, C], BF16, f"pw{bd}") for bd in range(2)]
    biasb = ptile([128, C], F32, "biasb")
    q2_sb = [ptile([128, N], BF16, f"q2_{bd}") for bd in range(2)]
    k2_sb = [ptile([128, N], BF16, f"k2_{bd}") for bd in range(2)]
    v_sb = [ptile([128, NP_ * 65], BF16, f"v_{nt}") for nt in range(JT)]
    out_pair = [ptile([128, N], BF16, f"op_{bd}") for bd in range(2)]

    # ---- input DMAs, ordered to feed the PE emission order below ----
    # 1) V weights interleaved with the high xt columns so the first V
    # matmul starts after two transfers (V tiles run jt=15..0)
    for i in range(4):
        nc.sync.dma_start(w_sb[i][:, 512:768],
                          wcat_d[i * 128:(i + 1) * 128, 512:768])
        nc.sync.dma_start(xt_sb[i][:, 1536:2048],
                          xt_d[i * 128:(i + 1) * 128, 1536:2048])
    # 2) q/k weights + rope tables
    for i in range(4):
        nc.sync.dma_start(w_sb[i][:, 0:512], wcat_d[i * 128:(i + 1) * 128, 0:512])
    nc.sync.dma_start(shifteye[:], shifteye_d[:])
    nc.sync.dma_start(cos2[:], cos2_d[:])
    nc.sync.dma_start(ssin2[:], ssin2_d[:])
    # 3) remaining xt (descending), bias tables, proj weights
    for blk in (2, 1, 0):
        for i in range(4):
            nc.sync.dma_start(xt_sb[i][:, blk * 512:(blk + 1) * 512],
                              xt_d[i * 128:(i + 1) * 128, blk * 512:(blk + 1) * 512])
    nc.sync.dma_start(master[:], master_d[:])
    for p in range(NP_):
        nc.sync.dma_start(c8eye_sb[p][:], c8eye_d[p])
    for bd in range(2):
        nc.sync.dma_start(projw_sb[bd][:], projwt_d[bd * 128:(bd + 1) * 128, :])
    nc.sync.dma_start(biasb[:], biasb_d[:])

    # ---- helpers ----
    def v_tile(jt):
        psv = pspool.tile([128, 256], F32, tag="aux", name="psv")
        for ci in range(4):
            nc.tensor.matmul(
                psv[:], xt_sb[ci][:, jt * 128:(jt + 1) * 128],
                w_sb[ci][:, 512:768],
                start=(ci == 0), stop=(ci == 3))
        vdst = v_sb[jt].rearrange("p (h e) -> p h e", e=65)
        nc.vector.tensor_copy(vdst[:, :, 0:64],
                              psv.rearrange("p (h e) -> p h e", e=64))
        nc.gpsimd.memset(vdst[:, :, 64:65], 1.0)

    def qk_chunk_a(bd, kind, ch):
        # projection matmuls + cos/sin products for one 512-token chunk of
        # bundle bd (positions 2bd, 2bd+1), kind 0=q (scaled) 1=k.
        w_off = kind * 256 + bd * 128
        c0 = ch * 512
        ps_q = pspool.tile([128, 512], F32, tag="aux", name="ps_q")
        for ci in range(4):
            nc.tensor.matmul(
                ps_q[:],
                w_sb[ci][:, w_off:w_off + 128],
                xt_sb[ci][:, c0:c0 + 512],
                start=(ci == 0), stop=(ci == 3))
        tc_c = rope_pool.tile([128, 512], BF16, tag="tc", name="tc_c")
        nc.vector.tensor_mul(tc_c[:], ps_q[:], cos2[:, c0:c0 + 512])
        tc_u = rope_pool.tile([128, 512], BF16, tag="tu", name="tc_u")
        nc.vector.tensor_mul(tc_u[:], ps_q[:], ssin2[:, c0:c0 + 512])
        return tc_c, tc_u

    def qk_chunk_b(bd, kind, ch, tc_c, tc_u):
        # combine: dst = tc_c + P32 @ tc_u (partition-XOR-32 via matmul)
        dst_sb = k2_sb[bd] if kind else q2_sb[bd]
        c0 = ch * 512
        ps2 = pspool.tile([128, 512], F32, tag="aux", name="ps2")
        nc.tensor.matmul(ps2[:], shifteye[:, 0:128], tc_c[:],
                         start=True, stop=False)
        nc.tensor.matmul(ps2[:], shifteye[:, 128:256], tc_u[:],
                         start=False, stop=True)
        nc.vector.tensor_copy(dst_sb[:, c0:c0 + 512], ps2[:])

    # software-pipelined chunk list -> closures (B of chunk i rides with
    # A of chunk i+1 so the PE never waits on the DVE products)
    def chunk_closures(chunks):
        state = {}

        def make(i, spec):
            def run():
                if i > 0:
                    pb, pkd, pch = chunks[i - 1]
                    qk_chunk_b(pb, pkd, pch, *state.pop(i - 1))
                if spec is not None:
                    bd, kd, ch = spec
                    state[i] = qk_chunk_a(bd, kd, ch)
            return run

        return [make(i, spec)
                for i, spec in enumerate(list(chunks) + [None])]

    def bundle_chunks(bd):
        return [(bd, 1, 3), (bd, 1, 2), (bd, 1, 1), (bd, 1, 0),
                (bd, 0, 0), (bd, 0, 1), (bd, 0, 2), (bd, 0, 3)]

    # ---- PE pre-phase: V tiles (desc) interleaved with bundle-0 qk ----
    qk0 = chunk_closures(bundle_chunks(0))
    vt = [lambda jt=jt: v_tile(jt) for jt in range(JT - 1, -1, -1)]
    pre = [vt[0], vt[1], qk0[0], vt[2], vt[3], qk0[1], vt[4], vt[5], qk0[2],
           vt[6], vt[7], qk0[3], vt[8], vt[9], qk0[4], vt[10], vt[11],
           qk0[5], vt[12], vt[13], qk0[6], vt[14], vt[15]]
    for f in pre:
        f()

    # fillers: project bundle 1 while streaming the first two bundle-0 heads;
    # chunks 0..7 of the output projection (whose inputs complete with the
    # sp=0 streams) run inside the sp=1 streams, spreading the output DMAs.
    qk1 = chunk_closures(bundle_chunks(1))
    fillers = {(0, 0): qk1[:5], (0, 1): qk1[5:],
               (0, 2): [qk0[7], qk0[8]],
               (1, 1): [lambda chk=chk: proj_chunk(chk) for chk in range(0, 4)],
               (1, 2): [lambda chk=chk: proj_chunk(chk) for chk in range(4, 8)]}

    def proj_chunk(chk):
        # full output projection of one 128-query chunk (own heads' partial)
        ps = pspool.tile([128, 512], F32, tag="aux", name="ps_proj")
        for bd in range(2):
            nc.tensor.matmul(ps[:], out_pair[bd][:, chk * 128:(chk + 1) * 128],
                             projw_sb[bd][:],
                             start=(bd == 0), stop=(bd == 1),
                             skip_group_check=True)
        fin = fin_pool.tile([128, 512], BF16, tag="f", name="fin")
        nc.vector.tensor_add(fin[:], ps[:], biasb[:])
        nc.sync.dma_start(out_d[chk * 128:(chk + 1) * 128, :], fin[:])

    def norm_sub(sp, p, avt, qi):
        # attn-V is [query-part, head-dim]: the softmax denominator is a
        # per-partition scalar -> reciprocal + tensor_scalar multiply, then
        # PE-transpose back to [dim, query] for the projection.
        bd, row = p // 2, (p % 2) * 64
        col = 128 * (qi % 4)
        rec = norm_pool.tile([128, 1], F32, tag="rc", name="rec")
        nc.vector.reciprocal(rec[:], avt[:, col + 64:col + 65])
        stag = norm_pool.tile([128, 64], BF16, tag="st", name="stag")
        nc.vector.tensor_scalar_mul(stag[:], avt[:, col:col + 64], rec[:])
        pst = pspool.tile([64, 128], BF16, tag="aux", name="pst")
        nc.tensor.transpose(pst[:], stag[:], shifteye[:, 0:128])
        nc.vector.tensor_copy(
            out_pair[bd][row:row + 64,
                         1024 * sp + 128 * qi:1024 * sp + 128 * qi + 128],
            pst[:])

    # ---- attention streams: one per (slot-pair sp, position p) ----
    for sp in range(2):
        for p in range(NP_):
            bd, row = p // 2, (p % 2) * 64
            fl = fillers.get((sp, p), [])
            fi = 0
            jts = list(range(JT - 1, JTMIN[p][sp] - 1, -1))
            av01 = (pspool.tile([128, 512], F32, tag="av0", bufs=1, name="av0"),
                    pspool.tile([128, 512], F32, tag="av1", bufs=1, name="av1"))
            # PSUM accumulation state is per-bank on hardware: only read a
            # subtile once ALL subtiles sharing its bank have stopped.
            norm_at = {}
            norm_at.setdefault(STOPJT[p][sp][4], []).extend([4, 5, 6, 7])
            norm_at.setdefault(STOPJT[p][sp][0], []).extend([0, 1, 2, 3])
            pend = []  # [(jt, av-mm closure)], delayed two steps
            for idx, jt in enumerate(jts):
                while fi < len(fl) and (fi + 1) * len(jts) <= idx * len(fl):
                    fl[fi]()
                    fi += 1
                qm0, qm1, ql = (QM[p][sp][0][jt], QM[p][sp][1][jt],
                                QLIM[p][sp][jt])
                br = BIASR[p][sp][jt]
                ps = pspool.tile([128, 1024], F32, tag="sc", name="ps_sc")
                nc.tensor.matmul(
                    ps[:, 0:qm0],
                    k2_sb[bd][row:row + 64, jt * 128:(jt + 1) * 128],
                    q2_sb[bd][row:row + 64, 1024 * sp:1024 * sp + qm0],
                    start=True, stop=(br is None), tile_position=(row, 0),
                    skip_group_check=True)
                if qm1 > 0:
                    nc.tensor.matmul(
                        ps[:, 512:512 + qm1],
                        k2_sb[bd][row:row + 64, jt * 128:(jt + 1) * 128],
                        q2_sb[bd][row:row + 64,
                                  1024 * sp + 512:1024 * sp + 512 + qm1],
                        start=True, stop=(br is None), tile_position=(row, 0),
                        skip_group_check=True)
                if br is not None:
                    o = 1024 * sp - 128 * jt
                    # split at the 512-col PSUM bank boundary
                    for lo, hi in ((br[0], min(br[1], 512)),
                                   (max(br[0], 512), br[1])):
                        if lo < hi:
                            nc.tensor.matmul(
                                ps[:, lo:hi], c8eye_sb[p][:],
                                master[:, lo + o:hi + o],
                                start=False, stop=True, tile_position=(0, 0),
                                skip_group_check=True)
                et = exp_pool.tile([128, 1024], BF16, tag="e", name="et")
                nc.scalar.activation(et[:, 0:ql], ps[:, 0:ql], Exp)

                def av_mms(jt=jt, ql=ql, et=et):
                    for qi in range((ql + 127) // 128):
                        m = min(128, ql - 128 * qi)
                        avt = av01[qi // 4]
                        col = 128 * (qi % 4)
                        nc.tensor.matmul(
                            avt[0:m, col:col + 65],
                            et[:, 128 * qi:128 * qi + m],
                            v_sb[jt][:, p * 65:(p + 1) * 65],
                            start=(jt == JT - 1 and qi % 4 == 0),
                            stop=(jt == STOPJT[p][sp][qi]),
                            skip_group_check=True)

                if len(pend) >= 5:
                    pjt, pfn = pend.pop(0)
                    pfn()
                    for qi in norm_at.get(pjt, []):
                        norm_sub(sp, p, av01[qi // 4], qi)
                pend.append((jt, av_mms))
            for pjt, pfn in pend:
                pfn()
                for qi in norm_at.get(pjt, []):
                    norm_sub(sp, p, av01[qi // 4], qi)
            while fi < len(fl):
                fl[fi]()
                fi += 1

    # ---- remaining output projection chunks (sp=1 queries) ----
    # software-pipelined: each chunk's bundle-0 partial runs during the final
    # norm drain; only the bundle-1 matmul waits on the last normalize.
    def proj_pre(chk):
        ps = pspool.tile([128, 512], F32, tag="aux", name="ps_proj")
        nc.tensor.matmul(ps[:], out_pair[0][:, chk * 128:(chk + 1) * 128],
                         projw_sb[0][:],
                         start=True, stop=False, skip_group_check=True)
        return ps

    def proj_fin(chk, ps):
        nc.tensor.matmul(ps[:], out_pair[1][:, chk * 128:(chk + 1) * 128],
                         projw_sb[1][:],
                         start=False, stop=True, skip_group_check=True)
        fin = fin_pool.tile([128, 512], BF16, tag="f", name="fin")
        nc.vector.tensor_add(fin[:], ps[:], biasb[:])
        nc.sync.dma_start(out_d[chk * 128:(chk + 1) * 128, :], fin[:])

    # slot1 chunks (12..15) are unblocked at the last stream's midpoint;
    # slot0 chunks (8..11) wait for the final normalize, so run them last.
    order = [12, 13, 14, 15, 8, 9, 10, 11]
    pend_ps = {order[0]: proj_pre(order[0]), order[1]: proj_pre(order[1])}
    for i, chk in enumerate(order):
        proj_fin(chk, pend_ps.pop(chk))
        if i + 2 < len(order):
            pend_ps[order[i + 2]] = proj_pre(order[i + 2])

    ctx.close()


@functools.lru_cache(maxsize=1)
def _graph():
    return _build_graph()


def kernel(x, qkv_w, proj_w, proj_b):
    global LAST_RESULT
    x = np.asarray(x, np.float32)
    qkv_w = np.asarray(qkv_w, np.float32)
    proj_w = np.asarray(proj_w, np.float32)
    proj_b = np.asarray(proj_b, np.float32)

    nc = _graph()
    shared, qw, proj = _shared_inputs(qkv_w, proj_w, proj_b)
    in_maps = [_core_inputs(c, x, shared, qw, proj) for c in range(NCORES)]
    res = bass_utils.run_bass_kernel_spmd(nc, in_maps,
                                          core_ids=list(range(NCORES)),
                                          trace=False)
    LAST_RESULT = res
    out = np.zeros((B, N, C), np.float32)
    for b in range(B):
        out[b] = (np.asarray(res.results[2 * b]["out"], np.float32)
                  + np.asarray(res.results[2 * b + 1]["out"], np.float32))
    return out


# revision 115
# speedup vs baseline: 1.1113x; 1.0112x over previous
"""Fused multi-head attention layer (RoPE + ALiBi + softmax + out-proj) on 8 TRN2 cores.

Sharding (v4, heads-split): core c -> (batch b = c//2, head group g = c%2).
Each core computes q/k/v for its 4 heads {g, 2+g, 4+g, 6+g} over ALL 2048
positions, runs banded attention, and projects through its heads' slice of
proj_w, producing a partial [N, C] output; the host sums the two partials
per batch. Pairing adjacent-radius heads per graph position keeps the SPMD
union of ALiBi bands tight, and query blocks have exact (not unioned)
trim bounds.

Pipeline features:
- RoPE rotate-half via a sign-folded sin table + partition-permutation
  matmul (no duplicate rot projections).
- ALiBi bias on PE as c8-scaled identity matmuls against a single shared
  anti-diagonal master pattern table (column-shifted AP views), trimmed to
  the band columns.
- Score/exp/attn-V column ranges prefix-trimmed per (position, block, jt).
- Streams software-pipelined; V/next-bundle projections fill PE gaps.
"""

import functools
import math
import os
import sys

import numpy as np

sys.path.insert(0, "/opt/trn_rl_repo")

import ml_dtypes  # noqa: E402

import concourse.bass as bass  # noqa: E402
import concourse.tile as tile  # noqa: E402
from concourse import bacc, mybir, bass_utils  # noqa: E402

BF16 = mybir.dt.bfloat16
F32 = mybir.dt.float32
NPBF = ml_dtypes.bfloat16

B, N, C, H, D = 4, 2048, 512, 8, 64
NCORES = 8
JT = N // 128        # 16 j-tiles of 128 key positions
NP_ = 4              # head positions per core
T_CUT = 21.0         # ALiBi cutoff in logits: exp(-21) is negligible
SCALE = D ** -0.5

# c8_h = alibi_slope_h * MAX_BIAS = 2^-(h+1) * 8 = 2^(2-h)
C8 = [2.0 ** (2 - h) for h in range(H)]
RADIUS = [T_CUT / c for c in C8]   # band reach (key positions) per head
# graph position p holds heads {2p, 2p+1}; the union band is the odd head's
UR = [RADIUS[2 * p + 1] for p in range(NP_)]

LAST_RESULT = None  # test harness reads exec_time_ns from here


def _clamp(v, lo, hi):
    return max(lo, min(hi, v))


# Frame for (position p, slot-pair sp): cols 0:512 = query block 2sp
# (i0 = 1024sp), cols 512:1024 = block 2sp+1 (i0 = 1024sp+512). Exact bounds.
def _qm(p, sp, sl, jt):
    i0 = 512 * (2 * sp + sl)
    return _clamp(int(math.floor(128 * jt + 127 + UR[p])) + 1 - i0, 0, 512)


QM = [[[[_qm(p, sp, sl, jt) for jt in range(JT)] for sl in range(2)]
       for sp in range(2)] for p in range(NP_)]
QLIM = [[[(QM[p][sp][0][jt] if QM[p][sp][0][jt] < 512
           else 512 + QM[p][sp][1][jt]) for jt in range(JT)]
         for sp in range(2)] for p in range(NP_)]
JTMIN = [[min(jt for jt in range(JT) if QLIM[p][sp][jt] > 0)
          for sp in range(2)] for p in range(NP_)]
# per 128-query subtile qi of the 1024-col frame: the last (smallest) jt
# whose kept prefix still reaches it (attn-V accumulation stop point)
STOPJT = [[[min(jt for jt in range(JT) if QLIM[p][sp][jt] > 128 * qi)
            for qi in range(8)] for sp in range(2)] for p in range(NP_)]


def _bias_range(p, sp, jt):
    qm0, qm1 = QM[p][sp][0][jt], QM[p][sp][1][jt]
    bs0 = max(0, 128 * jt - 1024 * sp + 1)
    bs1 = max(0, 128 * jt - 1024 * sp - 511)
    r = []
    if bs0 < qm0:
        r.append((bs0, qm0))
    if bs1 < qm1:
        r.append((512 + bs1, 512 + qm1))
    if len(r) == 2:
        assert r[0][1] == 512 and r[1][0] == 512, (p, sp, jt, r)
        r = [(r[0][0], r[1][1])]
    return r[0] if r else None


BIASR = [[[_bias_range(p, sp, jt) for jt in range(JT)] for sp in range(2)]
         for p in range(NP_)]


def _heads(g):
    return [2 * p + g for p in range(NP_)]


def _rope_tables():
    inv = 1.0 / (10000.0 ** (np.arange(0, D, 2, dtype=np.float32) / D))
    f = np.arange(N, dtype=np.float32)[:, None] * inv[None, :]
    sin = np.concatenate([np.sin(f), np.sin(f)], axis=-1).astype(np.float32)
    cos = np.concatenate([np.cos(f), np.cos(f)], axis=-1).astype(np.float32)
    return sin, cos  # [N, D]


def _st_table(sin):
    # sign-folded, half-swapped sin table, indexed by SOURCE row r: after the
    # XOR-32 partition permutation, dest row d gets rot_half(q)[d]*sin[d].
    st = np.empty_like(sin)            # [N, D]
    st[:, 0:32] = sin[:, 32:64]
    st[:, 32:64] = -sin[:, 0:32]
    return st


def _shared_inputs(qkv_w, proj_w, proj_b):
    # [I | P32]: P32 is the XOR-32 partition permutation (within 64-blocks)
    shifteye = np.zeros((128, 256), np.float32)
    shifteye[:, 0:128] = np.eye(128)
    for r in range(128):
        shifteye[r, 128 + (r ^ 32)] = 1.0

    # master ALiBi pattern: master[k, x] = min(k - x, 0); the tile for
    # (jt, block i0) is the column window shifted by o = i0 - 128*jt
    jl = np.arange(128, dtype=np.float32)[:, None]
    xl = np.arange(2048, dtype=np.float32)[None, :]
    master = np.minimum(jl - xl, 0.0).astype(NPBF)

    sin, cos = _rope_tables()
    cos2 = np.tile(cos.T, (2, 1)).astype(NPBF)    # [128, N]
    ssin2 = np.tile(_st_table(sin).T, (2, 1)).astype(NPBF)
    return {
        "shifteye": shifteye.astype(NPBF),
        "master": master,
        "cos2": cos2, "ssin2": ssin2,
    }, qkv_w, (proj_w, proj_b)


def _core_inputs(c, x, shared, qkv_w, proj):
    proj_w, proj_b = proj
    b, g = c // 2, c % 2
    heads = _heads(g)
    cols = np.concatenate([np.arange(64 * h, 64 * h + 64) for h in heads])

    wqT = np.ascontiguousarray(qkv_w[0:C].T)[:, cols] * SCALE     # [C, 256]
    wkT = np.ascontiguousarray(qkv_w[C:2 * C].T)[:, cols]
    wvT = np.ascontiguousarray(qkv_w[2 * C:3 * C].T)[:, cols]
    wcat = np.concatenate([wqT, wkT, wvT], axis=1).astype(NPBF)   # [C, 768]

    c8eye = np.zeros((NP_, 128, 128), np.float32)
    for p in range(NP_):
        np.fill_diagonal(c8eye[p], C8[heads[p]])

    projwt = np.ascontiguousarray(proj_w.T)[cols, :]              # [256, C]
    biasb = np.tile(proj_b[None, :], (128, 1)) if g == 0 else \
        np.zeros((128, C), np.float32)

    return {
        "xt": np.ascontiguousarray(x[b].T).astype(NPBF),          # [C, N]
        "wcat": wcat,
        "c8eye": c8eye.astype(NPBF),
        "projwt": projwt.astype(NPBF),
        "biasb": biasb.astype(np.float32),
        **shared,
    }


def _build_graph():
    nc = bacc.Bacc("TRN2", target_bir_lowering=False, debug=False,
                   num_devices=NCORES)

    xt_d = nc.dram_tensor("xt", [C, N], BF16, kind="ExternalInput").ap()
    wcat_d = nc.dram_tensor("wcat", [C, 768], BF16, kind="ExternalInput").ap()
    cos2_d = nc.dram_tensor("cos2", [128, N], BF16, kind="ExternalInput").ap()
    ssin2_d = nc.dram_tensor("ssin2", [128, N], BF16, kind="ExternalInput").ap()
    shifteye_d = nc.dram_tensor("shifteye", [128, 256], BF16, kind="ExternalInput").ap()
    c8eye_d = nc.dram_tensor("c8eye", [NP_, 128, 128], BF16, kind="ExternalInput").ap()
    master_d = nc.dram_tensor("master", [128, 2048], BF16, kind="ExternalInput").ap()
    projwt_d = nc.dram_tensor("projwt", [256, C], BF16, kind="ExternalInput").ap()
    biasb_d = nc.dram_tensor("biasb", [128, C], F32, kind="ExternalInput").ap()
    out_d = nc.dram_tensor("out", [N, C], BF16, kind="ExternalOutput").ap()

    with tile.TileContext(nc) as tc:
        _body(nc, tc, xt_d, wcat_d, cos2_d, ssin2_d, shifteye_d, c8eye_d,
              master_d, projwt_d, biasb_d, out_d)
    nc.compile()
    return nc


def _body(nc, tc, xt_d, wcat_d, cos2_d, ssin2_d, shifteye_d, c8eye_d,
          master_d, projwt_d, biasb_d, out_d):
    from contextlib import ExitStack
    ctx = ExitStack()
    persist = ctx.enter_context(tc.tile_pool(name="persist", bufs=1))
    rope_pool = ctx.enter_context(tc.tile_pool(name="rope", bufs=2))
    exp_pool = ctx.enter_context(tc.tile_pool(name="exp", bufs=6))
    norm_pool = ctx.enter_context(tc.tile_pool(name="norm", bufs=4))
    fin_pool = ctx.enter_context(tc.tile_pool(name="final", bufs=8))
    pspool = ctx.enter_context(tc.tile_pool(name="ps", bufs=2, space="PSUM"))

    def ptile(shape, dtype, tag):
        return persist.tile(shape, dtype, tag=tag, name=tag)

    Exp = mybir.ActivationFunctionType.Exp

    # ---- persistent SBUF tiles ----
    w_sb = [ptile([128, 768], BF16, f"w{i}") for i in range(4)]
    xt_sb = [ptile([128, N], BF16, f"xt{i}") for i in range(4)]
    cos2 = ptile([128, N], BF16, "cos2")
    ssin2 = ptile([128, N], BF16, "ssin2")
    shifteye = ptile([128, 256], BF16, "shifteye")
    c8eye_sb = [ptile([128, 128], BF16, f"c8e{p}") for p in range(NP_)]
    master = ptile([128, 2048], BF16, "master")
    projw_sb = [ptile([128, C], BF16, f"pw{bd}") for bd in range(2)]
    biasb = ptile([128, C], F32, "biasb")
    q2_sb = [ptile([128, N], BF16, f"q2_{bd}") for bd in range(2)]
    k2_sb = [ptile([128, N], BF16, f"k2_{bd}") for bd in range(2)]
    v_sb = [ptile([128, NP_ * 65], BF16, f"v_{nt}") for nt in range(JT)]
    out_pair = [ptile([128, N], BF16, f"op_{bd}") for bd in range(2)]

    # ---- input DMAs, ordered to feed the PE emission order below ----
    # 1) V weights interleaved with the high xt columns so the first V
    # matmul starts after two transfers (V tiles run jt=15..0)
    for i in range(4):
        nc.sync.dma_start(w_sb[i][:, 512:768],
                          wcat_d[i * 128:(i + 1) * 128, 512:768])
        nc.sync.dma_start(xt_sb[i][:, 1536:2048],
                          xt_d[i * 128:(i + 1) * 128, 1536:2048])
    # 2) q/k weights + rope tables
    for i in range(4):
        nc.sync.dma_start(w_sb[i][:, 0:512], wcat_d[i * 128:(i + 1) * 128, 0:512])
    nc.sync.dma_start(shifteye[:], shifteye_d[:])
    nc.sync.dma_start(cos2[:], cos2_d[:])
    nc.sync.dma_start(ssin2[:], ssin2_d[:])
    # 3) remaining xt (descending), bias tables, proj weights
    for blk in (2, 1, 0):
        for i in range(4):
            nc.sync.dma_start(xt_sb[i][:, blk * 512:(blk + 1) * 512],
                              xt_d[i * 128:(i + 1) * 128, blk * 512:(blk + 1) * 512])
    nc.sync.dma_start(master[:], master_d[:])
    for p in range(NP_):
        nc.sync.dma_start(c8eye_sb[p][:], c8eye_d[p])
    for bd in range(2):
        nc.sync.dma_start(projw_sb[bd][:], projwt_d[bd * 128:(bd + 1) * 128, :])
    nc.sync.dma_start(biasb[:], biasb_d[:])

    # ---- helpers ----
    def v_tile(jt):
        psv = pspool.tile([128, 256], F32, tag="aux", name="psv")
        for ci in range(4):
            nc.tensor.matmul(
                psv[:], xt_sb[ci][:, jt * 128:(jt + 1) * 128],
                w_sb[ci][:, 512:768],
                start=(ci == 0), stop=(ci == 3))
        vdst = v_sb[jt].rearrange("p (h e) -> p h e", e=65)
        nc.vector.tensor_copy(vdst[:, :, 0:64],
                              psv.rearrange("p (h e) -> p h e", e=64))
        nc.gpsimd.memset(vdst[:, :, 64:65], 1.0)

    def qk_chunk_a(bd, kind, ch):
        # projection matmuls + cos/sin products for one 512-token chunk of
        # bundle bd (positions 2bd, 2bd+1), kind 0=q (scaled) 1=k.
        w_off = kind * 256 + bd * 128
        c0 = ch * 512
        ps_q = pspool.tile([128, 512], F32, tag="aux", name="ps_q")
        for ci in range(4):
            nc.tensor.matmul(
                ps_q[:],
                w_sb[ci][:, w_off:w_off + 128],
                xt_sb[ci][:, c0:c0 + 512],
                start=(ci == 0), stop=(ci == 3))
        tc_c = rope_pool.tile([128, 512], BF16, tag="tc", name="tc_c")
        nc.vector.tensor_mul(tc_c[:], ps_q[:], cos2[:, c0:c0 + 512])
        tc_u = rope_pool.tile([128, 512], BF16, tag="tu", name="tc_u")
        nc.vector.tensor_mul(tc_u[:], ps_q[:], ssin2[:, c0:c0 + 512])
        return tc_c, tc_u

    def qk_chunk_b(bd, kind, ch, tc_c, tc_u):
        # combine: dst = tc_c + P32 @ tc_u (partition-XOR-32 via matmul)
        dst_sb = k2_sb[bd] if kind else q2_sb[bd]
        c0 = ch * 512
        ps2 = pspool.tile([128, 512], F32, tag="aux", name="ps2")
        nc.tensor.matmul(ps2[:], shifteye[:, 0:128], tc_c[:],
                         start=True, stop=False)
        nc.tensor.matmul(ps2[:], shifteye[:, 128:256], tc_u[:],
                         start=False, stop=True)
        nc.vector.tensor_copy(dst_sb[:, c0:c0 + 512], ps2[:])

    # software-pipelined chunk list -> closures (B of chunk i rides with
    # A of chunk i+1 so the PE never waits on the DVE products)
    def chunk_closures(chunks):
        state = {}

        def make(i, spec):
            def run():
                if i > 0:
                    pb, pkd, pch = chunks[i - 1]
                    qk_chunk_b(pb, pkd, pch, *state.pop(i - 1))
                if spec is not None:
                    bd, kd, ch = spec
                    state[i] = qk_chunk_a(bd, kd, ch)
            return run

        return [make(i, spec)
                for i, spec in enumerate(list(chunks) + [None])]

    def bundle_chunks(bd):
        return [(bd, 1, 3), (bd, 1, 2), (bd, 1, 1), (bd, 1, 0),
                (bd, 0, 0), (bd, 0, 1), (bd, 0, 2), (bd, 0, 3)]

    # ---- PE pre-phase: V tiles (desc) interleaved with bundle-0 qk ----
    qk0 = chunk_closures(bundle_chunks(0))
    vt = [lambda jt=jt: v_tile(jt) for jt in range(JT - 1, -1, -1)]
    pre = [vt[0], vt[1], qk0[0], vt[2], vt[3], qk0[1], vt[4], vt[5], qk0[2],
           vt[6], vt[7], qk0[3], vt[8], vt[9], qk0[4], vt[10], vt[11],
           qk0[5], vt[12], vt[13], qk0[6], vt[14], vt[15]]
    for f in pre:
        f()

    # fillers: project bundle 1 while streaming the first two bundle-0 heads;
    # chunks 0..7 of the output projection (whose inputs complete with the
    # sp=0 streams) run inside the sp=1 streams, spreading the output DMAs.
    qk1 = chunk_closures(bundle_chunks(1))
    fillers = {(0, 0): qk1[:5], (0, 1): qk1[5:],
               (0, 2): [qk0[7], qk0[8]],
               (1, 1): [lambda chk=chk: proj_chunk(chk) for chk in range(0, 4)],
               (1, 2): [lambda chk=chk: proj_chunk(chk) for chk in range(4, 8)]}

    def proj_chunk(chk):
        # full output projection of one 128-query chunk (own heads' partial)
        ps = pspool.tile([128, 512], F32, tag="aux", name="ps_proj")
        for bd in range(2):
            nc.tensor.matmul(ps[:], out_pair[bd][:, chk * 128:(chk + 1) * 128],
                             projw_sb[bd][:],
                             start=(bd == 0), stop=(bd == 1),
                             skip_group_check=True)
        fin = fin_pool.tile([128, 512], BF16, tag="f", name="fin")
        nc.vector.tensor_add(fin[:], ps[:], biasb[:])
        nc.sync.dma_start(out_d[chk * 128:(chk + 1) * 128, :], fin[:])

    def norm_sub(sp, p, avt, qi):
        # attn-V is [query-part, head-dim]: the softmax denominator is a
        # per-partition scalar -> reciprocal + tensor_scalar multiply, then
        # PE-transpose back to [dim, query] for the projection.
        bd, row = p // 2, (p % 2) * 64
        col = 128 * (qi % 4)
        rec = norm_pool.tile([128, 1], F32, tag="rc", name="rec")
        nc.vector.reciprocal(rec[:], avt[:, col + 64:col + 65])
        stag = norm_pool.tile([128, 64], BF16, tag="st", name="stag")
        nc.vector.tensor_scalar_mul(stag[:], avt[:, col:col + 64], rec[:])
        pst = pspool.tile([64, 128], BF16, tag="aux", name="pst")
        nc.tensor.transpose(pst[:], stag[:], shifteye[:, 0:128])
        nc.vector.tensor_copy(
            out_pair[bd][row:row + 64,
                         1024 * sp + 128 * qi:1024 * sp + 128 * qi + 128],
            pst[:])

    # ---- attention streams: one per (slot-pair sp, position p) ----
    for sp in range(2):
        for p in range(NP_):
            bd, row = p // 2, (p % 2) * 64
            fl = fillers.get((sp, p), [])
            fi = 0
            jts = list(range(JT - 1, JTMIN[p][sp] - 1, -1))
            av01 = (pspool.tile([128, 512], F32, tag="av0", bufs=1, name="av0"),
                    pspool.tile([128, 512], F32, tag="av1", bufs=1, name="av1"))
            # PSUM accumulation state is per-bank on hardware: only read a
            # subtile once ALL subtiles sharing its bank have stopped.
            norm_at = {}
            norm_at.setdefault(STOPJT[p][sp][4], []).extend([4, 5, 6, 7])
            norm_at.setdefault(STOPJT[p][sp][0], []).extend([0, 1, 2, 3])
            pend = []  # [(jt, av-mm closure)], delayed two steps
            for idx, jt in enumerate(jts):
                while fi < len(fl) and (fi + 1) * len(jts) <= idx * len(fl):
                    fl[fi]()
                    fi += 1
                qm0, qm1, ql = (QM[p][sp][0][jt], QM[p][sp][1][jt],
                                QLIM[p][sp][jt])
                br = BIASR[p][sp][jt]
                ps = pspool.tile([128, 1024], F32, tag="sc", name="ps_sc")
                nc.tensor.matmul(
                    ps[:, 0:qm0],
                    k2_sb[bd][row:row + 64, jt * 128:(jt + 1) * 128],
                    q2_sb[bd][row:row + 64, 1024 * sp:1024 * sp + qm0],
                    start=True, stop=(br is None), tile_position=(row, 0),
                    skip_group_check=True)
                if qm1 > 0:
                    nc.tensor.matmul(
                        ps[:, 512:512 + qm1],
                        k2_sb[bd][row:row + 64, jt * 128:(jt + 1) * 128],
                        q2_sb[bd][row:row + 64,
                                  1024 * sp + 512:1024 * sp + 512 + qm1],
                        start=True, stop=(br is None), tile_position=(row, 0),
                        skip_group_check=True)
                if br is not None:
                    o = 1024 * sp - 128 * jt
                    # split at the 512-col PSUM bank boundary
                    for lo, hi in ((br[0], min(br[1], 512)),
                                   (max(br[0], 512), br[1])):
                        if lo < hi:
                            nc.tensor.matmul(
                                ps[:, lo:hi], c8eye_sb[p][:],
                                master[:, lo + o:hi + o],
                                start=False, stop=True, tile_position=(0, 0),
                                skip_group_check=True)
                et = exp_pool.tile([128, 1024], BF16, tag="e", name="et")
                nc.scalar.activation(et[:, 0:ql], ps[:, 0:ql], Exp)

                def av_mms(jt=jt, ql=ql, et=et):
                    for qi in range((ql + 127) // 128):
                        m = min(128, ql - 128 * qi)
                        avt = av01[qi // 4]
                        col = 128 * (qi % 4)
                        nc.tensor.matmul(
                            avt[0:m, col:col + 65],
                            et[:, 128 * qi:128 * qi + m],
                            v_sb[jt][:, p * 65:(p + 1) * 65],
                            start=(jt == JT - 1 and qi % 4 == 0),
                            stop=(jt == STOPJT[p][sp][qi]),
                            skip_group_check=True)

                if len(pend) >= 5:
                    pjt, pfn = pend.pop(0)
                    pfn()
                    for qi in norm_at.get(pjt, []):
                        norm_sub(sp, p, av01[qi // 4], qi)
                pend.append((jt, av_mms))
            for pjt, pfn in pend:
                pfn()
                for qi in norm_at.get(pjt, []):
                    norm_sub(sp, p, av01[qi // 4], qi)
            while fi < len(fl):
                fl[fi]()
                fi += 1

    # ---- remaining output projection chunks (sp=1 queries) ----
    # software-pipelined: each chunk's bundle-0 partial runs during the final
    # norm drain; only the bundle-1 matmul waits on the last normalize.
    def proj_pre(chk):
        ps = pspool.tile([128, 512], F32, tag="aux", name="ps_proj")
        nc.tensor.matmul(ps[:], out_pair[0][:, chk * 128:(chk + 1) * 128],
                         projw_sb[0][:],
                         start=True, stop=False, skip_group_check=True)
        return ps

    def proj_fin(chk, ps):
        nc.tensor.matmul(ps[:], out_pair[1][:, chk * 128:(chk + 1) * 128],
                         projw_sb[1][:],
                         start=False, stop=True, skip_group_check=True)
        fin = fin_pool.tile([128, 512], BF16, tag="f", name="fin")
        nc.vector.tensor_add(fin[:], ps[:], biasb[:])
        nc.sync.dma_start(out_d[chk * 128:(chk + 1) * 128, :], fin[:])

    # slot1 chunks (12..15) are unblocked at the last stream's midpoint;
    # slot0 chunks (8..11) wait for the final normalize, so run them last.
    order = [12, 13, 14, 15, 8, 9, 10, 11]
    pend_ps = {order[0]: proj_pre(order[0]), order[1]: proj_pre(order[1])}
    for i, chk in enumerate(order):
        proj_fin(chk, pend_ps.pop(chk))
        if i + 2 < len(order):
            pend_ps[order[i + 2]] = proj_pre(order[i + 2])

    ctx.close()


@functools.lru_cache(maxsize=1)
def _graph():
    return _build_graph()


def kernel(x, qkv_w, proj_w, proj_b):
    global LAST_RESULT
    x = np.asarray(x, np.float32)
    qkv_w = np.asarray(qkv_w, np.float32)
    proj_w = np.asarray(proj_w, np.float32)
    proj_b = np.asarray(proj_b, np.float32)

    nc = _graph()
    shared, qw, proj = _shared_inputs(qkv_w, proj_w, proj_b)
    in_maps = [_core_inputs(c, x, shared, qw, proj) for c in range(NCORES)]
    res = bass_utils.run_bass_kernel_spmd(nc, in_maps,
                                          core_ids=list(range(NCORES)),
                                          trace=False)
    LAST_RESULT = res
    out = np.zeros((B, N, C), np.float32)
    for b in range(B):
        out[b] = (np.asarray(res.results[2 * b]["out"], np.float32)
                  + np.asarray(res.results[2 * b + 1]["out"], np.float32))
    return out


# revision 116
# speedup vs baseline: 1.1240x; 1.0114x over previous
"""Fused multi-head attention layer (RoPE + ALiBi + softmax + out-proj) on 8 TRN2 cores.

Sharding (v4, heads-split): core c -> (batch b = c//2, head group g = c%2).
Each core computes q/k/v for its 4 heads {g, 2+g, 4+g, 6+g} over ALL 2048
positions, runs banded attention, and projects through its heads' slice of
proj_w, producing a partial [N, C] output; the host sums the two partials
per batch. Pairing adjacent-radius heads per graph position keeps the SPMD
union of ALiBi bands tight, and query blocks have exact (not unioned)
trim bounds.

Pipeline features:
- RoPE rotate-half via a sign-folded sin table + partition-permutation
  matmul (no duplicate rot projections).
- ALiBi bias on PE as c8-scaled identity matmuls against a single shared
  anti-diagonal master pattern table (column-shifted AP views), trimmed to
  the band columns.
- Score/exp/attn-V column ranges prefix-trimmed per (position, block, jt).
- Streams software-pipelined; V/next-bundle projections fill PE gaps.
"""

import functools
import math
import os
import sys

import numpy as np

sys.path.insert(0, "/opt/trn_rl_repo")

import ml_dtypes  # noqa: E402

import concourse.bass as bass  # noqa: E402
import concourse.tile as tile  # noqa: E402
from concourse import bacc, mybir, bass_utils  # noqa: E402

BF16 = mybir.dt.bfloat16
F32 = mybir.dt.float32
NPBF = ml_dtypes.bfloat16

B, N, C, H, D = 4, 2048, 512, 8, 64
NCORES = 8
JT = N // 128        # 16 j-tiles of 128 key positions
NP_ = 4              # head positions per core
T_CUT = 17.0         # ALiBi cutoff in logits: exp(-17) is negligible
SCALE = D ** -0.5

# c8_h = alibi_slope_h * MAX_BIAS = 2^-(h+1) * 8 = 2^(2-h)
C8 = [2.0 ** (2 - h) for h in range(H)]
RADIUS = [T_CUT / c for c in C8]   # band reach (key positions) per head
# graph position p holds heads {2p, 2p+1}; the union band is the odd head's
UR = [RADIUS[2 * p + 1] for p in range(NP_)]

LAST_RESULT = None  # test harness reads exec_time_ns from here


def _clamp(v, lo, hi):
    return max(lo, min(hi, v))


# Frame for (position p, slot-pair sp): cols 0:512 = query block 2sp
# (i0 = 1024sp), cols 512:1024 = block 2sp+1 (i0 = 1024sp+512). Exact bounds.
def _qm(p, sp, sl, jt):
    i0 = 512 * (2 * sp + sl)
    return _clamp(int(math.floor(128 * jt + 127 + UR[p])) + 1 - i0, 0, 512)


QM = [[[[_qm(p, sp, sl, jt) for jt in range(JT)] for sl in range(2)]
       for sp in range(2)] for p in range(NP_)]
QLIM = [[[(QM[p][sp][0][jt] if QM[p][sp][0][jt] < 512
           else 512 + QM[p][sp][1][jt]) for jt in range(JT)]
         for sp in range(2)] for p in range(NP_)]
JTMIN = [[min(jt for jt in range(JT) if QLIM[p][sp][jt] > 0)
          for sp in range(2)] for p in range(NP_)]
# per 128-query subtile qi of the 1024-col frame: the last (smallest) jt
# whose kept prefix still reaches it (attn-V accumulation stop point)
STOPJT = [[[min(jt for jt in range(JT) if QLIM[p][sp][jt] > 128 * qi)
            for qi in range(8)] for sp in range(2)] for p in range(NP_)]


def _bias_range(p, sp, jt):
    qm0, qm1 = QM[p][sp][0][jt], QM[p][sp][1][jt]
    bs0 = max(0, 128 * jt - 1024 * sp + 1)
    bs1 = max(0, 128 * jt - 1024 * sp - 511)
    r = []
    if bs0 < qm0:
        r.append((bs0, qm0))
    if bs1 < qm1:
        r.append((512 + bs1, 512 + qm1))
    if len(r) == 2:
        assert r[0][1] == 512 and r[1][0] == 512, (p, sp, jt, r)
        r = [(r[0][0], r[1][1])]
    return r[0] if r else None


BIASR = [[[_bias_range(p, sp, jt) for jt in range(JT)] for sp in range(2)]
         for p in range(NP_)]


def _heads(g):
    return [2 * p + g for p in range(NP_)]


def _rope_tables():
    inv = 1.0 / (10000.0 ** (np.arange(0, D, 2, dtype=np.float32) / D))
    f = np.arange(N, dtype=np.float32)[:, None] * inv[None, :]
    sin = np.concatenate([np.sin(f), np.sin(f)], axis=-1).astype(np.float32)
    cos = np.concatenate([np.cos(f), np.cos(f)], axis=-1).astype(np.float32)
    return sin, cos  # [N, D]


def _st_table(sin):
    # sign-folded, half-swapped sin table, indexed by SOURCE row r: after the
    # XOR-32 partition permutation, dest row d gets rot_half(q)[d]*sin[d].
    st = np.empty_like(sin)            # [N, D]
    st[:, 0:32] = sin[:, 32:64]
    st[:, 32:64] = -sin[:, 0:32]
    return st


def _shared_inputs(qkv_w, proj_w, proj_b):
    # [I | P32]: P32 is the XOR-32 partition permutation (within 64-blocks)
    shifteye = np.zeros((128, 256), np.float32)
    shifteye[:, 0:128] = np.eye(128)
    for r in range(128):
        shifteye[r, 128 + (r ^ 32)] = 1.0

    # master ALiBi pattern: master[k, x] = min(k - x, 0); the tile for
    # (jt, block i0) is the column window shifted by o = i0 - 128*jt
    jl = np.arange(128, dtype=np.float32)[:, None]
    xl = np.arange(2048, dtype=np.float32)[None, :]
    master = np.minimum(jl - xl, 0.0).astype(NPBF)

    sin, cos = _rope_tables()
    cos2 = np.tile(cos.T, (2, 1)).astype(NPBF)    # [128, N]
    ssin2 = np.tile(_st_table(sin).T, (2, 1)).astype(NPBF)
    return {
        "shifteye": shifteye.astype(NPBF),
        "master": master,
        "cos2": cos2, "ssin2": ssin2,
    }, qkv_w, (proj_w, proj_b)


def _core_inputs(c, x, shared, qkv_w, proj):
    proj_w, proj_b = proj
    b, g = c // 2, c % 2
    heads = _heads(g)
    cols = np.concatenate([np.arange(64 * h, 64 * h + 64) for h in heads])

    wqT = np.ascontiguousarray(qkv_w[0:C].T)[:, cols] * SCALE     # [C, 256]
    wkT = np.ascontiguousarray(qkv_w[C:2 * C].T)[:, cols]
    wvT = np.ascontiguousarray(qkv_w[2 * C:3 * C].T)[:, cols]
    wcat = np.concatenate([wqT, wkT, wvT], axis=1).astype(NPBF)   # [C, 768]

    c8eye = np.zeros((NP_, 128, 128), np.float32)
    for p in range(NP_):
        np.fill_diagonal(c8eye[p], C8[heads[p]])

    projwt = np.ascontiguousarray(proj_w.T)[cols, :]              # [256, C]
    biasb = np.tile(proj_b[None, :], (128, 1)) if g == 0 else \
        np.zeros((128, C), np.float32)

    return {
        "xt": np.ascontiguousarray(x[b].T).astype(NPBF),          # [C, N]
        "wcat": wcat,
        "c8eye": c8eye.astype(NPBF),
        "projwt": projwt.astype(NPBF),
        "biasb": biasb.astype(np.float32),
        **shared,
    }


def _build_graph():
    nc = bacc.Bacc("TRN2", target_bir_lowering=False, debug=False,
                   num_devices=NCORES)

    xt_d = nc.dram_tensor("xt", [C, N], BF16, kind="ExternalInput").ap()
    wcat_d = nc.dram_tensor("wcat", [C, 768], BF16, kind="ExternalInput").ap()
    cos2_d = nc.dram_tensor("cos2", [128, N], BF16, kind="ExternalInput").ap()
    ssin2_d = nc.dram_tensor("ssin2", [128, N], BF16, kind="ExternalInput").ap()
    shifteye_d = nc.dram_tensor("shifteye", [128, 256], BF16, kind="ExternalInput").ap()
    c8eye_d = nc.dram_tensor("c8eye", [NP_, 128, 128], BF16, kind="ExternalInput").ap()
    master_d = nc.dram_tensor("master", [128, 2048], BF16, kind="ExternalInput").ap()
    projwt_d = nc.dram_tensor("projwt", [256, C], BF16, kind="ExternalInput").ap()
    biasb_d = nc.dram_tensor("biasb", [128, C], F32, kind="ExternalInput").ap()
    out_d = nc.dram_tensor("out", [N, C], BF16, kind="ExternalOutput").ap()

    with tile.TileContext(nc) as tc:
        _body(nc, tc, xt_d, wcat_d, cos2_d, ssin2_d, shifteye_d, c8eye_d,
              master_d, projwt_d, biasb_d, out_d)
    nc.compile()
    return nc


def _body(nc, tc, xt_d, wcat_d, cos2_d, ssin2_d, shifteye_d, c8eye_d,
          master_d, projwt_d, biasb_d, out_d):
    from contextlib import ExitStack
    ctx = ExitStack()
    persist = ctx.enter_context(tc.tile_pool(name="persist", bufs=1))
    rope_pool = ctx.enter_context(tc.tile_pool(name="rope", bufs=2))
    exp_pool = ctx.enter_context(tc.tile_pool(name="exp", bufs=6))
    norm_pool = ctx.enter_context(tc.tile_pool(name="norm", bufs=4))
    fin_pool = ctx.enter_context(tc.tile_pool(name="final", bufs=8))
    pspool = ctx.enter_context(tc.tile_pool(name="ps", bufs=2, space="PSUM"))

    def ptile(shape, dtype, tag):
        return persist.tile(shape, dtype, tag=tag, name=tag)

    Exp = mybir.ActivationFunctionType.Exp

    # ---- persistent SBUF tiles ----
    w_sb = [ptile([128, 768], BF16, f"w{i}") for i in range(4)]
    xt_sb = [ptile([128, N], BF16, f"xt{i}") for i in range(4)]
    cos2 = ptile([128, N], BF16, "cos2")
    ssin2 = ptile([128, N], BF16, "ssin2")
    shifteye = ptile([128, 256], BF16, "shifteye")
    c8eye_sb = [ptile([128, 128], BF16, f"c8e{p}") for p in range(NP_)]
    master = ptile([128, 2048], BF16, "master")
    projw_sb = [ptile([128, C], BF16, f"pw{bd}") for bd in range(2)]
    biasb = ptile([128, C], F32, "biasb")
    q2_sb = [ptile([128, N], BF16, f"q2_{bd}") for bd in range(2)]
    k2_sb = [ptile([128, N], BF16, f"k2_{bd}") for bd in range(2)]
    v_sb = [ptile([128, NP_ * 65], BF16, f"v_{nt}") for nt in range(JT)]
    out_pair = [ptile([128, N], BF16, f"op_{bd}") for bd in range(2)]

    # ---- input DMAs, ordered to feed the PE emission order below ----
    # 1) V weights interleaved with the high xt columns so the first V
    # matmul starts after two transfers (V tiles run jt=15..0)
    for i in range(4):
        nc.sync.dma_start(w_sb[i][:, 512:768],
                          wcat_d[i * 128:(i + 1) * 128, 512:768])
        nc.sync.dma_start(xt_sb[i][:, 1536:2048],
                          xt_d[i * 128:(i + 1) * 128, 1536:2048])
    # 2) q/k weights + rope tables
    for i in range(4):
        nc.sync.dma_start(w_sb[i][:, 0:512], wcat_d[i * 128:(i + 1) * 128, 0:512])
    nc.sync.dma_start(shifteye[:], shifteye_d[:])
    nc.sync.dma_start(cos2[:], cos2_d[:])
    nc.sync.dma_start(ssin2[:], ssin2_d[:])
    # 3) remaining xt (descending), bias tables, proj weights
    for blk in (2, 1, 0):
        for i in range(4):
            nc.sync.dma_start(xt_sb[i][:, blk * 512:(blk + 1) * 512],
                              xt_d[i * 128:(i + 1) * 128, blk * 512:(blk + 1) * 512])
    nc.sync.dma_start(master[:], master_d[:])
    for p in range(NP_):
        nc.sync.dma_start(c8eye_sb[p][:], c8eye_d[p])
    for bd in range(2):
        nc.sync.dma_start(projw_sb[bd][:], projwt_d[bd * 128:(bd + 1) * 128, :])
    nc.sync.dma_start(biasb[:], biasb_d[:])

    # ---- helpers ----
    def v_tile(jt):
        psv = pspool.tile([128, 256], F32, tag="aux", name="psv")
        for ci in range(4):
            nc.tensor.matmul(
                psv[:], xt_sb[ci][:, jt * 128:(jt + 1) * 128],
                w_sb[ci][:, 512:768],
                start=(ci == 0), stop=(ci == 3))
        vdst = v_sb[jt].rearrange("p (h e) -> p h e", e=65)
        nc.vector.tensor_copy(vdst[:, :, 0:64],
                              psv.rearrange("p (h e) -> p h e", e=64))
        nc.gpsimd.memset(vdst[:, :, 64:65], 1.0)

    def qk_chunk_a(bd, kind, ch):
        # projection matmuls + cos/sin products for one 512-token chunk of
        # bundle bd (positions 2bd, 2bd+1), kind 0=q (scaled) 1=k.
        w_off = kind * 256 + bd * 128
        c0 = ch * 512
        ps_q = pspool.tile([128, 512], F32, tag="aux", name="ps_q")
        for ci in range(4):
            nc.tensor.matmul(
                ps_q[:],
                w_sb[ci][:, w_off:w_off + 128],
                xt_sb[ci][:, c0:c0 + 512],
                start=(ci == 0), stop=(ci == 3))
        tc_c = rope_pool.tile([128, 512], BF16, tag="tc", name="tc_c")
        nc.vector.tensor_mul(tc_c[:], ps_q[:], cos2[:, c0:c0 + 512])
        tc_u = rope_pool.tile([128, 512], BF16, tag="tu", name="tc_u")
        nc.vector.tensor_mul(tc_u[:], ps_q[:], ssin2[:, c0:c0 + 512])
        return tc_c, tc_u

    def qk_chunk_b(bd, kind, ch, tc_c, tc_u):
        # combine: dst = tc_c + P32 @ tc_u (partition-XOR-32 via matmul)
        dst_sb = k2_sb[bd] if kind else q2_sb[bd]
        c0 = ch * 512
        ps2 = pspool.tile([128, 512], F32, tag="aux", name="ps2")
        nc.tensor.matmul(ps2[:], shifteye[:, 0:128], tc_c[:],
                         start=True, stop=False)
        nc.tensor.matmul(ps2[:], shifteye[:, 128:256], tc_u[:],
                         start=False, stop=True)
        nc.vector.tensor_copy(dst_sb[:, c0:c0 + 512], ps2[:])

    # software-pipelined chunk list -> closures (B of chunk i rides with
    # A of chunk i+1 so the PE never waits on the DVE products)
    def chunk_closures(chunks):
        state = {}

        def make(i, spec):
            def run():
                if i > 0:
                    pb, pkd, pch = chunks[i - 1]
                    qk_chunk_b(pb, pkd, pch, *state.pop(i - 1))
                if spec is not None:
                    bd, kd, ch = spec
                    state[i] = qk_chunk_a(bd, kd, ch)
            return run

        return [make(i, spec)
                for i, spec in enumerate(list(chunks) + [None])]

    def bundle_chunks(bd):
        return [(bd, 1, 3), (bd, 1, 2), (bd, 1, 1), (bd, 1, 0),
                (bd, 0, 0), (bd, 0, 1), (bd, 0, 2), (bd, 0, 3)]

    # ---- PE pre-phase: V tiles (desc) interleaved with bundle-0 qk ----
    qk0 = chunk_closures(bundle_chunks(0))
    vt = [lambda jt=jt: v_tile(jt) for jt in range(JT - 1, -1, -1)]
    pre = [vt[0], vt[1], qk0[0], vt[2], vt[3], qk0[1], vt[4], vt[5], qk0[2],
           vt[6], vt[7], qk0[3], vt[8], vt[9], qk0[4], vt[10], vt[11],
           qk0[5], vt[12], vt[13], qk0[6], vt[14], vt[15]]
    for f in pre:
        f()

    # fillers: project bundle 1 while streaming the first two bundle-0 heads;
    # chunks 0..7 of the output projection (whose inputs complete with the
    # sp=0 streams) run inside the sp=1 streams, spreading the output DMAs.
    qk1 = chunk_closures(bundle_chunks(1))
    fillers = {(0, 0): qk1[:5], (0, 1): qk1[5:],
               (0, 2): [qk0[7], qk0[8]],
               (1, 1): [lambda chk=chk: proj_chunk(chk) for chk in range(0, 4)],
               (1, 2): [lambda chk=chk: proj_chunk(chk) for chk in range(4, 8)]}

    def proj_chunk(chk):
        # full output projection of one 128-query chunk (own heads' partial)
        ps = pspool.tile([128, 512], F32, tag="aux", name="ps_proj")
        for bd in range(2):
            nc.tensor.matmul(ps[:], out_pair[bd][:, chk * 128:(chk + 1) * 128],
                             projw_sb[bd][:],
                             start=(bd == 0), stop=(bd == 1),
                             skip_group_check=True)
        fin = fin_pool.tile([128, 512], BF16, tag="f", name="fin")
        nc.vector.tensor_add(fin[:], ps[:], biasb[:])
        nc.sync.dma_start(out_d[chk * 128:(chk + 1) * 128, :], fin[:])

    def norm_sub(sp, p, avt, qi):
        # attn-V is [query-part, head-dim]: the softmax denominator is a
        # per-partition scalar -> reciprocal + tensor_scalar multiply, then
        # PE-transpose back to [dim, query] for the projection.
        bd, row = p // 2, (p % 2) * 64
        col = 128 * (qi % 4)
        rec = norm_pool.tile([128, 1], F32, tag="rc", name="rec")
        nc.vector.reciprocal(rec[:], avt[:, col + 64:col + 65])
        stag = norm_pool.tile([128, 64], BF16, tag="st", name="stag")
        nc.vector.tensor_scalar_mul(stag[:], avt[:, col:col + 64], rec[:])
        pst = pspool.tile([64, 128], BF16, tag="aux", name="pst")
        nc.tensor.transpose(pst[:], stag[:], shifteye[:, 0:128])
        nc.vector.tensor_copy(
            out_pair[bd][row:row + 64,
                         1024 * sp + 128 * qi:1024 * sp + 128 * qi + 128],
            pst[:])

    # ---- attention streams: one per (slot-pair sp, position p) ----
    for sp in range(2):
        for p in range(NP_):
            bd, row = p // 2, (p % 2) * 64
            fl = fillers.get((sp, p), [])
            fi = 0
            jts = list(range(JT - 1, JTMIN[p][sp] - 1, -1))
            av01 = (pspool.tile([128, 512], F32, tag="av0", bufs=1, name="av0"),
                    pspool.tile([128, 512], F32, tag="av1", bufs=1, name="av1"))
            # PSUM accumulation state is per-bank on hardware: only read a
            # subtile once ALL subtiles sharing its bank have stopped.
            norm_at = {}
            norm_at.setdefault(STOPJT[p][sp][4], []).extend([4, 5, 6, 7])
            norm_at.setdefault(STOPJT[p][sp][0], []).extend([0, 1, 2, 3])
            pend = []  # [(jt, av-mm closure)], delayed two steps
            for idx, jt in enumerate(jts):
                while fi < len(fl) and (fi + 1) * len(jts) <= idx * len(fl):
                    fl[fi]()
                    fi += 1
                qm0, qm1, ql = (QM[p][sp][0][jt], QM[p][sp][1][jt],
                                QLIM[p][sp][jt])
                br = BIASR[p][sp][jt]
                ps = pspool.tile([128, 1024], F32, tag="sc", name="ps_sc")
                nc.tensor.matmul(
                    ps[:, 0:qm0],
                    k2_sb[bd][row:row + 64, jt * 128:(jt + 1) * 128],
                    q2_sb[bd][row:row + 64, 1024 * sp:1024 * sp + qm0],
                    start=True, stop=(br is None), tile_position=(row, 0),
                    skip_group_check=True)
                if qm1 > 0:
                    nc.tensor.matmul(
                        ps[:, 512:512 + qm1],
                        k2_sb[bd][row:row + 64, jt * 128:(jt + 1) * 128],
                        q2_sb[bd][row:row + 64,
                                  1024 * sp + 512:1024 * sp + 512 + qm1],
                        start=True, stop=(br is None), tile_position=(row, 0),
                        skip_group_check=True)
                if br is not None:
                    o = 1024 * sp - 128 * jt
                    # split at the 512-col PSUM bank boundary
                    for lo, hi in ((br[0], min(br[1], 512)),
                                   (max(br[0], 512), br[1])):
                        if lo < hi:
                            nc.tensor.matmul(
                                ps[:, lo:hi], c8eye_sb[p][:],
                                master[:, lo + o:hi + o],
                                start=False, stop=True, tile_position=(0, 0),
                                skip_group_check=True)
                et = exp_pool.tile([128, 1024], BF16, tag="e", name="et")
                nc.scalar.activation(et[:, 0:ql], ps[:, 0:ql], Exp)

                def av_mms(jt=jt, ql=ql, et=et):
                    for qi in range((ql + 127) // 128):
                        m = min(128, ql - 128 * qi)
                        avt = av01[qi // 4]
                        col = 128 * (qi % 4)
                        nc.tensor.matmul(
                            avt[0:m, col:col + 65],
                            et[:, 128 * qi:128 * qi + m],
                            v_sb[jt][:, p * 65:(p + 1) * 65],
                            start=(jt == JT - 1 and qi % 4 == 0),
                            stop=(jt == STOPJT[p][sp][qi]),
                            skip_group_check=True)

                if len(pend) >= 5:
                    pjt, pfn = pend.pop(0)
                    pfn()
                    for qi in norm_at.get(pjt, []):
                        norm_sub(sp, p, av01[qi // 4], qi)
                pend.append((jt, av_mms))
            for pjt, pfn in pend:
                pfn()
                for qi in norm_at.get(pjt, []):
                    norm_sub(sp, p, av01[qi // 4], qi)
            while fi < len(fl):
                fl[fi]()
                fi += 1

    # ---- remaining output projection chunks (sp=1 queries) ----
    # software-pipelined: each chunk's bundle-0 partial runs during the final
    # norm drain; only the bundle-1 matmul waits on the last normalize.
    def proj_pre(chk):
        ps = pspool.tile([128, 512], F32, tag="aux", name="ps_proj")
        nc.tensor.matmul(ps[:], out_pair[0][:, chk * 128:(chk + 1) * 128],
                         projw_sb[0][:],
                         start=True, stop=False, skip_group_check=True)
        return ps

    def proj_fin(chk, ps):
        nc.tensor.matmul(ps[:], out_pair[1][:, chk * 128:(chk + 1) * 128],
                         projw_sb[1][:],
                         start=False, stop=True, skip_group_check=True)
        fin = fin_pool.tile([128, 512], BF16, tag="f", name="fin")
        nc.vector.tensor_add(fin[:], ps[:], biasb[:])
        nc.sync.dma_start(out_d[chk * 128:(chk + 1) * 128, :], fin[:])

    # slot1 chunks (12..15) are unblocked at the last stream's midpoint;
    # slot0 chunks (8..11) wait for the final normalize, so run them last.
    order = [12, 13, 14, 15, 8, 9, 10, 11]
    pend_ps = {order[0]: proj_pre(order[0]), order[1]: proj_pre(order[1])}
    for i, chk in enumerate(order):
        proj_fin(chk, pend_ps.pop(chk))
        if i + 2 < len(order):
            pend_ps[order[i + 2]] = proj_pre(order[i + 2])

    ctx.close()


@functools.lru_cache(maxsize=1)
def _graph():
    return _build_graph()


def kernel(x, qkv_w, proj_w, proj_b):
    global LAST_RESULT
    x = np.asarray(x, np.float32)
    qkv_w = np.asarray(qkv_w, np.float32)
    proj_w = np.asarray(proj_w, np.float32)
    proj_b = np.asarray(proj_b, np.float32)

    nc = _graph()
    shared, qw, proj = _shared_inputs(qkv_w, proj_w, proj_b)
    in_maps = [_core_inputs(c, x, shared, qw, proj) for c in range(NCORES)]
    res = bass_utils.run_bass_kernel_spmd(nc, in_maps,
                                          core_ids=list(range(NCORES)),
                                          trace=False)
    LAST_RESULT = res
    out = np.zeros((B, N, C), np.float32)
    for b in range(B):
        out[b] = (np.asarray(res.results[2 * b]["out"], np.float32)
                  + np.asarray(res.results[2 * b + 1]["out"], np.float32))
    return out


# revision 117
# speedup vs baseline: 1.1481x; 1.0214x over previous
"""Fused multi-head attention layer (RoPE + ALiBi + softmax + out-proj) on 8 TRN2 cores.

Sharding (v4, heads-split): core c -> (batch b = c//2, head group g = c%2).
Each core computes q/k/v for its 4 heads {g, 2+g, 4+g, 6+g} over ALL 2048
positions, runs banded attention, and projects through its heads' slice of
proj_w, producing a partial [N, C] output; the host sums the two partials
per batch. Pairing adjacent-radius heads per graph position keeps the SPMD
union of ALiBi bands tight, and query blocks have exact (not unioned)
trim bounds.

Pipeline features:
- RoPE rotate-half via a sign-folded sin table + partition-permutation
  matmul (no duplicate rot projections).
- ALiBi bias on PE as c8-scaled identity matmuls against a single shared
  anti-diagonal master pattern table (column-shifted AP views), trimmed to
  the band columns.
- Score/exp/attn-V column ranges prefix-trimmed per (position, block, jt).
- Streams software-pipelined; V/next-bundle projections fill PE gaps.
"""

import functools
import math
import os
import sys

import numpy as np

sys.path.insert(0, "/opt/trn_rl_repo")

import ml_dtypes  # noqa: E402

import concourse.bass as bass  # noqa: E402
import concourse.tile as tile  # noqa: E402
from concourse import bacc, mybir, bass_utils  # noqa: E402

BF16 = mybir.dt.bfloat16
F32 = mybir.dt.float32
NPBF = ml_dtypes.bfloat16

B, N, C, H, D = 4, 2048, 512, 8, 64
NCORES = 8
JT = N // 128        # 16 j-tiles of 128 key positions
NP_ = 4              # head positions per core
T_CUT = 13.0         # ALiBi cutoff in logits: exp(-13) is negligible
SCALE = D ** -0.5

# c8_h = alibi_slope_h * MAX_BIAS = 2^-(h+1) * 8 = 2^(2-h)
C8 = [2.0 ** (2 - h) for h in range(H)]
RADIUS = [T_CUT / c for c in C8]   # band reach (key positions) per head
# graph position p holds heads {2p, 2p+1}; the union band is the odd head's
UR = [RADIUS[2 * p + 1] for p in range(NP_)]

LAST_RESULT = None  # test harness reads exec_time_ns from here


def _clamp(v, lo, hi):
    return max(lo, min(hi, v))


# Frame for (position p, slot-pair sp): cols 0:512 = query block 2sp
# (i0 = 1024sp), cols 512:1024 = block 2sp+1 (i0 = 1024sp+512). Exact bounds.
def _qm(p, sp, sl, jt):
    i0 = 512 * (2 * sp + sl)
    return _clamp(int(math.floor(128 * jt + 127 + UR[p])) + 1 - i0, 0, 512)


QM = [[[[_qm(p, sp, sl, jt) for jt in range(JT)] for sl in range(2)]
       for sp in range(2)] for p in range(NP_)]
QLIM = [[[(QM[p][sp][0][jt] if QM[p][sp][0][jt] < 512
           else 512 + QM[p][sp][1][jt]) for jt in range(JT)]
         for sp in range(2)] for p in range(NP_)]
JTMIN = [[min(jt for jt in range(JT) if QLIM[p][sp][jt] > 0)
          for sp in range(2)] for p in range(NP_)]
# per 128-query subtile qi of the 1024-col frame: the last (smallest) jt
# whose kept prefix still reaches it (attn-V accumulation stop point)
STOPJT = [[[min(jt for jt in range(JT) if QLIM[p][sp][jt] > 128 * qi)
            for qi in range(8)] for sp in range(2)] for p in range(NP_)]


def _bias_range(p, sp, jt):
    qm0, qm1 = QM[p][sp][0][jt], QM[p][sp][1][jt]
    bs0 = max(0, 128 * jt - 1024 * sp + 1)
    bs1 = max(0, 128 * jt - 1024 * sp - 511)
    r = []
    if bs0 < qm0:
        r.append((bs0, qm0))
    if bs1 < qm1:
        r.append((512 + bs1, 512 + qm1))
    if len(r) == 2:
        assert r[0][1] == 512 and r[1][0] == 512, (p, sp, jt, r)
        r = [(r[0][0], r[1][1])]
    return r[0] if r else None


BIASR = [[[_bias_range(p, sp, jt) for jt in range(JT)] for sp in range(2)]
         for p in range(NP_)]


def _heads(g):
    return [2 * p + g for p in range(NP_)]


def _rope_tables():
    inv = 1.0 / (10000.0 ** (np.arange(0, D, 2, dtype=np.float32) / D))
    f = np.arange(N, dtype=np.float32)[:, None] * inv[None, :]
    sin = np.concatenate([np.sin(f), np.sin(f)], axis=-1).astype(np.float32)
    cos = np.concatenate([np.cos(f), np.cos(f)], axis=-1).astype(np.float32)
    return sin, cos  # [N, D]


def _st_table(sin):
    # sign-folded, half-swapped sin table, indexed by SOURCE row r: after the
    # XOR-32 partition permutation, dest row d gets rot_half(q)[d]*sin[d].
    st = np.empty_like(sin)            # [N, D]
    st[:, 0:32] = sin[:, 32:64]
    st[:, 32:64] = -sin[:, 0:32]
    return st


def _shared_inputs(qkv_w, proj_w, proj_b):
    # [I | P32]: P32 is the XOR-32 partition permutation (within 64-blocks)
    shifteye = np.zeros((128, 256), np.float32)
    shifteye[:, 0:128] = np.eye(128)
    for r in range(128):
        shifteye[r, 128 + (r ^ 32)] = 1.0

    # master ALiBi pattern: master[k, x] = min(k - x, 0); the tile for
    # (jt, block i0) is the column window shifted by o = i0 - 128*jt
    jl = np.arange(128, dtype=np.float32)[:, None]
    xl = np.arange(2048, dtype=np.float32)[None, :]
    master = np.minimum(jl - xl, 0.0).astype(NPBF)

    sin, cos = _rope_tables()
    cos2 = np.tile(cos.T, (2, 1)).astype(NPBF)    # [128, N]
    ssin2 = np.tile(_st_table(sin).T, (2, 1)).astype(NPBF)
    return {
        "shifteye": shifteye.astype(NPBF),
        "master": master,
        "cos2": cos2, "ssin2": ssin2,
    }, qkv_w, (proj_w, proj_b)


def _core_inputs(c, x, shared, qkv_w, proj):
    proj_w, proj_b = proj
    b, g = c // 2, c % 2
    heads = _heads(g)
    cols = np.concatenate([np.arange(64 * h, 64 * h + 64) for h in heads])

    wqT = np.ascontiguousarray(qkv_w[0:C].T)[:, cols] * SCALE     # [C, 256]
    wkT = np.ascontiguousarray(qkv_w[C:2 * C].T)[:, cols]
    wvT = np.ascontiguousarray(qkv_w[2 * C:3 * C].T)[:, cols]
    wcat = np.concatenate([wqT, wkT, wvT], axis=1).astype(NPBF)   # [C, 768]

    c8eye = np.zeros((NP_, 128, 128), np.float32)
    for p in range(NP_):
        np.fill_diagonal(c8eye[p], C8[heads[p]])

    projwt = np.ascontiguousarray(proj_w.T)[cols, :]              # [256, C]
    biasb = np.tile(proj_b[None, :], (128, 1)) if g == 0 else \
        np.zeros((128, C), np.float32)

    return {
        "xt": np.ascontiguousarray(x[b].T).astype(NPBF),          # [C, N]
        "wcat": wcat,
        "c8eye": c8eye.astype(NPBF),
        "projwt": projwt.astype(NPBF),
        "biasb": biasb.astype(np.float32),
        **shared,
    }


def _build_graph():
    nc = bacc.Bacc("TRN2", target_bir_lowering=False, debug=False,
                   num_devices=NCORES)

    xt_d = nc.dram_tensor("xt", [C, N], BF16, kind="ExternalInput").ap()
    wcat_d = nc.dram_tensor("wcat", [C, 768], BF16, kind="ExternalInput").ap()
    cos2_d = nc.dram_tensor("cos2", [128, N], BF16, kind="ExternalInput").ap()
    ssin2_d = nc.dram_tensor("ssin2", [128, N], BF16, kind="ExternalInput").ap()
    shifteye_d = nc.dram_tensor("shifteye", [128, 256], BF16, kind="ExternalInput").ap()
    c8eye_d = nc.dram_tensor("c8eye", [NP_, 128, 128], BF16, kind="ExternalInput").ap()
    master_d = nc.dram_tensor("master", [128, 2048], BF16, kind="ExternalInput").ap()
    projwt_d = nc.dram_tensor("projwt", [256, C], BF16, kind="ExternalInput").ap()
    biasb_d = nc.dram_tensor("biasb", [128, C], F32, kind="ExternalInput").ap()
    out_d = nc.dram_tensor("out", [N, C], BF16, kind="ExternalOutput").ap()

    with tile.TileContext(nc) as tc:
        _body(nc, tc, xt_d, wcat_d, cos2_d, ssin2_d, shifteye_d, c8eye_d,
              master_d, projwt_d, biasb_d, out_d)
    nc.compile()
    return nc


def _body(nc, tc, xt_d, wcat_d, cos2_d, ssin2_d, shifteye_d, c8eye_d,
          master_d, projwt_d, biasb_d, out_d):
    from contextlib import ExitStack
    ctx = ExitStack()
    persist = ctx.enter_context(tc.tile_pool(name="persist", bufs=1))
    rope_pool = ctx.enter_context(tc.tile_pool(name="rope", bufs=2))
    exp_pool = ctx.enter_context(tc.tile_pool(name="exp", bufs=6))
    norm_pool = ctx.enter_context(tc.tile_pool(name="norm", bufs=4))
    fin_pool = ctx.enter_context(tc.tile_pool(name="final", bufs=8))
    pspool = ctx.enter_context(tc.tile_pool(name="ps", bufs=2, space="PSUM"))

    def ptile(shape, dtype, tag):
        return persist.tile(shape, dtype, tag=tag, name=tag)

    Exp = mybir.ActivationFunctionType.Exp

    # ---- persistent SBUF tiles ----
    w_sb = [ptile([128, 768], BF16, f"w{i}") for i in range(4)]
    xt_sb = [ptile([128, N], BF16, f"xt{i}") for i in range(4)]
    cos2 = ptile([128, N], BF16, "cos2")
    ssin2 = ptile([128, N], BF16, "ssin2")
    shifteye = ptile([128, 256], BF16, "shifteye")
    c8eye_sb = [ptile([128, 128], BF16, f"c8e{p}") for p in range(NP_)]
    master = ptile([128, 2048], BF16, "master")
    projw_sb = [ptile([128, C], BF16, f"pw{bd}") for bd in range(2)]
    biasb = ptile([128, C], F32, "biasb")
    q2_sb = [ptile([128, N], BF16, f"q2_{bd}") for bd in range(2)]
    k2_sb = [ptile([128, N], BF16, f"k2_{bd}") for bd in range(2)]
    v_sb = [ptile([128, NP_ * 65], BF16, f"v_{nt}") for nt in range(JT)]
    out_pair = [ptile([128, N], BF16, f"op_{bd}") for bd in range(2)]

    # ---- input DMAs, ordered to feed the PE emission order below ----
    # 1) V weights interleaved with the high xt columns so the first V
    # matmul starts after two transfers (V tiles run jt=15..0)
    for i in range(4):
        nc.sync.dma_start(w_sb[i][:, 512:768],
                          wcat_d[i * 128:(i + 1) * 128, 512:768])
        nc.sync.dma_start(xt_sb[i][:, 1536:2048],
                          xt_d[i * 128:(i + 1) * 128, 1536:2048])
    # 2) q/k weights + rope tables
    for i in range(4):
        nc.sync.dma_start(w_sb[i][:, 0:512], wcat_d[i * 128:(i + 1) * 128, 0:512])
    nc.sync.dma_start(shifteye[:], shifteye_d[:])
    nc.sync.dma_start(cos2[:], cos2_d[:])
    nc.sync.dma_start(ssin2[:], ssin2_d[:])
    # 3) remaining xt (descending), bias tables, proj weights
    for blk in (2, 1, 0):
        for i in range(4):
            nc.sync.dma_start(xt_sb[i][:, blk * 512:(blk + 1) * 512],
                              xt_d[i * 128:(i + 1) * 128, blk * 512:(blk + 1) * 512])
    nc.sync.dma_start(master[:], master_d[:])
    for p in range(NP_):
        nc.sync.dma_start(c8eye_sb[p][:], c8eye_d[p])
    for bd in range(2):
        nc.sync.dma_start(projw_sb[bd][:], projwt_d[bd * 128:(bd + 1) * 128, :])
    nc.sync.dma_start(biasb[:], biasb_d[:])

    # ---- helpers ----
    def v_tile(jt):
        psv = pspool.tile([128, 256], F32, tag="aux", name="psv")
        for ci in range(4):
            nc.tensor.matmul(
                psv[:], xt_sb[ci][:, jt * 128:(jt + 1) * 128],
                w_sb[ci][:, 512:768],
                start=(ci == 0), stop=(ci == 3))
        vdst = v_sb[jt].rearrange("p (h e) -> p h e", e=65)
        nc.vector.tensor_copy(vdst[:, :, 0:64],
                              psv.rearrange("p (h e) -> p h e", e=64))
        nc.gpsimd.memset(vdst[:, :, 64:65], 1.0)

    def qk_chunk_a(bd, kind, ch):
        # projection matmuls + cos/sin products for one 512-token chunk of
        # bundle bd (positions 2bd, 2bd+1), kind 0=q (scaled) 1=k.
        w_off = kind * 256 + bd * 128
        c0 = ch * 512
        ps_q = pspool.tile([128, 512], F32, tag="aux", name="ps_q")
        for ci in range(4):
            nc.tensor.matmul(
                ps_q[:],
                w_sb[ci][:, w_off:w_off + 128],
                xt_sb[ci][:, c0:c0 + 512],
                start=(ci == 0), stop=(ci == 3))
        tc_c = rope_pool.tile([128, 512], BF16, tag="tc", name="tc_c")
        nc.vector.tensor_mul(tc_c[:], ps_q[:], cos2[:, c0:c0 + 512])
        tc_u = rope_pool.tile([128, 512], BF16, tag="tu", name="tc_u")
        nc.vector.tensor_mul(tc_u[:], ps_q[:], ssin2[:, c0:c0 + 512])
        return tc_c, tc_u

    def qk_chunk_b(bd, kind, ch, tc_c, tc_u):
        # combine: dst = tc_c + P32 @ tc_u (partition-XOR-32 via matmul)
        dst_sb = k2_sb[bd] if kind else q2_sb[bd]
        c0 = ch * 512
        ps2 = pspool.tile([128, 512], F32, tag="aux", name="ps2")
        nc.tensor.matmul(ps2[:], shifteye[:, 0:128], tc_c[:],
                         start=True, stop=False)
        nc.tensor.matmul(ps2[:], shifteye[:, 128:256], tc_u[:],
                         start=False, stop=True)
        nc.vector.tensor_copy(dst_sb[:, c0:c0 + 512], ps2[:])

    # software-pipelined chunk list -> closures (B of chunk i rides with
    # A of chunk i+1 so the PE never waits on the DVE products)
    def chunk_closures(chunks):
        state = {}

        def make(i, spec):
            def run():
                if i > 0:
                    pb, pkd, pch = chunks[i - 1]
                    qk_chunk_b(pb, pkd, pch, *state.pop(i - 1))
                if spec is not None:
                    bd, kd, ch = spec
                    state[i] = qk_chunk_a(bd, kd, ch)
            return run

        return [make(i, spec)
                for i, spec in enumerate(list(chunks) + [None])]

    def bundle_chunks(bd):
        return [(bd, 1, 3), (bd, 1, 2), (bd, 1, 1), (bd, 1, 0),
                (bd, 0, 0), (bd, 0, 1), (bd, 0, 2), (bd, 0, 3)]

    # ---- PE pre-phase: V tiles (desc) interleaved with bundle-0 qk ----
    qk0 = chunk_closures(bundle_chunks(0))
    vt = [lambda jt=jt: v_tile(jt) for jt in range(JT - 1, -1, -1)]
    pre = [vt[0], vt[1], qk0[0], vt[2], vt[3], qk0[1], vt[4], vt[5], qk0[2],
           vt[6], vt[7], qk0[3], vt[8], vt[9], qk0[4], vt[10], vt[11],
           qk0[5], vt[12], vt[13], qk0[6], vt[14], vt[15]]
    for f in pre:
        f()

    # fillers: project bundle 1 while streaming the first two bundle-0 heads;
    # chunks 0..7 of the output projection (whose inputs complete with the
    # sp=0 streams) run inside the sp=1 streams, spreading the output DMAs.
    qk1 = chunk_closures(bundle_chunks(1))
    fillers = {(0, 0): qk1[:5], (0, 1): qk1[5:],
               (0, 2): [qk0[7], qk0[8]],
               (1, 1): [lambda chk=chk: proj_chunk(chk) for chk in range(0, 4)],
               (1, 2): [lambda chk=chk: proj_chunk(chk) for chk in range(4, 8)]}

    def proj_chunk(chk):
        # full output projection of one 128-query chunk (own heads' partial)
        ps = pspool.tile([128, 512], F32, tag="aux", name="ps_proj")
        for bd in range(2):
            nc.tensor.matmul(ps[:], out_pair[bd][:, chk * 128:(chk + 1) * 128],
                             projw_sb[bd][:],
                             start=(bd == 0), stop=(bd == 1),
                             skip_group_check=True)
        fin = fin_pool.tile([128, 512], BF16, tag="f", name="fin")
        nc.vector.tensor_add(fin[:], ps[:], biasb[:])
        nc.sync.dma_start(out_d[chk * 128:(chk + 1) * 128, :], fin[:])

    def norm_sub(sp, p, avt, qi):
        # attn-V is [query-part, head-dim]: the softmax denominator is a
        # per-partition scalar -> reciprocal + tensor_scalar multiply, then
        # PE-transpose back to [dim, query] for the projection.
        bd, row = p // 2, (p % 2) * 64
        col = 128 * (qi % 4)
        rec = norm_pool.tile([128, 1], F32, tag="rc", name="rec")
        nc.vector.reciprocal(rec[:], avt[:, col + 64:col + 65])
        stag = norm_pool.tile([128, 64], BF16, tag="st", name="stag")
        nc.vector.tensor_scalar_mul(stag[:], avt[:, col:col + 64], rec[:])
        pst = pspool.tile([64, 128], BF16, tag="aux", name="pst")
        nc.tensor.transpose(pst[:], stag[:], shifteye[:, 0:128])
        nc.vector.tensor_copy(
            out_pair[bd][row:row + 64,
                         1024 * sp + 128 * qi:1024 * sp + 128 * qi + 128],
            pst[:])

    # ---- attention streams: one per (slot-pair sp, position p) ----
    for sp in range(2):
        for p in range(NP_):
            bd, row = p // 2, (p % 2) * 64
            fl = fillers.get((sp, p), [])
            fi = 0
            jts = list(range(JT - 1, JTMIN[p][sp] - 1, -1))
            av01 = (pspool.tile([128, 512], F32, tag="av0", bufs=1, name="av0"),
                    pspool.tile([128, 512], F32, tag="av1", bufs=1, name="av1"))
            # PSUM accumulation state is per-bank on hardware: only read a
            # subtile once ALL subtiles sharing its bank have stopped.
            norm_at = {}
            norm_at.setdefault(STOPJT[p][sp][4], []).extend([4, 5, 6, 7])
            norm_at.setdefault(STOPJT[p][sp][0], []).extend([0, 1, 2, 3])
            pend = []  # [(jt, av-mm closure)], delayed two steps
            for idx, jt in enumerate(jts):
                while fi < len(fl) and (fi + 1) * len(jts) <= idx * len(fl):
                    fl[fi]()
                    fi += 1
                qm0, qm1, ql = (QM[p][sp][0][jt], QM[p][sp][1][jt],
                                QLIM[p][sp][jt])
                br = BIASR[p][sp][jt]
                ps = pspool.tile([128, 1024], F32, tag="sc", name="ps_sc")
                nc.tensor.matmul(
                    ps[:, 0:qm0],
                    k2_sb[bd][row:row + 64, jt * 128:(jt + 1) * 128],
                    q2_sb[bd][row:row + 64, 1024 * sp:1024 * sp + qm0],
                    start=True, stop=(br is None), tile_position=(row, 0),
                    skip_group_check=True)
                if qm1 > 0:
                    nc.tensor.matmul(
                        ps[:, 512:512 + qm1],
                        k2_sb[bd][row:row + 64, jt * 128:(jt + 1) * 128],
                        q2_sb[bd][row:row + 64,
                                  1024 * sp + 512:1024 * sp + 512 + qm1],
                        start=True, stop=(br is None), tile_position=(row, 0),
                        skip_group_check=True)
                if br is not None:
                    o = 1024 * sp - 128 * jt
                    # split at the 512-col PSUM bank boundary
                    for lo, hi in ((br[0], min(br[1], 512)),
                                   (max(br[0], 512), br[1])):
                        if lo < hi:
                            nc.tensor.matmul(
                                ps[:, lo:hi], c8eye_sb[p][:],
                                master[:, lo + o:hi + o],
                                start=False, stop=True, tile_position=(0, 0),
                                skip_group_check=True)
                et = exp_pool.tile([128, 1024], BF16, tag="e", name="et")
                nc.scalar.activation(et[:, 0:ql], ps[:, 0:ql], Exp)

                def av_mms(jt=jt, ql=ql, et=et):
                    for qi in range((ql + 127) // 128):
                        m = min(128, ql - 128 * qi)
                        avt = av01[qi // 4]
                        col = 128 * (qi % 4)
                        nc.tensor.matmul(
                            avt[0:m, col:col + 65],
                            et[:, 128 * qi:128 * qi + m],
                            v_sb[jt][:, p * 65:(p + 1) * 65],
                            start=(jt == JT - 1 and qi % 4 == 0),
                            stop=(jt == STOPJT[p][sp][qi]),
                            skip_group_check=True)

                if len(pend) >= 5:
                    pjt, pfn = pend.pop(0)
                    pfn()
                    for qi in norm_at.get(pjt, []):
                        norm_sub(sp, p, av01[qi // 4], qi)
                pend.append((jt, av_mms))
            for pjt, pfn in pend:
                pfn()
                for qi in norm_at.get(pjt, []):
                    norm_sub(sp, p, av01[qi // 4], qi)
            while fi < len(fl):
                fl[fi]()
                fi += 1

    # ---- remaining output projection chunks (sp=1 queries) ----
    # software-pipelined: each chunk's bundle-0 partial runs during the final
    # norm drain; only the bundle-1 matmul waits on the last normalize.
    def proj_pre(chk):
        ps = pspool.tile([128, 512], F32, tag="aux", name="ps_proj")
        nc.tensor.matmul(ps[:], out_pair[0][:, chk * 128:(chk + 1) * 128],
                         projw_sb[0][:],
                         start=True, stop=False, skip_group_check=True)
        return ps

    def proj_fin(chk, ps):
        nc.tensor.matmul(ps[:], out_pair[1][:, chk * 128:(chk + 1) * 128],
                         projw_sb[1][:],
                         start=False, stop=True, skip_group_check=True)
        fin = fin_pool.tile([128, 512], BF16, tag="f", name="fin")
        nc.vector.tensor_add(fin[:], ps[:], biasb[:])
        nc.sync.dma_start(out_d[chk * 128:(chk + 1) * 128, :], fin[:])

    # slot1 chunks (12..15) are unblocked at the last stream's midpoint;
    # slot0 chunks (8..11) wait for the final normalize, so run them last.
    order = [12, 13, 14, 15, 8, 9, 10, 11]
    pend_ps = {order[0]: proj_pre(order[0]), order[1]: proj_pre(order[1])}
    for i, chk in enumerate(order):
        proj_fin(chk, pend_ps.pop(chk))
        if i + 2 < len(order):
            pend_ps[order[i + 2]] = proj_pre(order[i + 2])

    ctx.close()


@functools.lru_cache(maxsize=1)
def _graph():
    return _build_graph()


def kernel(x, qkv_w, proj_w, proj_b):
    global LAST_RESULT
    x = np.asarray(x, np.float32)
    qkv_w = np.asarray(qkv_w, np.float32)
    proj_w = np.asarray(proj_w, np.float32)
    proj_b = np.asarray(proj_b, np.float32)

    nc = _graph()
    shared, qw, proj = _shared_inputs(qkv_w, proj_w, proj_b)
    in_maps = [_core_inputs(c, x, shared, qw, proj) for c in range(NCORES)]
    res = bass_utils.run_bass_kernel_spmd(nc, in_maps,
                                          core_ids=list(range(NCORES)),
                                          trace=False)
    LAST_RESULT = res
    out = np.zeros((B, N, C), np.float32)
    for b in range(B):
        out[b] = (np.asarray(res.results[2 * b]["out"], np.float32)
                  + np.asarray(res.results[2 * b + 1]["out"], np.float32))
    return out


# revision 118
# speedup vs baseline: 1.1593x; 1.0097x over previous
"""Fused multi-head attention layer (RoPE + ALiBi + softmax + out-proj) on 8 TRN2 cores.

Sharding (v4, heads-split): core c -> (batch b = c//2, head group g = c%2).
Each core computes q/k/v for its 4 heads {g, 2+g, 4+g, 6+g} over ALL 2048
positions, runs banded attention, and projects through its heads' slice of
proj_w, producing a partial [N, C] output; the host sums the two partials
per batch. Pairing adjacent-radius heads per graph position keeps the SPMD
union of ALiBi bands tight, and query blocks have exact (not unioned)
trim bounds.

Pipeline features:
- RoPE rotate-half via a sign-folded sin table + partition-permutation
  matmul (no duplicate rot projections).
- ALiBi bias on PE as c8-scaled identity matmuls against a single shared
  anti-diagonal master pattern table (column-shifted AP views), trimmed to
  the band columns.
- Score/exp/attn-V column ranges prefix-trimmed per (position, block, jt).
- Streams software-pipelined; V/next-bundle projections fill PE gaps.
"""

import functools
import math
import os
import sys

import numpy as np

sys.path.insert(0, "/opt/trn_rl_repo")

import ml_dtypes  # noqa: E402

import concourse.bass as bass  # noqa: E402
import concourse.tile as tile  # noqa: E402
from concourse import bacc, mybir, bass_utils  # noqa: E402

BF16 = mybir.dt.bfloat16
F32 = mybir.dt.float32
NPBF = ml_dtypes.bfloat16

B, N, C, H, D = 4, 2048, 512, 8, 64
NCORES = 8
JT = N // 128        # 16 j-tiles of 128 key positions
NP_ = 4              # head positions per core
T_CUT = 10.0         # ALiBi cutoff in logits: exp(-10) is negligible
SCALE = D ** -0.5

# c8_h = alibi_slope_h * MAX_BIAS = 2^-(h+1) * 8 = 2^(2-h)
C8 = [2.0 ** (2 - h) for h in range(H)]
RADIUS = [T_CUT / c for c in C8]   # band reach (key positions) per head
# graph position p holds heads {2p, 2p+1}; the union band is the odd head's
UR = [RADIUS[2 * p + 1] for p in range(NP_)]

LAST_RESULT = None  # test harness reads exec_time_ns from here


def _clamp(v, lo, hi):
    return max(lo, min(hi, v))


# Frame for (position p, slot-pair sp): cols 0:512 = query block 2sp
# (i0 = 1024sp), cols 512:1024 = block 2sp+1 (i0 = 1024sp+512). Exact bounds.
def _qm(p, sp, sl, jt):
    i0 = 512 * (2 * sp + sl)
    return _clamp(int(math.floor(128 * jt + 127 + UR[p])) + 1 - i0, 0, 512)


QM = [[[[_qm(p, sp, sl, jt) for jt in range(JT)] for sl in range(2)]
       for sp in range(2)] for p in range(NP_)]
QLIM = [[[(QM[p][sp][0][jt] if QM[p][sp][0][jt] < 512
           else 512 + QM[p][sp][1][jt]) for jt in range(JT)]
         for sp in range(2)] for p in range(NP_)]
JTMIN = [[min(jt for jt in range(JT) if QLIM[p][sp][jt] > 0)
          for sp in range(2)] for p in range(NP_)]
# per 128-query subtile qi of the 1024-col frame: the last (smallest) jt
# whose kept prefix still reaches it (attn-V accumulation stop point)
STOPJT = [[[min(jt for jt in range(JT) if QLIM[p][sp][jt] > 128 * qi)
            for qi in range(8)] for sp in range(2)] for p in range(NP_)]


def _bias_range(p, sp, jt):
    qm0, qm1 = QM[p][sp][0][jt], QM[p][sp][1][jt]
    bs0 = max(0, 128 * jt - 1024 * sp + 1)
    bs1 = max(0, 128 * jt - 1024 * sp - 511)
    r = []
    if bs0 < qm0:
        r.append((bs0, qm0))
    if bs1 < qm1:
        r.append((512 + bs1, 512 + qm1))
    if len(r) == 2:
        assert r[0][1] == 512 and r[1][0] == 512, (p, sp, jt, r)
        r = [(r[0][0], r[1][1])]
    return r[0] if r else None


BIASR = [[[_bias_range(p, sp, jt) for jt in range(JT)] for sp in range(2)]
         for p in range(NP_)]


def _heads(g):
    return [2 * p + g for p in range(NP_)]


def _rope_tables():
    inv = 1.0 / (10000.0 ** (np.arange(0, D, 2, dtype=np.float32) / D))
    f = np.arange(N, dtype=np.float32)[:, None] * inv[None, :]
    sin = np.concatenate([np.sin(f), np.sin(f)], axis=-1).astype(np.float32)
    cos = np.concatenate([np.cos(f), np.cos(f)], axis=-1).astype(np.float32)
    return sin, cos  # [N, D]


def _st_table(sin):
    # sign-folded, half-swapped sin table, indexed by SOURCE row r: after the
    # XOR-32 partition permutation, dest row d gets rot_half(q)[d]*sin[d].
    st = np.empty_like(sin)            # [N, D]
    st[:, 0:32] = sin[:, 32:64]
    st[:, 32:64] = -sin[:, 0:32]
    return st


def _shared_inputs(qkv_w, proj_w, proj_b):
    # [I | P32]: P32 is the XOR-32 partition permutation (within 64-blocks)
    shifteye = np.zeros((128, 256), np.float32)
    shifteye[:, 0:128] = np.eye(128)
    for r in range(128):
        shifteye[r, 128 + (r ^ 32)] = 1.0

    # master ALiBi pattern: master[k, x] = min(k - x, 0); the tile for
    # (jt, block i0) is the column window shifted by o = i0 - 128*jt
    jl = np.arange(128, dtype=np.float32)[:, None]
    xl = np.arange(2048, dtype=np.float32)[None, :]
    master = np.minimum(jl - xl, 0.0).astype(NPBF)

    sin, cos = _rope_tables()
    cos2 = np.tile(cos.T, (2, 1)).astype(NPBF)    # [128, N]
    ssin2 = np.tile(_st_table(sin).T, (2, 1)).astype(NPBF)
    return {
        "shifteye": shifteye.astype(NPBF),
        "master": master,
        "cos2": cos2, "ssin2": ssin2,
    }, qkv_w, (proj_w, proj_b)


def _core_inputs(c, x, shared, qkv_w, proj):
    proj_w, proj_b = proj
    b, g = c // 2, c % 2
    heads = _heads(g)
    cols = np.concatenate([np.arange(64 * h, 64 * h + 64) for h in heads])

    wqT = np.ascontiguousarray(qkv_w[0:C].T)[:, cols] * SCALE     # [C, 256]
    wkT = np.ascontiguousarray(qkv_w[C:2 * C].T)[:, cols]
    wvT = np.ascontiguousarray(qkv_w[2 * C:3 * C].T)[:, cols]
    wcat = np.concatenate([wqT, wkT, wvT], axis=1).astype(NPBF)   # [C, 768]

    c8eye = np.zeros((NP_, 128, 128), np.float32)
    for p in range(NP_):
        np.fill_diagonal(c8eye[p], C8[heads[p]])

    projwt = np.ascontiguousarray(proj_w.T)[cols, :]              # [256, C]
    biasb = np.tile(proj_b[None, :], (128, 1)) if g == 0 else \
        np.zeros((128, C), np.float32)

    return {
        "xt": np.ascontiguousarray(x[b].T).astype(NPBF),          # [C, N]
        "wcat": wcat,
        "c8eye": c8eye.astype(NPBF),
        "projwt": projwt.astype(NPBF),
        "biasb": biasb.astype(np.float32),
        **shared,
    }


def _build_graph():
    nc = bacc.Bacc("TRN2", target_bir_lowering=False, debug=False,
                   num_devices=NCORES)

    xt_d = nc.dram_tensor("xt", [C, N], BF16, kind="ExternalInput").ap()
    wcat_d = nc.dram_tensor("wcat", [C, 768], BF16, kind="ExternalInput").ap()
    cos2_d = nc.dram_tensor("cos2", [128, N], BF16, kind="ExternalInput").ap()
    ssin2_d = nc.dram_tensor("ssin2", [128, N], BF16, kind="ExternalInput").ap()
    shifteye_d = nc.dram_tensor("shifteye", [128, 256], BF16, kind="ExternalInput").ap()
    c8eye_d = nc.dram_tensor("c8eye", [NP_, 128, 128], BF16, kind="ExternalInput").ap()
    master_d = nc.dram_tensor("master", [128, 2048], BF16, kind="ExternalInput").ap()
    projwt_d = nc.dram_tensor("projwt", [256, C], BF16, kind="ExternalInput").ap()
    biasb_d = nc.dram_tensor("biasb", [128, C], F32, kind="ExternalInput").ap()
    out_d = nc.dram_tensor("out", [N, C], BF16, kind="ExternalOutput").ap()

    with tile.TileContext(nc) as tc:
        _body(nc, tc, xt_d, wcat_d, cos2_d, ssin2_d, shifteye_d, c8eye_d,
              master_d, projwt_d, biasb_d, out_d)
    nc.compile()
    return nc


def _body(nc, tc, xt_d, wcat_d, cos2_d, ssin2_d, shifteye_d, c8eye_d,
          master_d, projwt_d, biasb_d, out_d):
    from contextlib import ExitStack
    ctx = ExitStack()
    persist = ctx.enter_context(tc.tile_pool(name="persist", bufs=1))
    rope_pool = ctx.enter_context(tc.tile_pool(name="rope", bufs=2))
    exp_pool = ctx.enter_context(tc.tile_pool(name="exp", bufs=6))
    norm_pool = ctx.enter_context(tc.tile_pool(name="norm", bufs=4))
    fin_pool = ctx.enter_context(tc.tile_pool(name="final", bufs=8))
    pspool = ctx.enter_context(tc.tile_pool(name="ps", bufs=2, space="PSUM"))

    def ptile(shape, dtype, tag):
        return persist.tile(shape, dtype, tag=tag, name=tag)

    Exp = mybir.ActivationFunctionType.Exp

    # ---- persistent SBUF tiles ----
    w_sb = [ptile([128, 768], BF16, f"w{i}") for i in range(4)]
    xt_sb = [ptile([128, N], BF16, f"xt{i}") for i in range(4)]
    cos2 = ptile([128, N], BF16, "cos2")
    ssin2 = ptile([128, N], BF16, "ssin2")
    shifteye = ptile([128, 256], BF16, "shifteye")
    c8eye_sb = [ptile([128, 128], BF16, f"c8e{p}") for p in range(NP_)]
    master = ptile([128, 2048], BF16, "master")
    projw_sb = [ptile([128, C], BF16, f"pw{bd}") for bd in range(2)]
    biasb = ptile([128, C], F32, "biasb")
    q2_sb = [ptile([128, N], BF16, f"q2_{bd}") for bd in range(2)]
    k2_sb = [ptile([128, N], BF16, f"k2_{bd}") for bd in range(2)]
    v_sb = [ptile([128, NP_ * 65], BF16, f"v_{nt}") for nt in range(JT)]
    out_pair = [ptile([128, N], BF16, f"op_{bd}") for bd in range(2)]

    # ---- input DMAs, ordered to feed the PE emission order below ----
    # 1) V weights interleaved with the high xt columns so the first V
    # matmul starts after two transfers (V tiles run jt=15..0)
    for i in range(4):
        nc.sync.dma_start(w_sb[i][:, 512:768],
                          wcat_d[i * 128:(i + 1) * 128, 512:768])
        nc.sync.dma_start(xt_sb[i][:, 1536:2048],
                          xt_d[i * 128:(i + 1) * 128, 1536:2048])
    # 2) q/k weights + rope tables
    for i in range(4):
        nc.sync.dma_start(w_sb[i][:, 0:512], wcat_d[i * 128:(i + 1) * 128, 0:512])
    nc.sync.dma_start(shifteye[:], shifteye_d[:])
    nc.sync.dma_start(cos2[:], cos2_d[:])
    nc.sync.dma_start(ssin2[:], ssin2_d[:])
    # 3) remaining xt (descending), bias tables, proj weights
    for blk in (2, 1, 0):
        for i in range(4):
            nc.sync.dma_start(xt_sb[i][:, blk * 512:(blk + 1) * 512],
                              xt_d[i * 128:(i + 1) * 128, blk * 512:(blk + 1) * 512])
    nc.sync.dma_start(master[:], master_d[:])
    for p in range(NP_):
        nc.sync.dma_start(c8eye_sb[p][:], c8eye_d[p])
    for bd in range(2):
        nc.sync.dma_start(projw_sb[bd][:], projwt_d[bd * 128:(bd + 1) * 128, :])
    nc.sync.dma_start(biasb[:], biasb_d[:])

    # ---- helpers ----
    def v_tile(jt):
        psv = pspool.tile([128, 256], F32, tag="aux", name="psv")
        for ci in range(4):
            nc.tensor.matmul(
                psv[:], xt_sb[ci][:, jt * 128:(jt + 1) * 128],
                w_sb[ci][:, 512:768],
                start=(ci == 0), stop=(ci == 3))
        vdst = v_sb[jt].rearrange("p (h e) -> p h e", e=65)
        nc.vector.tensor_copy(vdst[:, :, 0:64],
                              psv.rearrange("p (h e) -> p h e", e=64))
        nc.gpsimd.memset(vdst[:, :, 64:65], 1.0)

    def qk_chunk_a(bd, kind, ch):
        # projection matmuls + cos/sin products for one 512-token chunk of
        # bundle bd (positions 2bd, 2bd+1), kind 0=q (scaled) 1=k.
        w_off = kind * 256 + bd * 128
        c0 = ch * 512
        ps_q = pspool.tile([128, 512], F32, tag="aux", name="ps_q")
        for ci in range(4):
            nc.tensor.matmul(
                ps_q[:],
                w_sb[ci][:, w_off:w_off + 128],
                xt_sb[ci][:, c0:c0 + 512],
                start=(ci == 0), stop=(ci == 3))
        tc_c = rope_pool.tile([128, 512], BF16, tag="tc", name="tc_c")
        nc.vector.tensor_mul(tc_c[:], ps_q[:], cos2[:, c0:c0 + 512])
        tc_u = rope_pool.tile([128, 512], BF16, tag="tu", name="tc_u")
        nc.vector.tensor_mul(tc_u[:], ps_q[:], ssin2[:, c0:c0 + 512])
        return tc_c, tc_u

    def qk_chunk_b(bd, kind, ch, tc_c, tc_u):
        # combine: dst = tc_c + P32 @ tc_u (partition-XOR-32 via matmul)
        dst_sb = k2_sb[bd] if kind else q2_sb[bd]
        c0 = ch * 512
        ps2 = pspool.tile([128, 512], F32, tag="aux", name="ps2")
        nc.tensor.matmul(ps2[:], shifteye[:, 0:128], tc_c[:],
                         start=True, stop=False)
        nc.tensor.matmul(ps2[:], shifteye[:, 128:256], tc_u[:],
                         start=False, stop=True)
        nc.vector.tensor_copy(dst_sb[:, c0:c0 + 512], ps2[:])

    # software-pipelined chunk list -> closures (B of chunk i rides with
    # A of chunk i+1 so the PE never waits on the DVE products)
    def chunk_closures(chunks):
        state = {}

        def make(i, spec):
            def run():
                if i > 0:
                    pb, pkd, pch = chunks[i - 1]
                    qk_chunk_b(pb, pkd, pch, *state.pop(i - 1))
                if spec is not None:
                    bd, kd, ch = spec
                    state[i] = qk_chunk_a(bd, kd, ch)
            return run

        return [make(i, spec)
                for i, spec in enumerate(list(chunks) + [None])]

    def bundle_chunks(bd):
        return [(bd, 1, 3), (bd, 1, 2), (bd, 1, 1), (bd, 1, 0),
                (bd, 0, 0), (bd, 0, 1), (bd, 0, 2), (bd, 0, 3)]

    # ---- PE pre-phase: V tiles (desc) interleaved with bundle-0 qk ----
    qk0 = chunk_closures(bundle_chunks(0))
    vt = [lambda jt=jt: v_tile(jt) for jt in range(JT - 1, -1, -1)]
    pre = [vt[0], vt[1], qk0[0], vt[2], vt[3], qk0[1], vt[4], vt[5], qk0[2],
           vt[6], vt[7], qk0[3], vt[8], vt[9], qk0[4], vt[10], vt[11],
           qk0[5], vt[12], vt[13], qk0[6], vt[14], vt[15]]
    for f in pre:
        f()

    # fillers: project bundle 1 while streaming the first two bundle-0 heads;
    # chunks 0..7 of the output projection (whose inputs complete with the
    # sp=0 streams) run inside the sp=1 streams, spreading the output DMAs.
    qk1 = chunk_closures(bundle_chunks(1))
    fillers = {(0, 0): qk1[:5], (0, 1): qk1[5:],
               (0, 2): [qk0[7], qk0[8]],
               (1, 1): [lambda chk=chk: proj_chunk(chk) for chk in range(0, 4)],
               (1, 2): [lambda chk=chk: proj_chunk(chk) for chk in range(4, 8)]}

    def proj_chunk(chk):
        # full output projection of one 128-query chunk (own heads' partial)
        ps = pspool.tile([128, 512], F32, tag="aux", name="ps_proj")
        for bd in range(2):
            nc.tensor.matmul(ps[:], out_pair[bd][:, chk * 128:(chk + 1) * 128],
                             projw_sb[bd][:],
                             start=(bd == 0), stop=(bd == 1),
                             skip_group_check=True)
        fin = fin_pool.tile([128, 512], BF16, tag="f", name="fin")
        nc.vector.tensor_add(fin[:], ps[:], biasb[:])
        nc.sync.dma_start(out_d[chk * 128:(chk + 1) * 128, :], fin[:])

    def norm_sub(sp, p, avt, qi):
        # attn-V is [query-part, head-dim]: the softmax denominator is a
        # per-partition scalar -> reciprocal + tensor_scalar multiply, then
        # PE-transpose back to [dim, query] for the projection.
        bd, row = p // 2, (p % 2) * 64
        col = 128 * (qi % 4)
        rec = norm_pool.tile([128, 1], F32, tag="rc", name="rec")
        nc.vector.reciprocal(rec[:], avt[:, col + 64:col + 65])
        stag = norm_pool.tile([128, 64], BF16, tag="st", name="stag")
        nc.vector.tensor_scalar_mul(stag[:], avt[:, col:col + 64], rec[:])
        pst = pspool.tile([64, 128], BF16, tag="aux", name="pst")
        nc.tensor.transpose(pst[:], stag[:], shifteye[:, 0:128])
        nc.vector.tensor_copy(
            out_pair[bd][row:row + 64,
                         1024 * sp + 128 * qi:1024 * sp + 128 * qi + 128],
            pst[:])

    # ---- attention streams: one per (slot-pair sp, position p) ----
    for sp in range(2):
        for p in range(NP_):
            bd, row = p // 2, (p % 2) * 64
            fl = fillers.get((sp, p), [])
            fi = 0
            jts = list(range(JT - 1, JTMIN[p][sp] - 1, -1))
            av01 = (pspool.tile([128, 512], F32, tag="av0", bufs=1, name="av0"),
                    pspool.tile([128, 512], F32, tag="av1", bufs=1, name="av1"))
            # PSUM accumulation state is per-bank on hardware: only read a
            # subtile once ALL subtiles sharing its bank have stopped.
            norm_at = {}
            norm_at.setdefault(STOPJT[p][sp][4], []).extend([4, 5, 6, 7])
            norm_at.setdefault(STOPJT[p][sp][0], []).extend([0, 1, 2, 3])
            pend = []  # [(jt, av-mm closure)], delayed two steps
            for idx, jt in enumerate(jts):
                while fi < len(fl) and (fi + 1) * len(jts) <= idx * len(fl):
                    fl[fi]()
                    fi += 1
                qm0, qm1, ql = (QM[p][sp][0][jt], QM[p][sp][1][jt],
                                QLIM[p][sp][jt])
                br = BIASR[p][sp][jt]
                ps = pspool.tile([128, 1024], F32, tag="sc", name="ps_sc")
                nc.tensor.matmul(
                    ps[:, 0:qm0],
                    k2_sb[bd][row:row + 64, jt * 128:(jt + 1) * 128],
                    q2_sb[bd][row:row + 64, 1024 * sp:1024 * sp + qm0],
                    start=True, stop=(br is None), tile_position=(row, 0),
                    skip_group_check=True)
                if qm1 > 0:
                    nc.tensor.matmul(
                        ps[:, 512:512 + qm1],
                        k2_sb[bd][row:row + 64, jt * 128:(jt + 1) * 128],
                        q2_sb[bd][row:row + 64,
                                  1024 * sp + 512:1024 * sp + 512 + qm1],
                        start=True, stop=(br is None), tile_position=(row, 0),
                        skip_group_check=True)
                if br is not None:
                    o = 1024 * sp - 128 * jt
                    # split at the 512-col PSUM bank boundary
                    for lo, hi in ((br[0], min(br[1], 512)),
                                   (max(br[0], 512), br[1])):
                        if lo < hi:
                            nc.tensor.matmul(
                                ps[:, lo:hi], c8eye_sb[p][:],
                                master[:, lo + o:hi + o],
                                start=False, stop=True, tile_position=(0, 0),
                                skip_group_check=True)
                et = exp_pool.tile([128, 1024], BF16, tag="e", name="et")
                nc.scalar.activation(et[:, 0:ql], ps[:, 0:ql], Exp)

                def av_mms(jt=jt, ql=ql, et=et):
                    for qi in range((ql + 127) // 128):
                        m = min(128, ql - 128 * qi)
                        avt = av01[qi // 4]
                        col = 128 * (qi % 4)
                        nc.tensor.matmul(
                            avt[0:m, col:col + 65],
                            et[:, 128 * qi:128 * qi + m],
                            v_sb[jt][:, p * 65:(p + 1) * 65],
                            start=(jt == JT - 1 and qi % 4 == 0),
                            stop=(jt == STOPJT[p][sp][qi]),
                            skip_group_check=True)

                if len(pend) >= 5:
                    pjt, pfn = pend.pop(0)
                    pfn()
                    for qi in norm_at.get(pjt, []):
                        norm_sub(sp, p, av01[qi // 4], qi)
                pend.append((jt, av_mms))
            for pjt, pfn in pend:
                pfn()
                for qi in norm_at.get(pjt, []):
                    norm_sub(sp, p, av01[qi // 4], qi)
            while fi < len(fl):
                fl[fi]()
                fi += 1

    # ---- remaining output projection chunks (sp=1 queries) ----
    # software-pipelined: each chunk's bundle-0 partial runs during the final
    # norm drain; only the bundle-1 matmul waits on the last normalize.
    def proj_pre(chk):
        ps = pspool.tile([128, 512], F32, tag="aux", name="ps_proj")
        nc.tensor.matmul(ps[:], out_pair[0][:, chk * 128:(chk + 1) * 128],
                         projw_sb[0][:],
                         start=True, stop=False, skip_group_check=True)
        return ps

    def proj_fin(chk, ps):
        nc.tensor.matmul(ps[:], out_pair[1][:, chk * 128:(chk + 1) * 128],
                         projw_sb[1][:],
                         start=False, stop=True, skip_group_check=True)
        fin = fin_pool.tile([128, 512], BF16, tag="f", name="fin")
        nc.vector.tensor_add(fin[:], ps[:], biasb[:])
        nc.sync.dma_start(out_d[chk * 128:(chk + 1) * 128, :], fin[:])

    # slot1 chunks (12..15) are unblocked at the last stream's midpoint;
    # slot0 chunks (8..11) wait for the final normalize, so run them last.
    order = [12, 13, 14, 15, 8, 9, 10, 11]
    pend_ps = {order[0]: proj_pre(order[0]), order[1]: proj_pre(order[1])}
    for i, chk in enumerate(order):
        proj_fin(chk, pend_ps.pop(chk))
        if i + 2 < len(order):
            pend_ps[order[i + 2]] = proj_pre(order[i + 2])

    ctx.close()


@functools.lru_cache(maxsize=1)
def _graph():
    return _build_graph()


def kernel(x, qkv_w, proj_w, proj_b):
    global LAST_RESULT
    x = np.asarray(x, np.float32)
    qkv_w = np.asarray(qkv_w, np.float32)
    proj_w = np.asarray(proj_w, np.float32)
    proj_b = np.asarray(proj_b, np.float32)

    nc = _graph()
    shared, qw, proj = _shared_inputs(qkv_w, proj_w, proj_b)
    in_maps = [_core_inputs(c, x, shared, qw, proj) for c in range(NCORES)]
    res = bass_utils.run_bass_kernel_spmd(nc, in_maps,
                                          core_ids=list(range(NCORES)),
                                          trace=False)
    LAST_RESULT = res
    out = np.zeros((B, N, C), np.float32)
    for b in range(B):
        out[b] = (np.asarray(res.results[2 * b]["out"], np.float32)
                  + np.asarray(res.results[2 * b + 1]["out"], np.float32))
    return out


# revision 119
# speedup vs baseline: 1.1697x; 1.0089x over previous
"""Fused multi-head attention layer (RoPE + ALiBi + softmax + out-proj) on 8 TRN2 cores.

Sharding (v4, heads-split): core c -> (batch b = c//2, head group g = c%2).
Each core computes q/k/v for its 4 heads {g, 2+g, 4+g, 6+g} over ALL 2048
positions, runs banded attention, and projects through its heads' slice of
proj_w, producing a partial [N, C] output; the host sums the two partials
per batch. Pairing adjacent-radius heads per graph position keeps the SPMD
union of ALiBi bands tight, and query blocks have exact (not unioned)
trim bounds.

Pipeline features:
- RoPE rotate-half via a sign-folded sin table + partition-permutation
  matmul (no duplicate rot projections).
- ALiBi bias on PE as c8-scaled identity matmuls against a single shared
  anti-diagonal master pattern table (column-shifted AP views), trimmed to
  the band columns.
- Score/exp/attn-V column ranges prefix-trimmed per (position, block, jt).
- Streams software-pipelined; V/next-bundle projections fill PE gaps.
"""

import functools
import math
import os
import sys

import numpy as np

sys.path.insert(0, "/opt/trn_rl_repo")

import ml_dtypes  # noqa: E402

import concourse.bass as bass  # noqa: E402
import concourse.tile as tile  # noqa: E402
from concourse import bacc, mybir, bass_utils  # noqa: E402

BF16 = mybir.dt.bfloat16
F32 = mybir.dt.float32
NPBF = ml_dtypes.bfloat16

B, N, C, H, D = 4, 2048, 512, 8, 64
NCORES = 8
JT = N // 128        # 16 j-tiles of 128 key positions
NP_ = 4              # head positions per core
T_CUT = 8.0          # ALiBi cutoff in logits: exp(-8) is negligible
SCALE = D ** -0.5

# c8_h = alibi_slope_h * MAX_BIAS = 2^-(h+1) * 8 = 2^(2-h)
C8 = [2.0 ** (2 - h) for h in range(H)]
RADIUS = [T_CUT / c for c in C8]   # band reach (key positions) per head
# graph position p holds heads {2p, 2p+1}; the union band is the odd head's
UR = [RADIUS[2 * p + 1] for p in range(NP_)]

LAST_RESULT = None  # test harness reads exec_time_ns from here


def _clamp(v, lo, hi):
    return max(lo, min(hi, v))


# Frame for (position p, slot-pair sp): cols 0:512 = query block 2sp
# (i0 = 1024sp), cols 512:1024 = block 2sp+1 (i0 = 1024sp+512). Exact bounds.
def _qm(p, sp, sl, jt):
    i0 = 512 * (2 * sp + sl)
    return _clamp(int(math.floor(128 * jt + 127 + UR[p])) + 1 - i0, 0, 512)


QM = [[[[_qm(p, sp, sl, jt) for jt in range(JT)] for sl in range(2)]
       for sp in range(2)] for p in range(NP_)]
QLIM = [[[(QM[p][sp][0][jt] if QM[p][sp][0][jt] < 512
           else 512 + QM[p][sp][1][jt]) for jt in range(JT)]
         for sp in range(2)] for p in range(NP_)]
JTMIN = [[min(jt for jt in range(JT) if QLIM[p][sp][jt] > 0)
          for sp in range(2)] for p in range(NP_)]
# per 128-query subtile qi of the 1024-col frame: the last (smallest) jt
# whose kept prefix still reaches it (attn-V accumulation stop point)
STOPJT = [[[min(jt for jt in range(JT) if QLIM[p][sp][jt] > 128 * qi)
            for qi in range(8)] for sp in range(2)] for p in range(NP_)]


def _bias_range(p, sp, jt):
    qm0, qm1 = QM[p][sp][0][jt], QM[p][sp][1][jt]
    bs0 = max(0, 128 * jt - 1024 * sp + 1)
    bs1 = max(0, 128 * jt - 1024 * sp - 511)
    r = []
    if bs0 < qm0:
        r.append((bs0, qm0))
    if bs1 < qm1:
        r.append((512 + bs1, 512 + qm1))
    if len(r) == 2:
        assert r[0][1] == 512 and r[1][0] == 512, (p, sp, jt, r)
        r = [(r[0][0], r[1][1])]
    return r[0] if r else None


BIASR = [[[_bias_range(p, sp, jt) for jt in range(JT)] for sp in range(2)]
         for p in range(NP_)]


def _heads(g):
    return [2 * p + g for p in range(NP_)]


def _rope_tables():
    inv = 1.0 / (10000.0 ** (np.arange(0, D, 2, dtype=np.float32) / D))
    f = np.arange(N, dtype=np.float32)[:, None] * inv[None, :]
    sin = np.concatenate([np.sin(f), np.sin(f)], axis=-1).astype(np.float32)
    cos = np.concatenate([np.cos(f), np.cos(f)], axis=-1).astype(np.float32)
    return sin, cos  # [N, D]


def _st_table(sin):
    # sign-folded, half-swapped sin table, indexed by SOURCE row r: after the
    # XOR-32 partition permutation, dest row d gets rot_half(q)[d]*sin[d].
    st = np.empty_like(sin)            # [N, D]
    st[:, 0:32] = sin[:, 32:64]
    st[:, 32:64] = -sin[:, 0:32]
    return st


def _shared_inputs(qkv_w, proj_w, proj_b):
    # [I | P32]: P32 is the XOR-32 partition permutation (within 64-blocks)
    shifteye = np.zeros((128, 256), np.float32)
    shifteye[:, 0:128] = np.eye(128)
    for r in range(128):
        shifteye[r, 128 + (r ^ 32)] = 1.0

    # master ALiBi pattern: master[k, x] = min(k - x, 0); the tile for
    # (jt, block i0) is the column window shifted by o = i0 - 128*jt
    jl = np.arange(128, dtype=np.float32)[:, None]
    xl = np.arange(2048, dtype=np.float32)[None, :]
    master = np.minimum(jl - xl, 0.0).astype(NPBF)

    sin, cos = _rope_tables()
    cos2 = np.tile(cos.T, (2, 1)).astype(NPBF)    # [128, N]
    ssin2 = np.tile(_st_table(sin).T, (2, 1)).astype(NPBF)
    return {
        "shifteye": shifteye.astype(NPBF),
        "master": master,
        "cos2": cos2, "ssin2": ssin2,
    }, qkv_w, (proj_w, proj_b)


def _core_inputs(c, x, shared, qkv_w, proj):
    proj_w, proj_b = proj
    b, g = c // 2, c % 2
    heads = _heads(g)
    cols = np.concatenate([np.arange(64 * h, 64 * h + 64) for h in heads])

    wqT = np.ascontiguousarray(qkv_w[0:C].T)[:, cols] * SCALE     # [C, 256]
    wkT = np.ascontiguousarray(qkv_w[C:2 * C].T)[:, cols]
    wvT = np.ascontiguousarray(qkv_w[2 * C:3 * C].T)[:, cols]
    wcat = np.concatenate([wqT, wkT, wvT], axis=1).astype(NPBF)   # [C, 768]

    c8eye = np.zeros((NP_, 128, 128), np.float32)
    for p in range(NP_):
        np.fill_diagonal(c8eye[p], C8[heads[p]])

    projwt = np.ascontiguousarray(proj_w.T)[cols, :]              # [256, C]
    biasb = np.tile(proj_b[None, :], (128, 1)) if g == 0 else \
        np.zeros((128, C), np.float32)

    return {
        "xt": np.ascontiguousarray(x[b].T).astype(NPBF),          # [C, N]
        "wcat": wcat,
        "c8eye": c8eye.astype(NPBF),
        "projwt": projwt.astype(NPBF),
        "biasb": biasb.astype(np.float32),
        **shared,
    }


def _build_graph():
    nc = bacc.Bacc("TRN2", target_bir_lowering=False, debug=False,
                   num_devices=NCORES)

    xt_d = nc.dram_tensor("xt", [C, N], BF16, kind="ExternalInput").ap()
    wcat_d = nc.dram_tensor("wcat", [C, 768], BF16, kind="ExternalInput").ap()
    cos2_d = nc.dram_tensor("cos2", [128, N], BF16, kind="ExternalInput").ap()
    ssin2_d = nc.dram_tensor("ssin2", [128, N], BF16, kind="ExternalInput").ap()
    shifteye_d = nc.dram_tensor("shifteye", [128, 256], BF16, kind="ExternalInput").ap()
    c8eye_d = nc.dram_tensor("c8eye", [NP_, 128, 128], BF16, kind="ExternalInput").ap()
    master_d = nc.dram_tensor("master", [128, 2048], BF16, kind="ExternalInput").ap()
    projwt_d = nc.dram_tensor("projwt", [256, C], BF16, kind="ExternalInput").ap()
    biasb_d = nc.dram_tensor("biasb", [128, C], F32, kind="ExternalInput").ap()
    out_d = nc.dram_tensor("out", [N, C], BF16, kind="ExternalOutput").ap()

    with tile.TileContext(nc) as tc:
        _body(nc, tc, xt_d, wcat_d, cos2_d, ssin2_d, shifteye_d, c8eye_d,
              master_d, projwt_d, biasb_d, out_d)
    nc.compile()
    return nc


def _body(nc, tc, xt_d, wcat_d, cos2_d, ssin2_d, shifteye_d, c8eye_d,
          master_d, projwt_d, biasb_d, out_d):
    from contextlib import ExitStack
    ctx = ExitStack()
    persist = ctx.enter_context(tc.tile_pool(name="persist", bufs=1))
    rope_pool = ctx.enter_context(tc.tile_pool(name="rope", bufs=2))
    exp_pool = ctx.enter_context(tc.tile_pool(name="exp", bufs=6))
    norm_pool = ctx.enter_context(tc.tile_pool(name="norm", bufs=4))
    fin_pool = ctx.enter_context(tc.tile_pool(name="final", bufs=8))
    pspool = ctx.enter_context(tc.tile_pool(name="ps", bufs=2, space="PSUM"))

    def ptile(shape, dtype, tag):
        return persist.tile(shape, dtype, tag=tag, name=tag)

    Exp = mybir.ActivationFunctionType.Exp

    # ---- persistent SBUF tiles ----
    w_sb = [ptile([128, 768], BF16, f"w{i}") for i in range(4)]
    xt_sb = [ptile([128, N], BF16, f"xt{i}") for i in range(4)]
    cos2 = ptile([128, N], BF16, "cos2")
    ssin2 = ptile([128, N], BF16, "ssin2")
    shifteye = ptile([128, 256], BF16, "shifteye")
    c8eye_sb = [ptile([128, 128], BF16, f"c8e{p}") for p in range(NP_)]
    master = ptile([128, 2048], BF16, "master")
    projw_sb = [ptile([128, C], BF16, f"pw{bd}") for bd in range(2)]
    biasb = ptile([128, C], F32, "biasb")
    q2_sb = [ptile([128, N], BF16, f"q2_{bd}") for bd in range(2)]
    k2_sb = [ptile([128, N], BF16, f"k2_{bd}") for bd in range(2)]
    v_sb = [ptile([128, NP_ * 65], BF16, f"v_{nt}") for nt in range(JT)]
    out_pair = [ptile([128, N], BF16, f"op_{bd}") for bd in range(2)]

    # ---- input DMAs, ordered to feed the PE emission order below ----
    # 1) V weights interleaved with the high xt columns so the first V
    # matmul starts after two transfers (V tiles run jt=15..0)
    for i in range(4):
        nc.sync.dma_start(w_sb[i][:, 512:768],
                          wcat_d[i * 128:(i + 1) * 128, 512:768])
        nc.sync.dma_start(xt_sb[i][:, 1536:2048],
                          xt_d[i * 128:(i + 1) * 128, 1536:2048])
    # 2) q/k weights + rope tables
    for i in range(4):
        nc.sync.dma_start(w_sb[i][:, 0:512], wcat_d[i * 128:(i + 1) * 128, 0:512])
    nc.sync.dma_start(shifteye[:], shifteye_d[:])
    nc.sync.dma_start(cos2[:], cos2_d[:])
    nc.sync.dma_start(ssin2[:], ssin2_d[:])
    # 3) remaining xt (descending), bias tables, proj weights
    for blk in (2, 1, 0):
        for i in range(4):
            nc.sync.dma_start(xt_sb[i][:, blk * 512:(blk + 1) * 512],
                              xt_d[i * 128:(i + 1) * 128, blk * 512:(blk + 1) * 512])
    nc.sync.dma_start(master[:], master_d[:])
    for p in range(NP_):
        nc.sync.dma_start(c8eye_sb[p][:], c8eye_d[p])
    for bd in range(2):
        nc.sync.dma_start(projw_sb[bd][:], projwt_d[bd * 128:(bd + 1) * 128, :])
    nc.sync.dma_start(biasb[:], biasb_d[:])

    # ---- helpers ----
    def v_tile(jt):
        psv = pspool.tile([128, 256], F32, tag="aux", name="psv")
        for ci in range(4):
            nc.tensor.matmul(
                psv[:], xt_sb[ci][:, jt * 128:(jt + 1) * 128],
                w_sb[ci][:, 512:768],
                start=(ci == 0), stop=(ci == 3))
        vdst = v_sb[jt].rearrange("p (h e) -> p h e", e=65)
        nc.vector.tensor_copy(vdst[:, :, 0:64],
                              psv.rearrange("p (h e) -> p h e", e=64))
        nc.gpsimd.memset(vdst[:, :, 64:65], 1.0)

    def qk_chunk_a(bd, kind, ch):
        # projection matmuls + cos/sin products for one 512-token chunk of
        # bundle bd (positions 2bd, 2bd+1), kind 0=q (scaled) 1=k.
        w_off = kind * 256 + bd * 128
        c0 = ch * 512
        ps_q = pspool.tile([128, 512], F32, tag="aux", name="ps_q")
        for ci in range(4):
            nc.tensor.matmul(
                ps_q[:],
                w_sb[ci][:, w_off:w_off + 128],
                xt_sb[ci][:, c0:c0 + 512],
                start=(ci == 0), stop=(ci == 3))
        tc_c = rope_pool.tile([128, 512], BF16, tag="tc", name="tc_c")
        nc.vector.tensor_mul(tc_c[:], ps_q[:], cos2[:, c0:c0 + 512])
        tc_u = rope_pool.tile([128, 512], BF16, tag="tu", name="tc_u")
        nc.vector.tensor_mul(tc_u[:], ps_q[:], ssin2[:, c0:c0 + 512])
        return tc_c, tc_u

    def qk_chunk_b(bd, kind, ch, tc_c, tc_u):
        # combine: dst = tc_c + P32 @ tc_u (partition-XOR-32 via matmul)
        dst_sb = k2_sb[bd] if kind else q2_sb[bd]
        c0 = ch * 512
        ps2 = pspool.tile([128, 512], F32, tag="aux", name="ps2")
        nc.tensor.matmul(ps2[:], shifteye[:, 0:128], tc_c[:],
                         start=True, stop=False)
        nc.tensor.matmul(ps2[:], shifteye[:, 128:256], tc_u[:],
                         start=False, stop=True)
        nc.vector.tensor_copy(dst_sb[:, c0:c0 + 512], ps2[:])

    # software-pipelined chunk list -> closures (B of chunk i rides with
    # A of chunk i+1 so the PE never waits on the DVE products)
    def chunk_closures(chunks):
        state = {}

        def make(i, spec):
            def run():
                if i > 0:
                    pb, pkd, pch = chunks[i - 1]
                    qk_chunk_b(pb, pkd, pch, *state.pop(i - 1))
                if spec is not None:
                    bd, kd, ch = spec
                    state[i] = qk_chunk_a(bd, kd, ch)
            return run

        return [make(i, spec)
                for i, spec in enumerate(list(chunks) + [None])]

    def bundle_chunks(bd):
        return [(bd, 1, 3), (bd, 1, 2), (bd, 1, 1), (bd, 1, 0),
                (bd, 0, 0), (bd, 0, 1), (bd, 0, 2), (bd, 0, 3)]

    # ---- PE pre-phase: V tiles (desc) interleaved with bundle-0 qk ----
    qk0 = chunk_closures(bundle_chunks(0))
    vt = [lambda jt=jt: v_tile(jt) for jt in range(JT - 1, -1, -1)]
    pre = [vt[0], vt[1], qk0[0], vt[2], vt[3], qk0[1], vt[4], vt[5], qk0[2],
           vt[6], vt[7], qk0[3], vt[8], vt[9], qk0[4], vt[10], vt[11],
           qk0[5], vt[12], vt[13], qk0[6], vt[14], vt[15]]
    for f in pre:
        f()

    # fillers: project bundle 1 while streaming the first two bundle-0 heads;
    # chunks 0..7 of the output projection (whose inputs complete with the
    # sp=0 streams) run inside the sp=1 streams, spreading the output DMAs.
    qk1 = chunk_closures(bundle_chunks(1))
    fillers = {(0, 0): qk1[:5], (0, 1): qk1[5:],
               (0, 2): [qk0[7], qk0[8]],
               (1, 1): [lambda chk=chk: proj_chunk(chk) for chk in range(0, 4)],
               (1, 2): [lambda chk=chk: proj_chunk(chk) for chk in range(4, 8)]}

    def proj_chunk(chk):
        # full output projection of one 128-query chunk (own heads' partial)
        ps = pspool.tile([128, 512], F32, tag="aux", name="ps_proj")
        for bd in range(2):
            nc.tensor.matmul(ps[:], out_pair[bd][:, chk * 128:(chk + 1) * 128],
                             projw_sb[bd][:],
                             start=(bd == 0), stop=(bd == 1),
                             skip_group_check=True)
        fin = fin_pool.tile([128, 512], BF16, tag="f", name="fin")
        nc.vector.tensor_add(fin[:], ps[:], biasb[:])
        nc.sync.dma_start(out_d[chk * 128:(chk + 1) * 128, :], fin[:])

    def norm_sub(sp, p, avt, qi):
        # attn-V is [query-part, head-dim]: the softmax denominator is a
        # per-partition scalar -> reciprocal + tensor_scalar multiply, then
        # PE-transpose back to [dim, query] for the projection.
        bd, row = p // 2, (p % 2) * 64
        col = 128 * (qi % 4)
        rec = norm_pool.tile([128, 1], F32, tag="rc", name="rec")
        nc.vector.reciprocal(rec[:], avt[:, col + 64:col + 65])
        stag = norm_pool.tile([128, 64], BF16, tag="st", name="stag")
        nc.vector.tensor_scalar_mul(stag[:], avt[:, col:col + 64], rec[:])
        pst = pspool.tile([64, 128], BF16, tag="aux", name="pst")
        nc.tensor.transpose(pst[:], stag[:], shifteye[:, 0:128])
        nc.vector.tensor_copy(
            out_pair[bd][row:row + 64,
                         1024 * sp + 128 * qi:1024 * sp + 128 * qi + 128],
            pst[:])

    # ---- attention streams: one per (slot-pair sp, position p) ----
    for sp in range(2):
        for p in range(NP_):
            bd, row = p // 2, (p % 2) * 64
            fl = fillers.get((sp, p), [])
            fi = 0
            jts = list(range(JT - 1, JTMIN[p][sp] - 1, -1))
            av01 = (pspool.tile([128, 512], F32, tag="av0", bufs=1, name="av0"),
                    pspool.tile([128, 512], F32, tag="av1", bufs=1, name="av1"))
            # PSUM accumulation state is per-bank on hardware: only read a
            # subtile once ALL subtiles sharing its bank have stopped.
            norm_at = {}
            norm_at.setdefault(STOPJT[p][sp][4], []).extend([4, 5, 6, 7])
            norm_at.setdefault(STOPJT[p][sp][0], []).extend([0, 1, 2, 3])
            pend = []  # [(jt, av-mm closure)], delayed two steps
            for idx, jt in enumerate(jts):
                while fi < len(fl) and (fi + 1) * len(jts) <= idx * len(fl):
                    fl[fi]()
                    fi += 1
                qm0, qm1, ql = (QM[p][sp][0][jt], QM[p][sp][1][jt],
                                QLIM[p][sp][jt])
                br = BIASR[p][sp][jt]
                ps = pspool.tile([128, 1024], F32, tag="sc", name="ps_sc")
                nc.tensor.matmul(
                    ps[:, 0:qm0],
                    k2_sb[bd][row:row + 64, jt * 128:(jt + 1) * 128],
                    q2_sb[bd][row:row + 64, 1024 * sp:1024 * sp + qm0],
                    start=True, stop=(br is None), tile_position=(row, 0),
                    skip_group_check=True)
                if qm1 > 0:
                    nc.tensor.matmul(
                        ps[:, 512:512 + qm1],
                        k2_sb[bd][row:row + 64, jt * 128:(jt + 1) * 128],
                        q2_sb[bd][row:row + 64,
                                  1024 * sp + 512:1024 * sp + 512 + qm1],
                        start=True, stop=(br is None), tile_position=(row, 0),
                        skip_group_check=True)
                if br is not None:
                    o = 1024 * sp - 128 * jt
                    # split at the 512-col PSUM bank boundary
                    for lo, hi in ((br[0], min(br[1], 512)),
                                   (max(br[0], 512), br[1])):
                        if lo < hi:
                            nc.tensor.matmul(
                                ps[:, lo:hi], c8eye_sb[p][:],
                                master[:, lo + o:hi + o],
                                start=False, stop=True, tile_position=(0, 0),
                                skip_group_check=True)
                et = exp_pool.tile([128, 1024], BF16, tag="e", name="et")
                nc.scalar.activation(et[:, 0:ql], ps[:, 0:ql], Exp)

                def av_mms(jt=jt, ql=ql, et=et):
                    for qi in range((ql + 127) // 128):
                        m = min(128, ql - 128 * qi)
                        avt = av01[qi // 4]
                        col = 128 * (qi % 4)
                        nc.tensor.matmul(
                            avt[0:m, col:col + 65],
                            et[:, 128 * qi:128 * qi + m],
                            v_sb[jt][:, p * 65:(p + 1) * 65],
                            start=(jt == JT - 1 and qi % 4 == 0),
                            stop=(jt == STOPJT[p][sp][qi]),
                            skip_group_check=True)

                if len(pend) >= 5:
                    pjt, pfn = pend.pop(0)
                    pfn()
                    for qi in norm_at.get(pjt, []):
                        norm_sub(sp, p, av01[qi // 4], qi)
                pend.append((jt, av_mms))
            for pjt, pfn in pend:
                pfn()
                for qi in norm_at.get(pjt, []):
                    norm_sub(sp, p, av01[qi // 4], qi)
            while fi < len(fl):
                fl[fi]()
                fi += 1

    # ---- remaining output projection chunks (sp=1 queries) ----
    # software-pipelined: each chunk's bundle-0 partial runs during the final
    # norm drain; only the bundle-1 matmul waits on the last normalize.
    def proj_pre(chk):
        ps = pspool.tile([128, 512], F32, tag="aux", name="ps_proj")
        nc.tensor.matmul(ps[:], out_pair[0][:, chk * 128:(chk + 1) * 128],
                         projw_sb[0][:],
                         start=True, stop=False, skip_group_check=True)
        return ps

    def proj_fin(chk, ps):
        nc.tensor.matmul(ps[:], out_pair[1][:, chk * 128:(chk + 1) * 128],
                         projw_sb[1][:],
                         start=False, stop=True, skip_group_check=True)
        fin = fin_pool.tile([128, 512], BF16, tag="f", name="fin")
        nc.vector.tensor_add(fin[:], ps[:], biasb[:])
        nc.sync.dma_start(out_d[chk * 128:(chk + 1) * 128, :], fin[:])

    # slot1 chunks (12..15) are unblocked at the last stream's midpoint;
    # slot0 chunks (8..11) wait for the final normalize, so run them last.
    order = [12, 13, 14, 15, 8, 9, 10, 11]
    pend_ps = {order[0]: proj_pre(order[0]), order[1]: proj_pre(order[1])}
    for i, chk in enumerate(order):
        proj_fin(chk, pend_ps.pop(chk))
        if i + 2 < len(order):
            pend_ps[order[i + 2]] = proj_pre(order[i + 2])

    ctx.close()


@functools.lru_cache(maxsize=1)
def _graph():
    return _build_graph()


def kernel(x, qkv_w, proj_w, proj_b):
    global LAST_RESULT
    x = np.asarray(x, np.float32)
    qkv_w = np.asarray(qkv_w, np.float32)
    proj_w = np.asarray(proj_w, np.float32)
    proj_b = np.asarray(proj_b, np.float32)

    nc = _graph()
    shared, qw, proj = _shared_inputs(qkv_w, proj_w, proj_b)
    in_maps = [_core_inputs(c, x, shared, qw, proj) for c in range(NCORES)]
    res = bass_utils.run_bass_kernel_spmd(nc, in_maps,
                                          core_ids=list(range(NCORES)),
                                          trace=False)
    LAST_RESULT = res
    out = np.zeros((B, N, C), np.float32)
    for b in range(B):
        out[b] = (np.asarray(res.results[2 * b]["out"], np.float32)
                  + np.asarray(res.results[2 * b + 1]["out"], np.float32))
    return out
